# revision 1
# baseline (speedup 1.0000x reference)
"""Trainium2 Bass kernel for nn_ABL_SPARSE_87694642250045 (GMN graph matching).

Data-parallel over B=64 graph pairs: 8 pairs (16 graphs) per NeuronCore, 8 cores.
No collectives — output is per-pair scalars, concatenated host-side.

Device decomposition (per core):
  - gathers/segment-sums as one-hot matmuls (one-hots precomputed host-side)
  - message MLP with W-swap trick: both directions in one [*,512] hidden
  - residual update folded into (W_upd_a + I)
  - node sinkhorn in log space (PE transposes for column steps)
  - edge sinkhorn multiplicative (column sums via ones-matmul, no transposes)
  - L1 cdists: tensor_scalar |a-b| with d-on-partitions + ones-matmul reduce
    (strips) + tile_position rotation + SBUF DMA reshape
"""
import numpy as np

NCORE = 8
B, N, E = 64, 32, 96
NPROP, TEMP, ITERS = 5, 0.1, 20
BL = B // NCORE          # 8 pairs / core
GL = 2 * BL              # 16 graphs / core
VL = GL * N              # 512 nodes / core
EL = GL * E              # 1536 edges / core

_CACHE = {}


# ----------------------------------------------------------------- host prep
def _onehot(idx, n):
    out = np.zeros((len(idx), n), np.float32)
    out[np.arange(len(idx)), idx] = 1.0
    return out


def _host_prep(inputs):
    f32 = np.float32
    nf = np.asarray(inputs["node_features"], f32)
    ef = np.asarray(inputs["edge_features"], f32)
    fr_all = np.asarray(inputs["from_idx"]).astype(np.int64)
    to_all = np.asarray(inputs["to_idx"]).astype(np.int64)

    W_enc = np.asarray(inputs["W_enc"], f32); b_enc = np.asarray(inputs["b_enc"], f32)
    W1 = np.asarray(inputs["W_msg1"], f32); b1 = np.asarray(inputs["b_msg1"], f32)
    W2 = np.asarray(inputs["W_msg2"], f32); b2 = np.asarray(inputs["b_msg2"], f32)
    Wu = np.asarray(inputs["W_upd"], f32); bu = np.asarray(inputs["b_upd"], f32)
    Wsk1 = np.asarray(inputs["W_sk1"], f32); bsk1 = np.asarray(inputs["b_sk1"], f32)
    Wsk2 = np.asarray(inputs["W_sk2"], f32); bsk2 = np.asarray(inputs["b_sk2"], f32)
    Wl1 = np.asarray(inputs["W_lrl1"], f32); bl1 = np.asarray(inputs["b_lrl1"], f32)
    Wl2 = np.asarray(inputs["W_lrl2"], f32); bl2 = np.asarray(inputs["b_lrl2"], f32)

    def ext(Wm, bm):
        Wswap = np.concatenate([Wm[128:256], Wm[0:128], Wm[256:257]], axis=0)
        Wcat = np.concatenate([Wm, Wswap], axis=1)               # [257,512]
        bcat = np.concatenate([bm, bm])[None]                    # [1,512]
        return np.ascontiguousarray(np.concatenate([Wcat, bcat], axis=0))  # [258,512]

    W1ext = ext(W1, b1)
    Wl1ext = ext(Wl1, bl1)

    shared = {
        "w1a": W1ext[0:128], "w1b": W1ext[128:256], "w1c": W1ext[256:258],
        "wl1a": Wl1ext[0:128], "wl1b": Wl1ext[128:256], "wl1c": Wl1ext[256:258],
        "w2a": W2[0:128], "w2b": W2[128:256],
        "wl2a": Wl2[0:128], "wl2b": Wl2[128:256],
        "b2r": b2[None], "bl2r": (2.0 * bl2)[None],
        "wuaI": Wu[0:128] + np.eye(128, dtype=f32),
        "wub_a": Wu[128:256], "wub_b": Wu[256:384], "bur": bu[None],
        "wenc": W_enc, "bencr": b_enc[None],
        "wsk1": Wsk1, "bsk1r": bsk1[None], "wsk2": Wsk2, "bsk2r": bsk2[None],
    }
    shared = {k: np.ascontiguousarray(v, f32) for k, v in shared.items()}

    in_maps = []
    for c in range(NCORE):
        nfc = nf[c*VL:(c+1)*VL]                                  # [512,32]
        efc = ef[c*EL:(c+1)*EL]                                  # [1536,1]
        fr = fr_all[c*EL:(c+1)*EL] - c*VL
        to = to_all[c*EL:(c+1)*EL] - c*VL

        gfT = np.zeros((128, EL), f32)
        gtT = np.zeros((128, EL), f32)
        for g in range(4):
            e0, v0 = 384*g, 128*g
            gfT[:, e0:e0+384] = _onehot(fr[e0:e0+384] - v0, 128).T
            gtT[:, e0:e0+384] = _onehot(to[e0:e0+384] - v0, 128).T

        sT = np.zeros((128, 12*128), f32)
        sF = np.zeros((128, 12*128), f32)
        for kt in range(12):
            e0, g = 128*kt, kt // 3
            sT[:, 128*kt:128*(kt+1)] = _onehot(to[e0:e0+128] - 128*g, 128)
            sF[:, 128*kt:128*(kt+1)] = _onehot(fr[e0:e0+128] - 128*g, 128)

        frg = fr.reshape(GL, E) - (np.arange(GL) * N)[:, None]
        tog = to.reshape(GL, E) - (np.arange(GL) * N)[:, None]
        kfq = np.zeros((32, BL*E), f32); ktq = np.zeros((32, BL*E), f32)
        kfc = np.zeros((32, BL*E), f32); ktc = np.zeros((32, BL*E), f32)
        for p in range(BL):
            s = slice(E*p, E*(p+1))
            kfq[:, s] = _onehot(frg[2*p], N).T
            ktq[:, s] = _onehot(tog[2*p], N).T
            kfc[:, s] = _onehot(frg[2*p+1], N).T
            ktc[:, s] = _onehot(tog[2*p+1], N).T

        e1 = np.concatenate([efc.T, np.ones((1, EL), f32)], axis=0)  # [2,1536]

        m = dict(shared)
        m.update({
            "nfT": np.ascontiguousarray(nfc.T),      # [32,512]
            "e1": np.ascontiguousarray(e1),
            "gfT": gfT, "gtT": gtT, "sT": sT, "sF": sF,
            "kfq": kfq, "ktq": ktq, "kfc": kfc, "ktc": ktc,
        })
        in_maps.append(m)
    return in_maps


# --------------------------------------------------------------- bass builder
def _build(debug=False):
    import concourse.bass as bass
    import concourse.bacc as bacc
    import concourse.mybir as mybir
    import concourse.tile as tile
    from concourse.masks import make_identity

    f32 = mybir.dt.float32
    bf16 = mybir.dt.bfloat16
    f32r = mybir.dt.float32r
    Alu = mybir.AluOpType
    Act = mybir.ActivationFunctionType
    AX = mybir.AxisListType

    nc = bacc.Bacc("TRN2", target_bir_lowering=False)

    # ---- dram declarations
    dr = {}
    decls = {
        "nfT": (32, VL), "e1": (2, EL), "gfT": (128, EL), "gtT": (128, EL),
        "sT": (128, 12*128), "sF": (128, 12*128),
        "kfq": (32, BL*E), "ktq": (32, BL*E), "kfc": (32, BL*E), "ktc": (32, BL*E),
        "w1a": (128, 512), "w1b": (128, 512), "w1c": (2, 512),
        "wl1a": (128, 512), "wl1b": (128, 512), "wl1c": (2, 512),
        "w2a": (128, 256), "w2b": (128, 256), "wl2a": (128, 256), "wl2b": (128, 256),
        "b2r": (1, 256), "bl2r": (1, 256),
        "wuaI": (128, 128), "wub_a": (128, 128), "wub_b": (128, 128), "bur": (1, 128),
        "wenc": (32, 128), "bencr": (1, 128),
        "wsk1": (128, 32), "bsk1r": (1, 32), "wsk2": (32, 32), "bsk2r": (1, 32),
    }
    f32r_names = set(['wl1a', 'wl1b', 'wl1c', 'wl2a', 'wl2b', 'bl2r'])
    for k, shp in decls.items():
        dt_ = f32r if k in f32r_names else f32
        dr[k] = nc.declare_dram_parameter(k, list(shp), dt_, isOutput=False)
    out_ext = nc.declare_dram_parameter("out", [1, BL], f32, isOutput=True)
    dbg = {}
    if debug:
        for k, shp in {
            "dbg_hT0": (128, 512), "dbg_hT": (128, 512), "dbg_tqT": (32, 256),
            "dbg_tcT": (32, 256), "dbg_cost": (32, 256), "dbg_nplan": (32, 256),
            "dbg_M0": (96, 768), "dbg_eplan": (96, 768), "dbg_D": (96, 768),
            "dbg_ncd": (32, 256),
        }.items():
            dbg[k] = nc.declare_dram_parameter(k, list(shp), f32, isOutput=True)

    with tile.TileContext(nc) as tc:
        _emit(nc, tc, dr, out_ext, dbg, f32, bf16, f32r, Alu, Act, AX, make_identity)
    nc.compile()
    return nc


def _emit(nc, tc, dr, out_ext, dbg, f32, bf16, f32r, Alu, Act, AX, make_identity):
    import concourse.bass as bass
    from contextlib import ExitStack

    ctx = ExitStack()
    const = ctx.enter_context(tc.tile_pool(name="const", bufs=1))
    persist = ctx.enter_context(tc.tile_pool(name="persist", bufs=1))
    wrk = ctx.enter_context(tc.tile_pool(name="wrk", bufs=1))
    hpool = ctx.enter_context(tc.tile_pool(name="hpool", bufs=2))
    hidp = ctx.enter_context(tc.tile_pool(name="hidp", bufs=1))
    stg = ctx.enter_context(tc.tile_pool(name="stg", bufs=3))
    stg2 = ctx.enter_context(tc.tile_pool(name="stg2", bufs=2))
    stg1 = ctx.enter_context(tc.tile_pool(name="stg1", bufs=1))
    pbig = ctx.enter_context(tc.tile_pool(name="pbig", bufs=3, space="PSUM"))
    pmsg = pbig
    pagg = ctx.enter_context(tc.tile_pool(name="pagg", bufs=1, space="PSUM"))
    psm = ctx.enter_context(tc.tile_pool(name="psm", bufs=3, space="PSUM"))

    def mm(out, lhsT, rhs, start, stop, dt=None, tile_position=None):
        if dt is not None:
            lhsT = lhsT.bitcast(dt)
            rhs = rhs.bitcast(dt)
        nc.tensor.matmul(out, lhsT, rhs, start=start, stop=stop,
                         tile_position=tile_position)

    def bcast_in(ap, n):
        # [P, F] -> [P, F, n] with stride-0 inner free dim
        a = ap
        return bass.AP(tensor=a.tensor, offset=a.offset,
                       ap=list(a.ap) + [[0, n]])

    # ---------- constants to SBUF
    cs = {}
    for k, shp in {
        "nfT": (32, VL), "e1": (2, EL), "gfT": (128, EL), "gtT": (128, EL),
        "sT": (128, 12*128), "sF": (128, 12*128),
        "kfq": (32, BL*E), "ktq": (32, BL*E), "kfc": (32, BL*E), "ktc": (32, BL*E),
        "w1a": (128, 512), "w1b": (128, 512), "w1c": (2, 512),
        "wl1a": (128, 512), "wl1b": (128, 512), "wl1c": (2, 512),
        "w2a": (128, 256), "w2b": (128, 256), "wl2a": (128, 256), "wl2b": (128, 256),
        "b2r": (1, 256), "bl2r": (1, 256),
        "wuaI": (128, 128), "wub_a": (128, 128), "wub_b": (128, 128), "bur": (1, 128),
        "wenc": (32, 128), "bencr": (1, 128),
        "wsk1": (128, 32), "bsk1r": (1, 32), "wsk2": (32, 32), "bsk2r": (1, 32),
    }.items():
        dt_ = f32r if k in ['wl1a', 'wl1b', 'wl1c', 'wl2a', 'wl2b', 'bl2r'] else f32
        t = const.tile(list(shp), dt_, tag=k)
        nc.sync.dma_start(out=t[:], in_=dr[k][:])
        cs[k] = t

    e1r = const.tile([2, EL], f32r, tag="e1r")
    nc.vector.tensor_copy(e1r[:], cs["e1"][:])
    identf = const.tile([128, 128], f32, tag="identf")
    make_identity(nc, identf[:])
    ones96sq = const.tile([96, 96], f32, tag="ones96sq")
    nc.vector.memset(ones96sq[:], 1.0)
    identr = const.tile([128, 128], f32r, tag="identr")
    nc.vector.tensor_copy(identr[:], identf[:])
    ones1f = const.tile([1, 512], f32, tag="ones1f")
    nc.vector.memset(ones1f[:], 1.0)
    ones1 = const.tile([1, 512], f32r, tag="ones1")
    nc.vector.tensor_copy(ones1[:], ones1f[:])
    ones128f = const.tile([128, 1], f32, tag="ones128f")
    nc.vector.memset(ones128f[:], 1.0)
    ones128r = const.tile([128, 1], f32r, tag="ones128r")
    nc.vector.tensor_copy(ones128r[:], ones128f[:])
    ones128b = const.tile([128, 1], bf16, tag="ones128b")
    nc.vector.memset(ones128b[:], 1.0)
    ones32 = const.tile([32, 1], f32r, tag="ones32")
    nc.vector.tensor_copy(ones32[:], ones128f[:32, :])
    ones32f = const.tile([32, 1], f32, tag="ones32f")
    nc.vector.memset(ones32f[:], 1.0)

    # ---------- phase 1: encoder -> hT [128,512], hrm [128,(4g,128f)]
    hT = persist.tile([128, 512], f32, tag="hT")
    hrm = persist.tile([128, 512], f32, tag="hrm")

    ps = pbig.tile([128, 512], f32, tag="pa")
    mm(ps[:], cs["wenc"][:], cs["nfT"][:], start=True, stop=False)
    mm(ps[:], cs["bencr"][:], ones1f[:], start=False, stop=True)
    nc.scalar.activation(out=hT[:], in_=ps[:], func=Act.Copy)
    for g in range(4):
        psg = psm.tile([128, 128], f32, tag="ps_s")
        mm(psg[:], cs["nfT"][:, 128*g:128*(g+1)], cs["wenc"][:],
           start=True, stop=False)
        mm(psg[:], ones1f[:1, :128], cs["bencr"][:], start=False, stop=True)
        nc.vector.tensor_copy(hrm[:, 128*g:128*(g+1)], psg[:])
    if dbg:
        nc.sync.dma_start(out=dbg["dbg_hT0"][:], in_=hT[:])

    # ---------- phase 2: propagation steps
    def message_layer(hrm_t, wa, wb, wc2, w2_a, w2_b, b2row, lrl):
        """gathers + L1; returns hid tile [128, 4*1536] (mtile m at cols 1536m)"""
        mdt = f32r if lrl else None
        tdt = f32r if lrl else f32
        e1t = e1r if lrl else cs["e1"]
        srcT = wrk.tile([128, EL], tdt, tag="srcT")
        dstT = wrk.tile([128, EL], tdt, tag="dstT")
        for g in range(4):
            psrc = pmsg.tile([128, 384], f32, tag="pa")
            pdst = pmsg.tile([128, 384], f32, tag="pa")
            hg = hrm_t[:, 128*g:128*(g+1)]
            mm(psrc[:], hg, cs["gfT"][:, 384*g:384*(g+1)], start=True, stop=True)
            mm(pdst[:], hg, cs["gtT"][:, 384*g:384*(g+1)], start=True, stop=True)
            nc.scalar.activation(out=srcT[:, 384*g:384*(g+1)], in_=psrc[:],
                                 func=Act.Copy)
            nc.scalar.activation(out=dstT[:, 384*g:384*(g+1)], in_=pdst[:],
                                 func=Act.Copy)
        hid = hidp.tile([128, 4*EL], tdt, tag="hid")
        for m in range(4):
            for n in range(3):
                ph = pbig.tile([128, 512], f32, tag="pa")
                ns = slice(512*n, 512*(n+1))
                mm(ph[:], wa[:, 128*m:128*(m+1)], srcT[:, ns], True, False, dt=mdt)
                mm(ph[:], wb[:, 128*m:128*(m+1)], dstT[:, ns], False, False, dt=mdt)
                mm(ph[:], wc2[:, 128*m:128*(m+1)], e1t[:, ns], False, True, dt=mdt)
                dst_ap = hid[:, EL*m + 512*n: EL*m + 512*(n+1)]
                nc.scalar.activation(out=dst_ap, in_=ph[:], func=Act.Relu)
        return hid

    for step in range(NPROP):
        hid = message_layer(hrm, cs["w1a"], cs["w1b"], cs["w1c"],
                            cs["w2a"], cs["w2b"], cs["b2r"], lrl=False)
        # L2 row-major per edge block + wide scatter
        paggT0 = pagg.tile([128, 512], f32, tag="ps_agg0")
        paggT1 = pagg.tile([128, 512], f32, tag="ps_agg1")
        for eb in range(12):
            pmf = pmsg.tile([128, 256], f32, tag="pa")
            pmb = pmsg.tile([128, 256], f32, tag="pa")
            ebs = slice(128*eb, 128*(eb+1))
            mm(pmf[:], hid[:, EL*0 + 128*eb: EL*0 + 128*(eb+1)], cs["w2a"][:],
               True, False)
            mm(pmf[:], hid[:, EL*1 + 128*eb: EL*1 + 128*(eb+1)], cs["w2b"][:],
               False, False)
            mm(pmf[:], ones1f[:1, :128], cs["b2r"][:], False, True)
            mm(pmb[:], hid[:, EL*2 + 128*eb: EL*2 + 128*(eb+1)], cs["w2a"][:],
               True, False)
            mm(pmb[:], hid[:, EL*3 + 128*eb: EL*3 + 128*(eb+1)], cs["w2b"][:],
               False, False)
            mm(pmb[:], ones1f[:1, :128], cs["b2r"][:], False, True)
            mf = stg.tile([128, 256], f32, tag="mf")
            mb = stg.tile([128, 256], f32, tag="mb")
            nc.scalar.activation(out=mf[:], in_=pmf[:], func=Act.Copy)
            nc.scalar.activation(out=mb[:], in_=pmb[:], func=Act.Copy)
            kts = slice(128*eb, 128*(eb+1))
            g = eb // 3
            gs = slice(128*g, 128*(g+1))
            first = (eb % 3 == 0)
            last = (eb % 3 == 2)
            mm(paggT0[:, gs], mf[:, 0:128], cs["sT"][:, kts], first, False)
            mm(paggT0[:, gs], mb[:, 0:128], cs["sF"][:, kts], False, last)
            mm(paggT1[:, gs], mf[:, 128:256], cs["sT"][:, kts], first, False)
            mm(paggT1[:, gs], mb[:, 128:256], cs["sF"][:, kts], False, last)
        aggT0 = hpool.tile([128, 512], f32, tag="aggT0")
        aggT1 = hpool.tile([128, 512], f32, tag="aggT1")
        nc.scalar.activation(out=aggT0[:], in_=paggT0[:], func=Act.Copy)
        nc.scalar.activation(out=aggT1[:], in_=paggT1[:], func=Act.Copy)
        # update
        pnew = pbig.tile([128, 512], f32, tag="pa")
        mm(pnew[:], cs["wuaI"][:], hT[:], True, False)
        mm(pnew[:], cs["wub_a"][:], aggT0[:], False, False)
        mm(pnew[:], cs["wub_b"][:], aggT1[:], False, False)
        mm(pnew[:], cs["bur"][:], ones1f[:], False, True)
        hT_new = hpool.tile([128, 512], f32, tag="hTn")
        nc.scalar.activation(out=hT_new[:], in_=pnew[:], func=Act.Copy)
        hrm_new = hpool.tile([128, 512], f32, tag="hrmn")
        for g in range(4):
            pt = psm.tile([128, 128], f32, tag="ps_s")
            nc.tensor.transpose(pt[:], hT_new[:, 128*g:128*(g+1)], identf[:])
            nc.scalar.activation(out=hrm_new[:, 128*g:128*(g+1)], in_=pt[:],
                                 func=Act.Copy)
        hT, hrm = hT_new, hrm_new
    if dbg:
        nc.sync.dma_start(out=dbg["dbg_hT"][:], in_=hT[:])

    # ---------- phase 3: sk path (tqT/tcT [32, (8p,32n)])
    def h_cols(par):  # par=0 query, 1 corpus -> [128, (8p, 32n)] AP view
        v = hT[:].rearrange("p (g x n) -> p g x n", x=2, n=32)
        return v[:, :, par, :]

    tqT = persist.tile([32, 256], f32, tag="tqT")
    tcT = persist.tile([32, 256], f32, tag="tcT")
    for par, dst in ((0, tqT), (1, tcT)):
        p1 = psm.tile([32, 256], f32, tag="ps_s")
        mm(p1[:], cs["wsk1"][:], h_cols(par), True, False)
        mm(p1[:], cs["bsk1r"][:], ones1f[:1, :256], False, True)
        s1 = stg.tile([32, 256], f32, tag="sk_s1")
        nc.scalar.activation(out=s1[:], in_=p1[:], func=Act.Relu)
        p2 = psm.tile([32, 256], f32, tag="ps_s")
        mm(p2[:], cs["wsk2"][:], s1[:], True, False)
        mm(p2[:], cs["bsk2r"][:], ones1f[:1, :256], False, True)
        nc.vector.tensor_copy(dst[:], p2[:])
    if dbg:
        nc.sync.dma_start(out=dbg["dbg_tqT"][:], in_=tqT[:])
        nc.sync.dma_start(out=dbg["dbg_tcT"][:], in_=tcT[:])

    # ---------- phase 4: node cost [32, (8p,32j)] via strips
    cost = persist.tile([32, 256], f32, tag="cost")

    def strip_cdist(out_tile, blk_of_ph, cols_of_ph, dpart, blk, nacc, tag, dt_, defer=False):
        """out_tile[i, blk*p+j] = sum_d |blk_of_ph(p,hh)[d,j] - cols_of_ph(p,hh)[d,i]|.

        Wide-batched: per (p, hh, 16-i chunk): one TT subtract + one STT abs over
        [dpart, 16*blk], then 4 strip matmuls (tile_position rotation c=0..3,
        4 i-strips each) reduce over d into PSUM rows {0,32,64,96}; evacuate
        via full-tile copy + strided SBUF->SBUF DMA.
        """
        ones_l = ones128b if dt_ == bf16 else (ones32f if dpart == 32 else ones128f)
        units = []
        for p in range(BL):
            for ib in range(6 if blk == 96 else 2):
                units.append((p, ib))
        closures = []
        def make_unit(p, ib):
            def unit():
                pstr = psm.tile([128, 4 * blk], f32, tag="ps_s")
                st0 = stg2.tile([dpart, 16 * blk], dt_, tag=tag + "_s0")
                st1 = None
                srcs = [st0]
                if nacc == 2:
                    st1 = stg2.tile([dpart, 16 * blk], dt_, tag=tag + "_s1")
                    srcs.append(st1)
                for hh in range(nacc):
                    stt_t = srcs[hh]
                    blk_ap = blk_of_ph(p, hh)
                    cols_ap = cols_of_ph(p, hh, 16 * ib, 16)
                    in0 = bass.AP(tensor=blk_ap.tensor, offset=blk_ap.offset,
                                  ap=[blk_ap.ap[0], [0, 16]] + list(blk_ap.ap[1:]))
                    in1 = bass.AP(tensor=cols_ap.tensor, offset=cols_ap.offset,
                                  ap=list(cols_ap.ap) + [[0, blk]])
                    v3 = stt_t[:].rearrange("p (i j) -> p i j", j=blk)
                    nc.vector.tensor_tensor(out=v3, in0=in0, in1=in1,
                                            op=Alu.subtract)
                    nc.vector.scalar_tensor_tensor(
                        out=v3, in0=v3, scalar=-1.0, in1=v3,
                        op0=Alu.mult, op1=Alu.max)
                for c in range(4):
                    cs_ = slice(4 * blk * c, 4 * blk * (c + 1))
                    mm(pstr[32*c:32*c+1, :], ones_l[:], st0[:, cs_],
                       True, nacc == 1, tile_position=(0, 32*c))
                    if nacc == 2:
                        mm(pstr[32*c:32*c+1, :], ones_l[:], st1[:, cs_],
                           False, True, tile_position=(0, 32*c))
                s2 = stg2.tile([128, 4 * blk], f32, tag=tag + "_s2")
                nc.vector.tensor_copy(s2[:], pstr[:])
                sv = s2[:]
                iv = bass.AP(tensor=sv.tensor, offset=sv.offset,
                             ap=[[32 * sv.ap[0][0], 4], [blk, 4], [1, blk]])
                nc.sync.dma_start(
                    out=out_tile[16*ib:16*(ib+1), blk*p:blk*(p+1)], in_=iv)
            return unit
        for (p, ib) in units:
            closures.append(make_unit(p, ib))
        if defer:
            return closures
        for cl in closures:
            cl()

    strip_cdist(cost,
                blk_of_ph=lambda p, hh: tcT[:, 32*p:32*(p+1)],
                cols_of_ph=lambda p, hh, i0, ni: tqT[:, 32*p+i0:32*p+i0+ni],
                dpart=32, blk=32, nacc=1, tag="nc", dt_=f32)
    if dbg:
        nc.sync.dma_start(out=dbg["dbg_cost"][:], in_=cost[:])

    # ---------- phase 8/9: lrl embeddings + edge cdist D [96, (8p,96j)]
    hidL = message_layer(hrm, cs["wl1a"], cs["wl1b"], cs["wl1c"],
                         cs["wl2a"], cs["wl2b"], cs["bl2r"], lrl=True)
    bid0b = persist.tile([128, EL], bf16, tag="bid0b")
    bid1b = persist.tile([128, EL], bf16, tag="bid1b")
    for mt, dst in ((0, bid0b), (1, bid1b)):
        for n in range(3):
            pb2 = pbig.tile([128, 512], f32, tag="pa")
            ns = slice(512*n, 512*(n+1))
            mm(pb2[:], cs["wl2a"][:, 128*mt:128*(mt+1)],
               hidL[:, EL*0 + 512*n: EL*0 + 512*(n+1)], True, False, dt=f32r)
            mm(pb2[:], cs["wl2b"][:, 128*mt:128*(mt+1)],
               hidL[:, EL*1 + 512*n: EL*1 + 512*(n+1)], False, False, dt=f32r)
            mm(pb2[:], cs["wl2a"][:, 128*mt:128*(mt+1)],
               hidL[:, EL*2 + 512*n: EL*2 + 512*(n+1)], False, False, dt=f32r)
            mm(pb2[:], cs["wl2b"][:, 128*mt:128*(mt+1)],
               hidL[:, EL*3 + 512*n: EL*3 + 512*(n+1)], False, False, dt=f32r)
            mm(pb2[:], cs["bl2r"][:, 128*mt:128*(mt+1)], ones1[:], False, True)
            nc.scalar.activation(out=dst[:, ns], in_=pb2[:], func=Act.Copy)

    # ---------- phase 5: node sinkhorn, log space
    la = persist.tile([32, 256], f32, tag="la")
    nc.vector.tensor_scalar(out=la[:], in0=cost[:], scalar1=float(-1.0/TEMP),
                            scalar2=None, op0=Alu.mult)

    def ns_norm_step(t):
        """log-space normalize along each 32-wide free block of t [32, 256]."""
        t3 = t[:].rearrange("p (b j) -> p b j", j=32)
        rm = stg.tile([32, 8], f32, tag="ns_rm")
        nc.vector.tensor_reduce(out=rm[:], in_=t3, axis=AX.X, op=Alu.max,
                                negate=True)
        tmp = stg.tile([32, 256], f32, tag="ns_tmp")
        nc.vector.scalar_tensor_tensor(
            out=tmp[:].rearrange("p (b j) -> p b j", j=32), in0=t3, scalar=1.0,
            in1=bcast_in(rm[:], 32), op0=Alu.mult, op1=Alu.add)
        ex = stg.tile([32, 256], f32, tag="ns_ex")
        nc.scalar.activation(out=ex[:], in_=tmp[:], func=Act.Exp)
        sm = stg.tile([32, 8], f32, tag="ns_sm")
        nc.vector.tensor_reduce(out=sm[:], in_=ex[:].rearrange(
            "p (b j) -> p b j", j=32), axis=AX.X, op=Alu.add)
        ls = stg.tile([32, 8], f32, tag="ns_ls")
        nc.scalar.activation(out=ls[:], in_=sm[:], func=Act.Ln)
        lse = stg.tile([32, 8], f32, tag="ns_lse")
        nc.vector.tensor_tensor(out=lse[:], in0=ls[:], in1=rm[:], op=Alu.subtract)
        nc.vector.scalar_tensor_tensor(
            out=t3, in0=t3, scalar=1.0,
            in1=bcast_in(lse[:], 32), op0=Alu.mult, op1=Alu.subtract)

    ncd = persist.tile([32, 256], f32, tag="ncd")
    hTb = persist.tile([128, 512], bf16, tag="hTb")
    nc.vector.tensor_copy(hTb[:], hT[:])
    na_units = strip_cdist(ncd,
                blk_of_ph=lambda p, hh: hTb[:, 64*p+32:64*p+64],
                cols_of_ph=lambda p, hh, i0, ni: hTb[:, 64*p+i0:64*p+i0+ni],
                dpart=128, blk=32, nacc=1, tag="na", dt_=bf16, defer=True)

    lat = persist.tile([32, 256], f32, tag="lat")
    for it in range(ITERS):
        ns_norm_step(la)                      # row step
        nc.vector.transpose(lat[:], la[:])    # per-pair 32x32 block transpose
        if na_units:
            na_units.pop(0)()
        ns_norm_step(lat)                     # col step (rows of transposed)
        nc.vector.transpose(la[:], lat[:])
    while na_units:
        na_units.pop(0)()
    nplan = persist.tile([32, 256], f32, tag="nplan")
    nc.scalar.activation(out=nplan[:], in_=la[:], func=Act.Exp)
    if dbg:
        nc.sync.dma_start(out=dbg["dbg_nplan"][:], in_=nplan[:])

    # ---------- phase 6: kron -> M0 edge [96, (8p,96j)]
    Me = persist.tile([96, 768], f32, tag="Me")
    for p in range(BL):
        Pp = nplan[:, 32*p:32*(p+1)]
        put = psm.tile([32, 96], f32, tag="ps_s")
        pvt = psm.tile([32, 96], f32, tag="ps_s")
        mm(put[:], Pp, cs["kfq"][:, 96*p:96*(p+1)], True, True)
        mm(pvt[:], Pp, cs["ktq"][:, 96*p:96*(p+1)], True, True)
        ut = stg.tile([32, 96], f32, tag="kr_ut")
        vt = stg.tile([32, 96], f32, tag="kr_vt")
        nc.vector.tensor_copy(ut[:], put[:])
        nc.vector.tensor_copy(vt[:], pvt[:])
        pA = psm.tile([96, 96], f32, tag="ps_s")
        pB = psm.tile([96, 96], f32, tag="ps_s")
        mm(pA[:], ut[:], cs["kfc"][:, 96*p:96*(p+1)], True, True)
        mm(pB[:], vt[:], cs["ktc"][:, 96*p:96*(p+1)], True, True)
        sA = stg.tile([96, 96], f32, tag="kr_sA")
        nc.scalar.activation(out=sA[:], in_=pA[:], func=Act.Copy)
        straight = stg.tile([96, 96], f32, tag="kr_str")
        nc.vector.tensor_tensor(out=straight[:], in0=sA[:], in1=pB[:], op=Alu.mult)
        pC = psm.tile([96, 96], f32, tag="ps_s")
        pD = psm.tile([96, 96], f32, tag="ps_s")
        mm(pC[:], ut[:], cs["ktc"][:, 96*p:96*(p+1)], True, True)
        mm(pD[:], vt[:], cs["kfc"][:, 96*p:96*(p+1)], True, True)
        sC = stg.tile([96, 96], f32, tag="kr_sC")
        nc.scalar.activation(out=sC[:], in_=pC[:], func=Act.Copy)
        cross = stg.tile([96, 96], f32, tag="kr_crs")
        nc.vector.tensor_tensor(out=cross[:], in0=sC[:], in1=pD[:], op=Alu.mult)
        gmax = stg.tile([96, 96], f32, tag="kr_gmax")
        nc.vector.tensor_tensor(out=gmax[:], in0=straight[:], in1=cross[:],
                                op=Alu.max)
        nc.scalar.activation(out=Me[:, 96*p:96*(p+1)], in_=gmax[:], func=Act.Exp,
                             scale=float(1.0/TEMP))
    if dbg:
        nc.sync.dma_start(out=dbg["dbg_M0"][:], in_=Me[:])

    # ---------- phase 7: edge sinkhorn (multiplicative) + interleaved cdist
    D = persist.tile([96, 768], f32, tag="D")
    ec_units = strip_cdist(D,
                blk_of_ph=lambda p, hh: (bid0b if hh == 0 else bid1b)[:, 192*p+96:192*p+192],
                cols_of_ph=lambda p, hh, i0, ni: (bid0b if hh == 0 else bid1b)
                    [:, 192*p+i0:192*p+i0+ni],
                dpart=128, blk=96, nacc=2, tag="ec", dt_=bf16, defer=True)
    # Lazy row normalization: stored Me is only col-normalized; the current
    # row factors rr (= 1/rowsum(Me)) are folded into the colsum stationary
    # each iteration and into the final plan*D dot, saving one [96,768]
    # DVE pass per iteration.
    Me3 = Me[:].rearrange("p (b j) -> p b j", j=96)
    rr = persist.tile([96, 8], f32, tag="es_rr")
    for it in range(ITERS):
        rs = stg.tile([96, 8], f32, tag="es_rs")
        nc.vector.tensor_reduce(out=rs[:], in_=Me3, axis=AX.X, op=Alu.add)
        nc.vector.reciprocal(out=rr[:], in_=rs[:])
        rc = stg1.tile([96, 768], f32, tag="big768")
        pcs_l = []
        for hh in range(2):
            pcs = psm.tile([96, 384], f32, tag="ps_s")
            for q in range(4):
                pp = 4*hh + q
                mm(pcs[:, 96*q:96*(q+1)], rr[:, pp:pp+1].to_broadcast((96, 96)),
                   Me[:, 96*pp:96*(pp+1)], True, True)
            pcs_l.append(pcs)
        for _ in range(2):
            if ec_units:
                ec_units.pop(0)()
        for hh in range(2):
            nc.vector.reciprocal_approx_fast(out=rc[:, 384*hh:384*(hh+1)],
                                             in_=pcs_l[hh][:])
        nc.vector.tensor_tensor(out=Me[:], in0=Me[:], in1=rc[:], op=Alu.mult)
    while ec_units:
        ec_units.pop(0)()
    if dbg:
        nc.vector.scalar_tensor_tensor(
            out=Me3, in0=Me3, scalar=1.0, in1=bcast_in(rr[:], 96),
            op0=Alu.mult, op1=Alu.mult)
        nc.sync.dma_start(out=dbg["dbg_eplan"][:], in_=Me[:])
        nc.sync.dma_start(out=dbg["dbg_D"][:], in_=D[:])


    # (ncd computed interleaved with node sinkhorn above)
    if dbg:
        nc.sync.dma_start(out=dbg["dbg_ncd"][:], in_=ncd[:])

    # ---------- phase 11: dots + output
    we = stg1.tile([96, 768], f32, tag="big768")
    nc.vector.tensor_tensor(out=we[:], in0=Me[:], in1=D[:], op=Alu.mult)
    ep = stg.tile([96, 8], f32, tag="dot_ep")
    nc.vector.tensor_reduce(out=ep[:], in_=we[:].rearrange(
        "p (b j) -> p b j", j=96), axis=AX.X, op=Alu.add)
    nc.vector.tensor_tensor(out=ep[:], in0=ep[:], in1=rr[:], op=Alu.mult)
    wn = stg.tile([32, 256], f32, tag="dot_wn")
    nc.vector.tensor_tensor(out=wn[:], in0=nplan[:], in1=ncd[:], op=Alu.mult)
    np_ = stg.tile([32, 8], f32, tag="dot_np")
    nc.vector.tensor_reduce(out=np_[:], in_=wn[:].rearrange(
        "p (b j) -> p b j", j=32), axis=AX.X, op=Alu.add)
    pout = psm.tile([1, 8], f32, tag="ps_s")
    mm(pout[:], ones96sq[:, 0:1], ep[:], True, False)
    mm(pout[:], ones32f[:], np_[:], False, True)
    osb = stg.tile([1, 8], f32, tag="osb")
    nc.vector.tensor_copy(osb[:], pout[:])
    nc.sync.dma_start(out=out_ext[:], in_=osb[:])

    ctx.close()


# ----------------------------------------------------------------- entry
def _get_nc(debug=False):
    key = ("nc", debug)
    if key not in _CACHE:
        _CACHE[key] = _build(debug=debug)
    return _CACHE[key]


def run_cores(inputs, debug=False, trace=False):
    from concourse.bass_utils import run_bass_kernel_spmd
    nc = _get_nc(debug=debug)
    in_maps = _host_prep(inputs)
    res = run_bass_kernel_spmd(nc, in_maps, core_ids=list(range(NCORE)),
                               trace=trace)
    return res


def kernel(**inputs):
    res = run_cores(inputs, debug=False, trace=False)
    out = np.concatenate([r["out"].reshape(-1) for r in res.results])
    return out.astype(np.float32)



# revision 13
# speedup vs baseline: 1.2630x; 1.2630x over previous
"""Trainium2 Bass kernel for nn_ABL_SPARSE_87694642250045 (GMN graph matching).

Data-parallel over B=64 graph pairs: 8 pairs (16 graphs) per NeuronCore, 8 cores.
No collectives — output is per-pair scalars, concatenated host-side.

Device decomposition (per core):
  - gathers/segment-sums as one-hot matmuls (one-hots precomputed host-side)
  - message MLP with W-swap trick: both directions in one [*,512] hidden
  - residual update folded into (W_upd_a + I)
  - node sinkhorn in log space (PE transposes for column steps)
  - edge sinkhorn multiplicative (column sums via ones-matmul, no transposes)
  - L1 cdists: tensor_scalar |a-b| with d-on-partitions + ones-matmul reduce
    (strips) + tile_position rotation + SBUF DMA reshape
"""
import numpy as np

NCORE = 8
B, N, E = 64, 32, 96
NPROP, TEMP, ITERS = 5, 0.1, 20
BL = B // NCORE          # 8 pairs / core
GL = 2 * BL              # 16 graphs / core
VL = GL * N              # 512 nodes / core
EL = GL * E              # 1536 edges / core

_CACHE = {}


# ----------------------------------------------------------------- host prep
def _onehot(idx, n):
    out = np.zeros((len(idx), n), np.float32)
    out[np.arange(len(idx)), idx] = 1.0
    return out


def _host_prep(inputs):
    f32 = np.float32
    nf = np.asarray(inputs["node_features"], f32)
    ef = np.asarray(inputs["edge_features"], f32)
    fr_all = np.asarray(inputs["from_idx"]).astype(np.int64)
    to_all = np.asarray(inputs["to_idx"]).astype(np.int64)

    W_enc = np.asarray(inputs["W_enc"], f32); b_enc = np.asarray(inputs["b_enc"], f32)
    W1 = np.asarray(inputs["W_msg1"], f32); b1 = np.asarray(inputs["b_msg1"], f32)
    W2 = np.asarray(inputs["W_msg2"], f32); b2 = np.asarray(inputs["b_msg2"], f32)
    Wu = np.asarray(inputs["W_upd"], f32); bu = np.asarray(inputs["b_upd"], f32)
    Wsk1 = np.asarray(inputs["W_sk1"], f32); bsk1 = np.asarray(inputs["b_sk1"], f32)
    Wsk2 = np.asarray(inputs["W_sk2"], f32); bsk2 = np.asarray(inputs["b_sk2"], f32)
    Wl1 = np.asarray(inputs["W_lrl1"], f32); bl1 = np.asarray(inputs["b_lrl1"], f32)
    Wl2 = np.asarray(inputs["W_lrl2"], f32); bl2 = np.asarray(inputs["b_lrl2"], f32)

    def ext(Wm, bm):
        Wswap = np.concatenate([Wm[128:256], Wm[0:128], Wm[256:257]], axis=0)
        Wcat = np.concatenate([Wm, Wswap], axis=1)               # [257,512]
        bcat = np.concatenate([bm, bm])[None]                    # [1,512]
        return np.ascontiguousarray(np.concatenate([Wcat, bcat], axis=0))  # [258,512]

    W1ext = ext(W1, b1)
    Wl1ext = ext(Wl1, bl1)

    shared = {
        "w1a": W1ext[0:128], "w1b": W1ext[128:256], "w1c": W1ext[256:258],
        "wl1a": Wl1ext[0:128], "wl1b": Wl1ext[128:256], "wl1c": Wl1ext[256:258],
        "w2a": W2[0:128], "w2b": W2[128:256],
        "wl2a": Wl2[0:128], "wl2b": Wl2[128:256],
        "b2r": b2[None], "bl2r": (2.0 * bl2)[None],
        "wuaI": Wu[0:128] + np.eye(128, dtype=f32),
        "wub_a": Wu[128:256], "wub_b": Wu[256:384], "bur": bu[None],
        "wenc": W_enc, "bencr": b_enc[None],
        "wsk1": Wsk1, "bsk1r": bsk1[None], "wsk2": Wsk2, "bsk2r": bsk2[None],
    }
    shared = {k: np.ascontiguousarray(v, f32) for k, v in shared.items()}

    in_maps = []
    for c in range(NCORE):
        nfc = nf[c*VL:(c+1)*VL]                                  # [512,32]
        efc = ef[c*EL:(c+1)*EL]                                  # [1536,1]
        fr = fr_all[c*EL:(c+1)*EL] - c*VL
        to = to_all[c*EL:(c+1)*EL] - c*VL

        gfT = np.zeros((128, EL), f32)
        gtT = np.zeros((128, EL), f32)
        for g in range(4):
            e0, v0 = 384*g, 128*g
            gfT[:, e0:e0+384] = _onehot(fr[e0:e0+384] - v0, 128).T
            gtT[:, e0:e0+384] = _onehot(to[e0:e0+384] - v0, 128).T

        sT = np.zeros((128, 12*128), f32)
        sF = np.zeros((128, 12*128), f32)
        for kt in range(12):
            e0, g = 128*kt, kt // 3
            sT[:, 128*kt:128*(kt+1)] = _onehot(to[e0:e0+128] - 128*g, 128)
            sF[:, 128*kt:128*(kt+1)] = _onehot(fr[e0:e0+128] - 128*g, 128)

        frg = fr.reshape(GL, E) - (np.arange(GL) * N)[:, None]
        tog = to.reshape(GL, E) - (np.arange(GL) * N)[:, None]
        kfq = np.zeros((32, BL*E), f32); ktq = np.zeros((32, BL*E), f32)
        kfc = np.zeros((32, BL*E), f32); ktc = np.zeros((32, BL*E), f32)
        for p in range(BL):
            s = slice(E*p, E*(p+1))
            kfq[:, s] = _onehot(frg[2*p], N).T
            ktq[:, s] = _onehot(tog[2*p], N).T
            kfc[:, s] = _onehot(frg[2*p+1], N).T
            ktc[:, s] = _onehot(tog[2*p+1], N).T

        e1 = np.concatenate([efc.T, np.ones((1, EL), f32)], axis=0)  # [2,1536]

        m = dict(shared)
        m.update({
            "nfT": np.ascontiguousarray(nfc.T),      # [32,512]
            "e1": np.ascontiguousarray(e1),
            "gfT": gfT, "gtT": gtT, "sT": sT, "sF": sF,
            "kfq": kfq, "ktq": ktq, "kfc": kfc, "ktc": ktc,
        })
        in_maps.append(m)
    return in_maps


# --------------------------------------------------------------- bass builder
def _build(debug=False):
    import concourse.bass as bass
    import concourse.bacc as bacc
    import concourse.mybir as mybir
    import concourse.tile as tile
    from concourse.masks import make_identity

    f32 = mybir.dt.float32
    bf16 = mybir.dt.bfloat16
    f32r = mybir.dt.float32r
    Alu = mybir.AluOpType
    Act = mybir.ActivationFunctionType
    AX = mybir.AxisListType

    nc = bacc.Bacc("TRN2", target_bir_lowering=False)

    # ---- dram declarations
    dr = {}
    decls = {
        "nfT": (32, VL), "e1": (2, EL), "gfT": (128, EL), "gtT": (128, EL),
        "sT": (128, 12*128), "sF": (128, 12*128),
        "kfq": (32, BL*E), "ktq": (32, BL*E), "kfc": (32, BL*E), "ktc": (32, BL*E),
        "w1a": (128, 512), "w1b": (128, 512), "w1c": (2, 512),
        "wl1a": (128, 512), "wl1b": (128, 512), "wl1c": (2, 512),
        "w2a": (128, 256), "w2b": (128, 256), "wl2a": (128, 256), "wl2b": (128, 256),
        "b2r": (1, 256), "bl2r": (1, 256),
        "wuaI": (128, 128), "wub_a": (128, 128), "wub_b": (128, 128), "bur": (1, 128),
        "wenc": (32, 128), "bencr": (1, 128),
        "wsk1": (128, 32), "bsk1r": (1, 32), "wsk2": (32, 32), "bsk2r": (1, 32),
    }
    f32r_names = set(['wl1a', 'wl1b', 'wl1c', 'wl2a', 'wl2b', 'bl2r',
                      'w1a', 'w1b', 'w1c', 'w2a', 'w2b', 'b2r',
                      'wuaI', 'wub_a', 'wub_b', 'bur', 'gfT', 'gtT', 'e1',
                      'wsk1', 'bsk1r', 'wsk2', 'bsk2r'])
    for k, shp in decls.items():
        dt_ = f32r if k in f32r_names else f32
        dr[k] = nc.declare_dram_parameter(k, list(shp), dt_, isOutput=False)
    out_ext = nc.declare_dram_parameter("out", [1, BL], f32, isOutput=True)
    dbg = {}
    if debug:
        for k, shp in {
            "dbg_hT0": (128, 512), "dbg_hT": (128, 512), "dbg_tqT": (32, 256),
            "dbg_tcT": (32, 256), "dbg_cost": (32, 256), "dbg_nplan": (32, 256),
            "dbg_M0": (96, 768), "dbg_eplan": (96, 768), "dbg_D": (96, 768),
            "dbg_ncd": (32, 256),
        }.items():
            dbg[k] = nc.declare_dram_parameter(k, list(shp), f32, isOutput=True)

    with tile.TileContext(nc) as tc:
        _emit(nc, tc, dr, out_ext, dbg, f32, bf16, f32r, Alu, Act, AX, make_identity)
    nc.compile()
    return nc


def _emit(nc, tc, dr, out_ext, dbg, f32, bf16, f32r, Alu, Act, AX, make_identity):
    import concourse.bass as bass
    from contextlib import ExitStack

    ctx = ExitStack()
    const = ctx.enter_context(tc.tile_pool(name="const", bufs=1))
    persist = ctx.enter_context(tc.tile_pool(name="persist", bufs=1))
    wrk = ctx.enter_context(tc.tile_pool(name="wrk", bufs=1))
    hpool = ctx.enter_context(tc.tile_pool(name="hpool", bufs=2))
    hidp = ctx.enter_context(tc.tile_pool(name="hidp", bufs=1))
    stg = ctx.enter_context(tc.tile_pool(name="stg", bufs=3))
    stg2 = ctx.enter_context(tc.tile_pool(name="stg2", bufs=2))
    stg1 = ctx.enter_context(tc.tile_pool(name="stg1", bufs=1))
    pbig = ctx.enter_context(tc.tile_pool(name="pbig", bufs=3, space="PSUM"))
    pmsg = pbig
    pagg = ctx.enter_context(tc.tile_pool(name="pagg", bufs=1, space="PSUM"))
    psm = ctx.enter_context(tc.tile_pool(name="psm", bufs=3, space="PSUM"))

    def mm(out, lhsT, rhs, start, stop, dt=None, tile_position=None):
        if dt is not None:
            lhsT = lhsT.bitcast(dt)
            rhs = rhs.bitcast(dt)
        nc.tensor.matmul(out, lhsT, rhs, start=start, stop=stop,
                         tile_position=tile_position)

    def bcast_in(ap, n):
        # [P, F] -> [P, F, n] with stride-0 inner free dim
        a = ap
        return bass.AP(tensor=a.tensor, offset=a.offset,
                       ap=list(a.ap) + [[0, n]])

    # ---------- constants to SBUF
    cs = {}
    for k, shp in {
        "nfT": (32, VL), "e1": (2, EL), "gfT": (128, EL), "gtT": (128, EL),
        "sT": (128, 12*128), "sF": (128, 12*128),
        "kfq": (32, BL*E), "ktq": (32, BL*E), "kfc": (32, BL*E), "ktc": (32, BL*E),
        "w1a": (128, 512), "w1b": (128, 512), "w1c": (2, 512),
        "wl1a": (128, 512), "wl1b": (128, 512), "wl1c": (2, 512),
        "w2a": (128, 256), "w2b": (128, 256), "wl2a": (128, 256), "wl2b": (128, 256),
        "b2r": (1, 256), "bl2r": (1, 256),
        "wuaI": (128, 128), "wub_a": (128, 128), "wub_b": (128, 128), "bur": (1, 128),
        "wenc": (32, 128), "bencr": (1, 128),
        "wsk1": (128, 32), "bsk1r": (1, 32), "wsk2": (32, 32), "bsk2r": (1, 32),
    }.items():
        dt_ = f32r if k in ['wl1a', 'wl1b', 'wl1c', 'wl2a', 'wl2b', 'bl2r',
                            'w1a', 'w1b', 'w1c', 'w2a', 'w2b', 'b2r',
                            'wuaI', 'wub_a', 'wub_b', 'bur', 'gfT', 'gtT',
                            'e1', 'wsk1', 'bsk1r', 'wsk2', 'bsk2r'] else f32
        t = const.tile(list(shp), dt_, tag=k)
        nc.sync.dma_start(out=t[:], in_=dr[k][:])
        cs[k] = t

    e1r = cs["e1"]
    identf = const.tile([128, 128], f32, tag="identf")
    make_identity(nc, identf[:])
    ones96sq = const.tile([96, 96], f32, tag="ones96sq")
    nc.vector.memset(ones96sq[:], 1.0)
    identr = const.tile([128, 128], f32r, tag="identr")
    nc.vector.tensor_copy(identr[:], identf[:])
    ones1f = const.tile([1, 512], f32, tag="ones1f")
    nc.vector.memset(ones1f[:], 1.0)
    ones1 = const.tile([1, 512], f32r, tag="ones1")
    nc.vector.tensor_copy(ones1[:], ones1f[:])
    ones128f = const.tile([128, 1], f32, tag="ones128f")
    nc.vector.memset(ones128f[:], 1.0)
    ones128r = const.tile([128, 1], f32r, tag="ones128r")
    nc.vector.tensor_copy(ones128r[:], ones128f[:])
    ones128b = const.tile([128, 1], bf16, tag="ones128b")
    nc.vector.memset(ones128b[:], 1.0)
    ones32 = const.tile([32, 1], f32r, tag="ones32")
    nc.vector.tensor_copy(ones32[:], ones128f[:32, :])
    ones32f = const.tile([32, 1], f32, tag="ones32f")
    nc.vector.memset(ones32f[:], 1.0)

    # ---------- phase 1: encoder -> hT [128,512], hrm [128,(4g,128f)]
    hT = persist.tile([128, 512], f32r, tag="hT")
    hrm = persist.tile([128, 512], f32r, tag="hrm")

    ps = pbig.tile([128, 512], f32, tag="pa")
    mm(ps[:], cs["wenc"][:], cs["nfT"][:], start=True, stop=False)
    mm(ps[:], cs["bencr"][:], ones1f[:], start=False, stop=True)
    nc.scalar.activation(out=hT[:], in_=ps[:], func=Act.Copy)
    for g in range(4):
        psg = psm.tile([128, 128], f32, tag="ps_s")
        mm(psg[:], cs["nfT"][:, 128*g:128*(g+1)], cs["wenc"][:],
           start=True, stop=False)
        mm(psg[:], ones1f[:1, :128], cs["bencr"][:], start=False, stop=True)
        nc.vector.tensor_copy(hrm[:, 128*g:128*(g+1)], psg[:])
    if dbg:
        nc.sync.dma_start(out=dbg["dbg_hT0"][:], in_=hT[:])

    # ---------- phase 2: propagation steps
    def message_layer(hrm_t, wa, wb, wc2, w2_a, w2_b, b2row, lrl):
        """gathers + L1; returns hid tile [128, 4*1536] (mtile m at cols 1536m)"""
        mdt = f32r
        tdt = f32r
        e1t = cs["e1"]
        srcT = wrk.tile([128, EL], tdt, tag="srcT")
        dstT = wrk.tile([128, EL], tdt, tag="dstT")
        for g in range(4):
            psrc = pmsg.tile([128, 384], f32, tag="pa")
            pdst = pmsg.tile([128, 384], f32, tag="pa")
            hg = hrm_t[:, 128*g:128*(g+1)]
            mm(psrc[:], hg, cs["gfT"][:, 384*g:384*(g+1)], start=True, stop=True,
               dt=f32r)
            mm(pdst[:], hg, cs["gtT"][:, 384*g:384*(g+1)], start=True, stop=True,
               dt=f32r)
            nc.scalar.activation(out=srcT[:, 384*g:384*(g+1)], in_=psrc[:],
                                 func=Act.Copy)
            nc.scalar.activation(out=dstT[:, 384*g:384*(g+1)], in_=pdst[:],
                                 func=Act.Copy)
        hid = hidp.tile([128, 4*EL], tdt, tag="hid")
        for m in range(4):
            for n in range(3):
                ph = pbig.tile([128, 512], f32, tag="pa")
                ns = slice(512*n, 512*(n+1))
                mm(ph[:], wa[:, 128*m:128*(m+1)], srcT[:, ns], True, False, dt=mdt)
                mm(ph[:], wb[:, 128*m:128*(m+1)], dstT[:, ns], False, False, dt=mdt)
                mm(ph[:], wc2[:, 128*m:128*(m+1)], e1t[:, ns], False, True, dt=mdt)
                dst_ap = hid[:, EL*m + 512*n: EL*m + 512*(n+1)]
                nc.scalar.activation(out=dst_ap, in_=ph[:], func=Act.Relu)
        return hid

    for step in range(NPROP):
        hid = message_layer(hrm, cs["w1a"], cs["w1b"], cs["w1c"],
                            cs["w2a"], cs["w2b"], cs["b2r"], lrl=False)
        # L2 row-major per edge block + wide scatter
        paggT0 = pagg.tile([128, 512], f32, tag="ps_agg0")
        paggT1 = pagg.tile([128, 512], f32, tag="ps_agg1")
        for eb in range(12):
            pmf = pmsg.tile([128, 256], f32, tag="pa")
            pmb = pmsg.tile([128, 256], f32, tag="pa")
            ebs = slice(128*eb, 128*(eb+1))
            mm(pmf[:], hid[:, EL*0 + 128*eb: EL*0 + 128*(eb+1)], cs["w2a"][:],
               True, False, dt=f32r)
            mm(pmf[:], hid[:, EL*1 + 128*eb: EL*1 + 128*(eb+1)], cs["w2b"][:],
               False, False, dt=f32r)
            mm(pmf[:], ones1[:1, :128], cs["b2r"][:], False, True, dt=f32r)
            mm(pmb[:], hid[:, EL*2 + 128*eb: EL*2 + 128*(eb+1)], cs["w2a"][:],
               True, False, dt=f32r)
            mm(pmb[:], hid[:, EL*3 + 128*eb: EL*3 + 128*(eb+1)], cs["w2b"][:],
               False, False, dt=f32r)
            mm(pmb[:], ones1[:1, :128], cs["b2r"][:], False, True, dt=f32r)
            mf = stg.tile([128, 256], f32, tag="mf")
            mb = stg.tile([128, 256], f32, tag="mb")
            nc.scalar.activation(out=mf[:], in_=pmf[:], func=Act.Copy)
            nc.scalar.activation(out=mb[:], in_=pmb[:], func=Act.Copy)
            kts = slice(128*eb, 128*(eb+1))
            g = eb // 3
            gs = slice(128*g, 128*(g+1))
            first = (eb % 3 == 0)
            last = (eb % 3 == 2)
            mm(paggT0[:, gs], mf[:, 0:128], cs["sT"][:, kts], first, False)
            mm(paggT0[:, gs], mb[:, 0:128], cs["sF"][:, kts], False, last)
            mm(paggT1[:, gs], mf[:, 128:256], cs["sT"][:, kts], first, False)
            mm(paggT1[:, gs], mb[:, 128:256], cs["sF"][:, kts], False, last)
        aggT0 = hpool.tile([128, 512], f32r, tag="aggT0")
        aggT1 = hpool.tile([128, 512], f32r, tag="aggT1")
        nc.scalar.activation(out=aggT0[:], in_=paggT0[:], func=Act.Copy)
        nc.scalar.activation(out=aggT1[:], in_=paggT1[:], func=Act.Copy)
        # update
        pnew = pbig.tile([128, 512], f32, tag="pa")
        mm(pnew[:], cs["wuaI"][:], hT[:], True, False, dt=f32r)
        mm(pnew[:], cs["wub_a"][:], aggT0[:], False, False, dt=f32r)
        mm(pnew[:], cs["wub_b"][:], aggT1[:], False, False, dt=f32r)
        mm(pnew[:], cs["bur"][:], ones1[:], False, True, dt=f32r)
        hT_new = hpool.tile([128, 512], f32r, tag="hTn")
        nc.scalar.activation(out=hT_new[:], in_=pnew[:], func=Act.Copy)
        hrm_new = hpool.tile([128, 512], f32r, tag="hrmn")
        for g in range(4):
            pt = psm.tile([128, 128], f32r, tag="ps_s")
            nc.tensor.transpose(pt[:], hT_new[:, 128*g:128*(g+1)], identr[:])
            nc.scalar.activation(out=hrm_new[:, 128*g:128*(g+1)], in_=pt[:],
                                 func=Act.Copy)
        hT, hrm = hT_new, hrm_new
    if dbg:
        nc.sync.dma_start(out=dbg["dbg_hT"][:], in_=hT[:])

    # ---------- phase 3: sk path (tqT/tcT [32, (8p,32n)])
    def h_cols(par):  # par=0 query, 1 corpus -> [128, (8p, 32n)] AP view
        v = hT[:].rearrange("p (g x n) -> p g x n", x=2, n=32)
        return v[:, :, par, :]

    tqT = persist.tile([32, 256], f32, tag="tqT")
    tcT = persist.tile([32, 256], f32, tag="tcT")
    for par, dst in ((0, tqT), (1, tcT)):
        p1 = psm.tile([32, 256], f32, tag="ps_s")
        mm(p1[:], cs["wsk1"][:], h_cols(par), True, False, dt=f32r)
        mm(p1[:], cs["bsk1r"][:], ones1[:1, :256], False, True, dt=f32r)
        s1 = stg.tile([32, 256], f32r, tag="sk_s1")
        nc.scalar.activation(out=s1[:], in_=p1[:], func=Act.Relu)
        p2 = psm.tile([32, 256], f32, tag="ps_s")
        mm(p2[:], cs["wsk2"][:], s1[:], True, False, dt=f32r)
        mm(p2[:], cs["bsk2r"][:], ones1[:1, :256], False, True, dt=f32r)
        nc.vector.tensor_copy(dst[:], p2[:])
    if dbg:
        nc.sync.dma_start(out=dbg["dbg_tqT"][:], in_=tqT[:])
        nc.sync.dma_start(out=dbg["dbg_tcT"][:], in_=tcT[:])

    # ---------- phase 4: node cost [32, (8p,32j)] via strips
    cost = persist.tile([32, 256], f32, tag="cost")

    def strip_cdist(out_tile, blk_of_ph, cols_of_ph, dpart, blk, nacc, tag, dt_, defer=False):
        """out_tile[i, blk*p+j] = sum_d |blk_of_ph(p,hh)[d,j] - cols_of_ph(p,hh)[d,i]|.

        Wide-batched: per (p, hh, 16-i chunk): one TT subtract + one STT abs over
        [dpart, 16*blk], then 4 strip matmuls (tile_position rotation c=0..3,
        4 i-strips each) reduce over d into PSUM rows {0,32,64,96}; evacuate
        via full-tile copy + strided SBUF->SBUF DMA.
        """
        ones_l = ones128b if dt_ == bf16 else (ones32f if dpart == 32 else ones128f)
        units = []
        for p in range(BL):
            for ib in range(6 if blk == 96 else 2):
                units.append((p, ib))
        closures = []
        def make_unit(p, ib):
            def unit():
                pstr = psm.tile([128, 4 * blk], f32, tag="ps_s")
                st0 = stg2.tile([dpart, 16 * blk], dt_, tag=tag + "_s0")
                st1 = None
                srcs = [st0]
                if nacc == 2:
                    st1 = stg2.tile([dpart, 16 * blk], dt_, tag=tag + "_s1")
                    srcs.append(st1)
                for hh in range(nacc):
                    stt_t = srcs[hh]
                    blk_ap = blk_of_ph(p, hh)
                    cols_ap = cols_of_ph(p, hh, 16 * ib, 16)
                    in0 = bass.AP(tensor=blk_ap.tensor, offset=blk_ap.offset,
                                  ap=[blk_ap.ap[0], [0, 16]] + list(blk_ap.ap[1:]))
                    in1 = bass.AP(tensor=cols_ap.tensor, offset=cols_ap.offset,
                                  ap=list(cols_ap.ap) + [[0, blk]])
                    v3 = stt_t[:].rearrange("p (i j) -> p i j", j=blk)
                    nc.vector.tensor_tensor(out=v3, in0=in0, in1=in1,
                                            op=Alu.subtract)
                    nc.vector.scalar_tensor_tensor(
                        out=v3, in0=v3, scalar=-1.0, in1=v3,
                        op0=Alu.mult, op1=Alu.max)
                for c in range(4):
                    cs_ = slice(4 * blk * c, 4 * blk * (c + 1))
                    mm(pstr[32*c:32*c+1, :], ones_l[:], st0[:, cs_],
                       True, nacc == 1, tile_position=(0, 32*c))
                    if nacc == 2:
                        mm(pstr[32*c:32*c+1, :], ones_l[:], st1[:, cs_],
                           False, True, tile_position=(0, 32*c))
                s2 = stg2.tile([128, 4 * blk], f32, tag=tag + "_s2")
                nc.vector.tensor_copy(s2[:], pstr[:])
                sv = s2[:]
                iv = bass.AP(tensor=sv.tensor, offset=sv.offset,
                             ap=[[32 * sv.ap[0][0], 4], [blk, 4], [1, blk]])
                nc.sync.dma_start(
                    out=out_tile[16*ib:16*(ib+1), blk*p:blk*(p+1)], in_=iv)
            return unit
        for (p, ib) in units:
            closures.append(make_unit(p, ib))
        if defer:
            return closures
        for cl in closures:
            cl()

    strip_cdist(cost,
                blk_of_ph=lambda p, hh: tcT[:, 32*p:32*(p+1)],
                cols_of_ph=lambda p, hh, i0, ni: tqT[:, 32*p+i0:32*p+i0+ni],
                dpart=32, blk=32, nacc=1, tag="nc", dt_=f32)
    if dbg:
        nc.sync.dma_start(out=dbg["dbg_cost"][:], in_=cost[:])

    # ---------- phase 8/9: lrl embeddings + edge cdist D [96, (8p,96j)]
    hidL = message_layer(hrm, cs["wl1a"], cs["wl1b"], cs["wl1c"],
                         cs["wl2a"], cs["wl2b"], cs["bl2r"], lrl=True)
    bid0b = persist.tile([128, EL], bf16, tag="bid0b")
    bid1b = persist.tile([128, EL], bf16, tag="bid1b")
    for mt, dst in ((0, bid0b), (1, bid1b)):
        for n in range(3):
            pb2 = pbig.tile([128, 512], f32, tag="pa")
            ns = slice(512*n, 512*(n+1))
            mm(pb2[:], cs["wl2a"][:, 128*mt:128*(mt+1)],
               hidL[:, EL*0 + 512*n: EL*0 + 512*(n+1)], True, False, dt=f32r)
            mm(pb2[:], cs["wl2b"][:, 128*mt:128*(mt+1)],
               hidL[:, EL*1 + 512*n: EL*1 + 512*(n+1)], False, False, dt=f32r)
            mm(pb2[:], cs["wl2a"][:, 128*mt:128*(mt+1)],
               hidL[:, EL*2 + 512*n: EL*2 + 512*(n+1)], False, False, dt=f32r)
            mm(pb2[:], cs["wl2b"][:, 128*mt:128*(mt+1)],
               hidL[:, EL*3 + 512*n: EL*3 + 512*(n+1)], False, False, dt=f32r)
            mm(pb2[:], cs["bl2r"][:, 128*mt:128*(mt+1)], ones1[:], False, True)
            nc.scalar.activation(out=dst[:, ns], in_=pb2[:], func=Act.Copy)

    # ---------- phase 5: node sinkhorn, log space
    la = persist.tile([32, 256], f32, tag="la")
    nc.vector.tensor_scalar(out=la[:], in0=cost[:], scalar1=float(-1.0/TEMP),
                            scalar2=None, op0=Alu.mult)

    def ns_norm_step(t):
        """log-space normalize along each 32-wide free block of t [32, 256]."""
        t3 = t[:].rearrange("p (b j) -> p b j", j=32)
        rm = stg.tile([32, 8], f32, tag="ns_rm")
        nc.vector.tensor_reduce(out=rm[:], in_=t3, axis=AX.X, op=Alu.max,
                                negate=True)
        tmp = stg.tile([32, 256], f32, tag="ns_tmp")
        nc.vector.scalar_tensor_tensor(
            out=tmp[:].rearrange("p (b j) -> p b j", j=32), in0=t3, scalar=1.0,
            in1=bcast_in(rm[:], 32), op0=Alu.mult, op1=Alu.add)
        ex = stg.tile([32, 256], f32, tag="ns_ex")
        nc.scalar.activation(out=ex[:], in_=tmp[:], func=Act.Exp)
        sm = stg.tile([32, 8], f32, tag="ns_sm")
        nc.vector.tensor_reduce(out=sm[:], in_=ex[:].rearrange(
            "p (b j) -> p b j", j=32), axis=AX.X, op=Alu.add)
        ls = stg.tile([32, 8], f32, tag="ns_ls")
        nc.scalar.activation(out=ls[:], in_=sm[:], func=Act.Ln)
        lse = stg.tile([32, 8], f32, tag="ns_lse")
        nc.vector.tensor_tensor(out=lse[:], in0=ls[:], in1=rm[:], op=Alu.subtract)
        nc.vector.scalar_tensor_tensor(
            out=t3, in0=t3, scalar=1.0,
            in1=bcast_in(lse[:], 32), op0=Alu.mult, op1=Alu.subtract)

    ncd = persist.tile([32, 256], f32, tag="ncd")
    hTb = persist.tile([128, 512], bf16, tag="hTb")
    nc.vector.tensor_copy(hTb[:], hT[:])
    na_units = strip_cdist(ncd,
                blk_of_ph=lambda p, hh: hTb[:, 64*p+32:64*p+64],
                cols_of_ph=lambda p, hh, i0, ni: hTb[:, 64*p+i0:64*p+i0+ni],
                dpart=128, blk=32, nacc=1, tag="na", dt_=bf16, defer=True)

    lat = persist.tile([32, 256], f32, tag="lat")
    for it in range(ITERS):
        ns_norm_step(la)                      # row step
        nc.vector.transpose(lat[:], la[:])    # per-pair 32x32 block transpose
        if na_units:
            na_units.pop(0)()
        ns_norm_step(lat)                     # col step (rows of transposed)
        nc.vector.transpose(la[:], lat[:])
    while na_units:
        na_units.pop(0)()
    nplan = persist.tile([32, 256], f32, tag="nplan")
    nc.scalar.activation(out=nplan[:], in_=la[:], func=Act.Exp)
    if dbg:
        nc.sync.dma_start(out=dbg["dbg_nplan"][:], in_=nplan[:])

    # ---------- phase 6: kron -> M0 edge [96, (8p,96j)]
    Me = persist.tile([96, 768], f32, tag="Me")
    for p in range(BL):
        Pp = nplan[:, 32*p:32*(p+1)]
        put = psm.tile([32, 96], f32, tag="ps_s")
        pvt = psm.tile([32, 96], f32, tag="ps_s")
        mm(put[:], Pp, cs["kfq"][:, 96*p:96*(p+1)], True, True)
        mm(pvt[:], Pp, cs["ktq"][:, 96*p:96*(p+1)], True, True)
        ut = stg.tile([32, 96], f32, tag="kr_ut")
        vt = stg.tile([32, 96], f32, tag="kr_vt")
        nc.vector.tensor_copy(ut[:], put[:])
        nc.vector.tensor_copy(vt[:], pvt[:])
        pA = psm.tile([96, 96], f32, tag="ps_s")
        pB = psm.tile([96, 96], f32, tag="ps_s")
        mm(pA[:], ut[:], cs["kfc"][:, 96*p:96*(p+1)], True, True)
        mm(pB[:], vt[:], cs["ktc"][:, 96*p:96*(p+1)], True, True)
        sA = stg.tile([96, 96], f32, tag="kr_sA")
        nc.scalar.activation(out=sA[:], in_=pA[:], func=Act.Copy)
        straight = stg.tile([96, 96], f32, tag="kr_str")
        nc.vector.tensor_tensor(out=straight[:], in0=sA[:], in1=pB[:], op=Alu.mult)
        pC = psm.tile([96, 96], f32, tag="ps_s")
        pD = psm.tile([96, 96], f32, tag="ps_s")
        mm(pC[:], ut[:], cs["ktc"][:, 96*p:96*(p+1)], True, True)
        mm(pD[:], vt[:], cs["kfc"][:, 96*p:96*(p+1)], True, True)
        sC = stg.tile([96, 96], f32, tag="kr_sC")
        nc.scalar.activation(out=sC[:], in_=pC[:], func=Act.Copy)
        cross = stg.tile([96, 96], f32, tag="kr_crs")
        nc.vector.tensor_tensor(out=cross[:], in0=sC[:], in1=pD[:], op=Alu.mult)
        gmax = stg.tile([96, 96], f32, tag="kr_gmax")
        nc.vector.tensor_tensor(out=gmax[:], in0=straight[:], in1=cross[:],
                                op=Alu.max)
        nc.scalar.activation(out=Me[:, 96*p:96*(p+1)], in_=gmax[:], func=Act.Exp,
                             scale=float(1.0/TEMP))
    if dbg:
        nc.sync.dma_start(out=dbg["dbg_M0"][:], in_=Me[:])

    # ---------- phase 7: edge sinkhorn (multiplicative) + interleaved cdist
    D = persist.tile([96, 768], f32, tag="D")
    ec_units = strip_cdist(D,
                blk_of_ph=lambda p, hh: (bid0b if hh == 0 else bid1b)[:, 192*p+96:192*p+192],
                cols_of_ph=lambda p, hh, i0, ni: (bid0b if hh == 0 else bid1b)
                    [:, 192*p+i0:192*p+i0+ni],
                dpart=128, blk=96, nacc=2, tag="ec", dt_=bf16, defer=True)
    # Lazy row normalization: stored Me is only col-normalized; the current
    # row factors rr (= 1/rowsum(Me)) are folded into the colsum stationary
    # each iteration and into the final plan*D dot, saving one [96,768]
    # DVE pass per iteration.
    Me3 = Me[:].rearrange("p (b j) -> p b j", j=96)
    rr = persist.tile([96, 8], f32, tag="es_rr")
    for it in range(ITERS):
        rs = stg.tile([96, 8], f32, tag="es_rs")
        nc.vector.tensor_reduce(out=rs[:], in_=Me3, axis=AX.X, op=Alu.add)
        nc.vector.reciprocal(out=rr[:], in_=rs[:])
        rc = stg1.tile([96, 768], f32, tag="big768")
        pcs_l = []
        for hh in range(2):
            pcs = psm.tile([96, 384], f32, tag="ps_s")
            for q in range(4):
                pp = 4*hh + q
                mm(pcs[:, 96*q:96*(q+1)], rr[:, pp:pp+1].to_broadcast((96, 96)),
                   Me[:, 96*pp:96*(pp+1)], True, True)
            pcs_l.append(pcs)
        for _ in range(2):
            if ec_units:
                ec_units.pop(0)()
        for hh in range(2):
            nc.vector.reciprocal_approx_fast(out=rc[:, 384*hh:384*(hh+1)],
                                             in_=pcs_l[hh][:])
        nc.vector.tensor_tensor(out=Me[:], in0=Me[:], in1=rc[:], op=Alu.mult)
    while ec_units:
        ec_units.pop(0)()
    if dbg:
        nc.vector.scalar_tensor_tensor(
            out=Me3, in0=Me3, scalar=1.0, in1=bcast_in(rr[:], 96),
            op0=Alu.mult, op1=Alu.mult)
        nc.sync.dma_start(out=dbg["dbg_eplan"][:], in_=Me[:])
        nc.sync.dma_start(out=dbg["dbg_D"][:], in_=D[:])


    # (ncd computed interleaved with node sinkhorn above)
    if dbg:
        nc.sync.dma_start(out=dbg["dbg_ncd"][:], in_=ncd[:])

    # ---------- phase 11: dots + output
    we = stg1.tile([96, 768], f32, tag="big768")
    nc.vector.tensor_tensor(out=we[:], in0=Me[:], in1=D[:], op=Alu.mult)
    ep = stg.tile([96, 8], f32, tag="dot_ep")
    nc.vector.tensor_reduce(out=ep[:], in_=we[:].rearrange(
        "p (b j) -> p b j", j=96), axis=AX.X, op=Alu.add)
    nc.vector.tensor_tensor(out=ep[:], in0=ep[:], in1=rr[:], op=Alu.mult)
    wn = stg.tile([32, 256], f32, tag="dot_wn")
    nc.vector.tensor_tensor(out=wn[:], in0=nplan[:], in1=ncd[:], op=Alu.mult)
    np_ = stg.tile([32, 8], f32, tag="dot_np")
    nc.vector.tensor_reduce(out=np_[:], in_=wn[:].rearrange(
        "p (b j) -> p b j", j=32), axis=AX.X, op=Alu.add)
    pout = psm.tile([1, 8], f32, tag="ps_s")
    mm(pout[:], ones96sq[:, 0:1], ep[:], True, False)
    mm(pout[:], ones32f[:], np_[:], False, True)
    osb = stg.tile([1, 8], f32, tag="osb")
    nc.vector.tensor_copy(osb[:], pout[:])
    nc.sync.dma_start(out=out_ext[:], in_=osb[:])

    ctx.close()


# ----------------------------------------------------------------- entry
def _get_nc(debug=False):
    key = ("nc", debug)
    if key not in _CACHE:
        _CACHE[key] = _build(debug=debug)
    return _CACHE[key]


def run_cores(inputs, debug=False, trace=False):
    from concourse.bass_utils import run_bass_kernel_spmd
    nc = _get_nc(debug=debug)
    in_maps = _host_prep(inputs)
    res = run_bass_kernel_spmd(nc, in_maps, core_ids=list(range(NCORE)),
                               trace=trace)
    return res


def kernel(**inputs):
    res = run_cores(inputs, debug=False, trace=False)
    out = np.concatenate([r["out"].reshape(-1) for r in res.results])
    return out.astype(np.float32)



# revision 16
# speedup vs baseline: 1.2902x; 1.0216x over previous
"""Trainium2 Bass kernel for nn_ABL_SPARSE_87694642250045 (GMN graph matching).

Data-parallel over B=64 graph pairs: 8 pairs (16 graphs) per NeuronCore, 8 cores.
No collectives — output is per-pair scalars, concatenated host-side.

Device decomposition (per core):
  - gathers/segment-sums as one-hot matmuls (one-hots precomputed host-side)
  - message MLP with W-swap trick: both directions in one [*,512] hidden
  - residual update folded into (W_upd_a + I)
  - node sinkhorn in log space (PE transposes for column steps)
  - edge sinkhorn multiplicative (column sums via ones-matmul, no transposes)
  - L1 cdists: tensor_scalar |a-b| with d-on-partitions + ones-matmul reduce
    (strips) + tile_position rotation + SBUF DMA reshape
"""
import numpy as np

NCORE = 8
B, N, E = 64, 32, 96
NPROP, TEMP, ITERS = 5, 0.1, 20
BL = B // NCORE          # 8 pairs / core
GL = 2 * BL              # 16 graphs / core
VL = GL * N              # 512 nodes / core
EL = GL * E              # 1536 edges / core

_CACHE = {}


# ----------------------------------------------------------------- host prep
def _onehot(idx, n):
    out = np.zeros((len(idx), n), np.float32)
    out[np.arange(len(idx)), idx] = 1.0
    return out


def _host_prep(inputs):
    f32 = np.float32
    nf = np.asarray(inputs["node_features"], f32)
    ef = np.asarray(inputs["edge_features"], f32)
    fr_all = np.asarray(inputs["from_idx"]).astype(np.int64)
    to_all = np.asarray(inputs["to_idx"]).astype(np.int64)

    W_enc = np.asarray(inputs["W_enc"], f32); b_enc = np.asarray(inputs["b_enc"], f32)
    W1 = np.asarray(inputs["W_msg1"], f32); b1 = np.asarray(inputs["b_msg1"], f32)
    W2 = np.asarray(inputs["W_msg2"], f32); b2 = np.asarray(inputs["b_msg2"], f32)
    Wu = np.asarray(inputs["W_upd"], f32); bu = np.asarray(inputs["b_upd"], f32)
    Wsk1 = np.asarray(inputs["W_sk1"], f32); bsk1 = np.asarray(inputs["b_sk1"], f32)
    Wsk2 = np.asarray(inputs["W_sk2"], f32); bsk2 = np.asarray(inputs["b_sk2"], f32)
    Wl1 = np.asarray(inputs["W_lrl1"], f32); bl1 = np.asarray(inputs["b_lrl1"], f32)
    Wl2 = np.asarray(inputs["W_lrl2"], f32); bl2 = np.asarray(inputs["b_lrl2"], f32)

    def ext(Wm, bm):
        Wswap = np.concatenate([Wm[128:256], Wm[0:128], Wm[256:257]], axis=0)
        Wcat = np.concatenate([Wm, Wswap], axis=1)               # [257,512]
        bcat = np.concatenate([bm, bm])[None]                    # [1,512]
        return np.ascontiguousarray(np.concatenate([Wcat, bcat], axis=0))  # [258,512]

    W1ext = ext(W1, b1)
    Wl1ext = ext(Wl1, bl1)

    shared = {
        "w1a": W1ext[0:128], "w1b": W1ext[128:256], "w1c": W1ext[256:258],
        "wl1a": Wl1ext[0:128], "wl1b": Wl1ext[128:256], "wl1c": Wl1ext[256:258],
        "w2a": W2[0:128], "w2b": W2[128:256],
        "wl2a": Wl2[0:128], "wl2b": Wl2[128:256],
        "b2r": b2[None], "bl2r": (2.0 * bl2)[None],
        "wuaI": Wu[0:128] + np.eye(128, dtype=f32),
        "wub_a": Wu[128:256], "wub_b": Wu[256:384], "bur": bu[None],
        "wenc": W_enc, "bencr": b_enc[None],
        "wsk1": Wsk1, "bsk1r": bsk1[None], "wsk2": Wsk2, "bsk2r": bsk2[None],
    }
    shared = {k: np.ascontiguousarray(v, f32) for k, v in shared.items()}
    import ml_dtypes
    for k in ("w1c", "wl1c"):
        shared[k] = shared[k].astype(ml_dtypes.bfloat16)

    in_maps = []
    for c in range(NCORE):
        nfc = nf[c*VL:(c+1)*VL]                                  # [512,32]
        efc = ef[c*EL:(c+1)*EL]                                  # [1536,1]
        fr = fr_all[c*EL:(c+1)*EL] - c*VL
        to = to_all[c*EL:(c+1)*EL] - c*VL

        gfT = np.zeros((128, EL), f32)
        gtT = np.zeros((128, EL), f32)
        for g in range(4):
            e0, v0 = 384*g, 128*g
            gfT[:, e0:e0+384] = _onehot(fr[e0:e0+384] - v0, 128).T
            gtT[:, e0:e0+384] = _onehot(to[e0:e0+384] - v0, 128).T

        # scatter one-hots, 256-wide (group-pair local) so the scatter matmul
        # free dim is >=256 and f32r runs at 1 cycle/row
        sT = np.zeros((128, 12*256), f32)
        sF = np.zeros((128, 12*256), f32)
        for kt in range(12):
            e0, g = 128*kt, kt // 3
            gp0 = (g // 2) * 2                # group-pair base group
            sT[:, 256*kt:256*(kt+1)] = _onehot(to[e0:e0+128] - 128*gp0, 256)
            sF[:, 256*kt:256*(kt+1)] = _onehot(fr[e0:e0+128] - 128*gp0, 256)

        frg = fr.reshape(GL, E) - (np.arange(GL) * N)[:, None]
        tog = to.reshape(GL, E) - (np.arange(GL) * N)[:, None]
        kfq = np.zeros((32, BL*E), f32); ktq = np.zeros((32, BL*E), f32)
        kfc = np.zeros((32, BL*E), f32); ktc = np.zeros((32, BL*E), f32)
        for p in range(BL):
            s = slice(E*p, E*(p+1))
            kfq[:, s] = _onehot(frg[2*p], N).T
            ktq[:, s] = _onehot(tog[2*p], N).T
            kfc[:, s] = _onehot(frg[2*p+1], N).T
            ktc[:, s] = _onehot(tog[2*p+1], N).T

        e1 = np.concatenate([efc.T, np.ones((1, EL), f32)], axis=0)  # [2,1536]

        import ml_dtypes
        m = dict(shared)
        m.update({
            "nfT": np.ascontiguousarray(nfc.T),      # [32,512]
            "e1": np.ascontiguousarray(e1).astype(ml_dtypes.bfloat16),
            "gfT": gfT, "gtT": gtT, "sT": sT, "sF": sF,
            "kfq": kfq, "ktq": ktq, "kfc": kfc, "ktc": ktc,
        })
        in_maps.append(m)
    return in_maps


# --------------------------------------------------------------- bass builder
def _build(debug=False):
    import concourse.bass as bass
    import concourse.bacc as bacc
    import concourse.mybir as mybir
    import concourse.tile as tile
    from concourse.masks import make_identity

    f32 = mybir.dt.float32
    bf16 = mybir.dt.bfloat16
    f32r = mybir.dt.float32r
    Alu = mybir.AluOpType
    Act = mybir.ActivationFunctionType
    AX = mybir.AxisListType

    nc = bacc.Bacc("TRN2", target_bir_lowering=False)

    # ---- dram declarations
    dr = {}
    decls = {
        "nfT": (32, VL), "e1": (2, EL), "gfT": (128, EL), "gtT": (128, EL),
        "sT": (128, 12*256), "sF": (128, 12*256),
        "kfq": (32, BL*E), "ktq": (32, BL*E), "kfc": (32, BL*E), "ktc": (32, BL*E),
        "w1a": (128, 512), "w1b": (128, 512), "w1c": (2, 512),
        "wl1a": (128, 512), "wl1b": (128, 512), "wl1c": (2, 512),
        "w2a": (128, 256), "w2b": (128, 256), "wl2a": (128, 256), "wl2b": (128, 256),
        "b2r": (1, 256), "bl2r": (1, 256),
        "wuaI": (128, 128), "wub_a": (128, 128), "wub_b": (128, 128), "bur": (1, 128),
        "wenc": (32, 128), "bencr": (1, 128),
        "wsk1": (128, 32), "bsk1r": (1, 32), "wsk2": (32, 32), "bsk2r": (1, 32),
    }
    f32r_names = set(['wl1a', 'wl1b', 'wl1c', 'wl2a', 'wl2b', 'bl2r',
                      'w1a', 'w1b', 'w1c', 'w2a', 'w2b', 'b2r',
                      'wuaI', 'wub_a', 'wub_b', 'bur', 'gfT', 'gtT',
                      'wsk1', 'bsk1r', 'wsk2', 'bsk2r', 'sT', 'sF'])
    bf16_names = set(['w1c', 'wl1c', 'e1'])
    for k, shp in decls.items():
        dt_ = bf16 if k in bf16_names else (f32r if k in f32r_names else f32)
        dr[k] = nc.declare_dram_parameter(k, list(shp), dt_, isOutput=False)
    out_ext = nc.declare_dram_parameter("out", [1, BL], f32, isOutput=True)
    dbg = {}
    if debug:
        for k, shp in {
            "dbg_hT0": (128, 512), "dbg_hT": (128, 512), "dbg_tqT": (32, 256),
            "dbg_tcT": (32, 256), "dbg_cost": (32, 256), "dbg_nplan": (32, 256),
            "dbg_M0": (96, 768), "dbg_eplan": (96, 768), "dbg_D": (96, 768),
            "dbg_ncd": (32, 256),
        }.items():
            dbg[k] = nc.declare_dram_parameter(k, list(shp), f32, isOutput=True)

    with tile.TileContext(nc) as tc:
        _emit(nc, tc, dr, out_ext, dbg, f32, bf16, f32r, Alu, Act, AX, make_identity)
    nc.compile()
    return nc


def _emit(nc, tc, dr, out_ext, dbg, f32, bf16, f32r, Alu, Act, AX, make_identity):
    import concourse.bass as bass
    from contextlib import ExitStack

    ctx = ExitStack()
    const = ctx.enter_context(tc.tile_pool(name="const", bufs=1))
    persist = ctx.enter_context(tc.tile_pool(name="persist", bufs=1))
    wrk = ctx.enter_context(tc.tile_pool(name="wrk", bufs=1))
    hpool = ctx.enter_context(tc.tile_pool(name="hpool", bufs=2))
    hidp = ctx.enter_context(tc.tile_pool(name="hidp", bufs=1))
    stg = ctx.enter_context(tc.tile_pool(name="stg", bufs=3))
    stg2 = ctx.enter_context(tc.tile_pool(name="stg2", bufs=2))
    stg1 = ctx.enter_context(tc.tile_pool(name="stg1", bufs=1))
    pbig = ctx.enter_context(tc.tile_pool(name="pbig", bufs=3, space="PSUM"))
    pmsg = pbig
    pagg = ctx.enter_context(tc.tile_pool(name="pagg", bufs=1, space="PSUM"))
    psm = ctx.enter_context(tc.tile_pool(name="psm", bufs=3, space="PSUM"))

    def mm(out, lhsT, rhs, start, stop, dt=None, tile_position=None):
        if dt is not None:
            lhsT = lhsT.bitcast(dt)
            rhs = rhs.bitcast(dt)
        nc.tensor.matmul(out, lhsT, rhs, start=start, stop=stop,
                         tile_position=tile_position)

    def bcast_in(ap, n):
        # [P, F] -> [P, F, n] with stride-0 inner free dim
        a = ap
        return bass.AP(tensor=a.tensor, offset=a.offset,
                       ap=list(a.ap) + [[0, n]])

    # ---------- constants to SBUF
    cs = {}
    for k, shp in {
        "nfT": (32, VL), "e1": (2, EL), "gfT": (128, EL), "gtT": (128, EL),
        "sT": (128, 12*256), "sF": (128, 12*256),
        "kfq": (32, BL*E), "ktq": (32, BL*E), "kfc": (32, BL*E), "ktc": (32, BL*E),
        "w1a": (128, 512), "w1b": (128, 512), "w1c": (2, 512),
        "wl1a": (128, 512), "wl1b": (128, 512), "wl1c": (2, 512),
        "w2a": (128, 256), "w2b": (128, 256), "wl2a": (128, 256), "wl2b": (128, 256),
        "b2r": (1, 256), "bl2r": (1, 256),
        "wuaI": (128, 128), "wub_a": (128, 128), "wub_b": (128, 128), "bur": (1, 128),
        "wenc": (32, 128), "bencr": (1, 128),
        "wsk1": (128, 32), "bsk1r": (1, 32), "wsk2": (32, 32), "bsk2r": (1, 32),
    }.items():
        if k in ('w1c', 'wl1c', 'e1'):
            dt_ = bf16
        elif k in ['wl1a', 'wl1b', 'wl2a', 'wl2b', 'bl2r',
                   'w1a', 'w1b', 'w2a', 'w2b', 'b2r',
                   'wuaI', 'wub_a', 'wub_b', 'bur', 'gfT', 'gtT',
                   'wsk1', 'bsk1r', 'wsk2', 'bsk2r', 'sT', 'sF']:
            dt_ = f32r
        else:
            dt_ = f32
        t = const.tile(list(shp), dt_, tag=k)
        nc.sync.dma_start(out=t[:], in_=dr[k][:])
        cs[k] = t

    e1r = cs["e1"]
    identf = const.tile([128, 128], f32, tag="identf")
    make_identity(nc, identf[:])
    ones96sq = const.tile([96, 96], f32, tag="ones96sq")
    nc.vector.memset(ones96sq[:], 1.0)
    identr = const.tile([128, 128], f32r, tag="identr")
    nc.vector.tensor_copy(identr[:], identf[:])
    ones1f = const.tile([1, 512], f32, tag="ones1f")
    nc.vector.memset(ones1f[:], 1.0)
    ones1 = const.tile([1, 512], f32r, tag="ones1")
    nc.vector.tensor_copy(ones1[:], ones1f[:])
    ones128f = const.tile([128, 1], f32, tag="ones128f")
    nc.vector.memset(ones128f[:], 1.0)
    ones128r = const.tile([128, 1], f32r, tag="ones128r")
    nc.vector.tensor_copy(ones128r[:], ones128f[:])
    ones128b = const.tile([128, 1], bf16, tag="ones128b")
    nc.vector.memset(ones128b[:], 1.0)
    ones32 = const.tile([32, 1], f32r, tag="ones32")
    nc.vector.tensor_copy(ones32[:], ones128f[:32, :])
    ones32f = const.tile([32, 1], f32, tag="ones32f")
    nc.vector.memset(ones32f[:], 1.0)

    # ---------- phase 1: encoder -> hT [128,512], hrm [128,(4g,128f)]
    hT = persist.tile([128, 512], f32r, tag="hT")
    hrm = persist.tile([128, 512], f32r, tag="hrm")

    ps = pbig.tile([128, 512], f32, tag="pa")
    mm(ps[:], cs["wenc"][:], cs["nfT"][:], start=True, stop=False)
    mm(ps[:], cs["bencr"][:], ones1f[:], start=False, stop=True)
    nc.scalar.activation(out=hT[:], in_=ps[:], func=Act.Copy)
    for g in range(4):
        psg = psm.tile([128, 128], f32, tag="ps_s")
        mm(psg[:], cs["nfT"][:, 128*g:128*(g+1)], cs["wenc"][:],
           start=True, stop=False)
        mm(psg[:], ones1f[:1, :128], cs["bencr"][:], start=False, stop=True)
        nc.vector.tensor_copy(hrm[:, 128*g:128*(g+1)], psg[:])
    if dbg:
        nc.sync.dma_start(out=dbg["dbg_hT0"][:], in_=hT[:])

    # ---------- phase 2: propagation steps
    def message_layer(hrm_t, wa, wb, wc2, w2_a, w2_b, b2row, lrl):
        """gathers + L1; returns hid tile [128, 4*1536] (mtile m at cols 1536m)"""
        mdt = f32r
        tdt = f32r
        e1t = cs["e1"]
        srcT = wrk.tile([128, EL], tdt, tag="srcT")
        dstT = wrk.tile([128, EL], tdt, tag="dstT")
        for g in range(4):
            psrc = pmsg.tile([128, 384], f32, tag="pa")
            pdst = pmsg.tile([128, 384], f32, tag="pa")
            hg = hrm_t[:, 128*g:128*(g+1)]
            mm(psrc[:], hg, cs["gfT"][:, 384*g:384*(g+1)], start=True, stop=True,
               dt=f32r)
            mm(pdst[:], hg, cs["gtT"][:, 384*g:384*(g+1)], start=True, stop=True,
               dt=f32r)
            nc.scalar.activation(out=srcT[:, 384*g:384*(g+1)], in_=psrc[:],
                                 func=Act.Copy)
            nc.scalar.activation(out=dstT[:, 384*g:384*(g+1)], in_=pdst[:],
                                 func=Act.Copy)
        hid = hidp.tile([128, 4*EL], tdt, tag="hid")
        for m in range(4):
            for n in range(3):
                ph = pbig.tile([128, 512], f32, tag="pa")
                ns = slice(512*n, 512*(n+1))
                mm(ph[:], wa[:, 128*m:128*(m+1)], srcT[:, ns], True, False, dt=mdt)
                mm(ph[:], wb[:, 128*m:128*(m+1)], dstT[:, ns], False, False, dt=mdt)
                mm(ph[:], wc2[:, 128*m:128*(m+1)], e1t[:, ns], False, True)
                dst_ap = hid[:, EL*m + 512*n: EL*m + 512*(n+1)]
                nc.scalar.activation(out=dst_ap, in_=ph[:], func=Act.Relu)
        return hid

    for step in range(NPROP):
        hid = message_layer(hrm, cs["w1a"], cs["w1b"], cs["w1c"],
                            cs["w2a"], cs["w2b"], cs["b2r"], lrl=False)
        # L2 row-major per edge block + wide scatter
        paggT0 = pagg.tile([128, 512], f32, tag="ps_agg0")
        paggT1 = pagg.tile([128, 512], f32, tag="ps_agg1")
        for eb in range(12):
            pmf = pmsg.tile([128, 256], f32, tag="pa")
            pmb = pmsg.tile([128, 256], f32, tag="pa")
            ebs = slice(128*eb, 128*(eb+1))
            mm(pmf[:], hid[:, EL*0 + 128*eb: EL*0 + 128*(eb+1)], cs["w2a"][:],
               True, False, dt=f32r)
            mm(pmf[:], hid[:, EL*1 + 128*eb: EL*1 + 128*(eb+1)], cs["w2b"][:],
               False, False, dt=f32r)
            mm(pmf[:], ones1[:1, :128], cs["b2r"][:], False, True, dt=f32r)
            mm(pmb[:], hid[:, EL*2 + 128*eb: EL*2 + 128*(eb+1)], cs["w2a"][:],
               True, False, dt=f32r)
            mm(pmb[:], hid[:, EL*3 + 128*eb: EL*3 + 128*(eb+1)], cs["w2b"][:],
               False, False, dt=f32r)
            mm(pmb[:], ones1[:1, :128], cs["b2r"][:], False, True, dt=f32r)
            mf = stg.tile([128, 256], f32r, tag="mf")
            mb = stg.tile([128, 256], f32r, tag="mb")
            nc.scalar.activation(out=mf[:], in_=pmf[:], func=Act.Copy)
            nc.scalar.activation(out=mb[:], in_=pmb[:], func=Act.Copy)
            kts = slice(256*eb, 256*(eb+1))
            gp = (eb // 3) // 2               # group pair 0..1
            gs = slice(256*gp, 256*(gp+1))
            first = (eb % 6 == 0)
            last = (eb % 6 == 5)
            mm(paggT0[:, gs], mf[:, 0:128], cs["sT"][:, kts], first, False,
               dt=f32r)
            mm(paggT0[:, gs], mb[:, 0:128], cs["sF"][:, kts], False, last,
               dt=f32r)
            mm(paggT1[:, gs], mf[:, 128:256], cs["sT"][:, kts], first, False,
               dt=f32r)
            mm(paggT1[:, gs], mb[:, 128:256], cs["sF"][:, kts], False, last,
               dt=f32r)
        aggT0 = hpool.tile([128, 512], f32r, tag="aggT0")
        aggT1 = hpool.tile([128, 512], f32r, tag="aggT1")
        nc.scalar.activation(out=aggT0[:], in_=paggT0[:], func=Act.Copy)
        nc.scalar.activation(out=aggT1[:], in_=paggT1[:], func=Act.Copy)
        # update
        pnew = pbig.tile([128, 512], f32, tag="pa")
        mm(pnew[:], cs["wuaI"][:], hT[:], True, False, dt=f32r)
        mm(pnew[:], cs["wub_a"][:], aggT0[:], False, False, dt=f32r)
        mm(pnew[:], cs["wub_b"][:], aggT1[:], False, False, dt=f32r)
        mm(pnew[:], cs["bur"][:], ones1[:], False, True, dt=f32r)
        hT_new = hpool.tile([128, 512], f32r, tag="hTn")
        nc.scalar.activation(out=hT_new[:], in_=pnew[:], func=Act.Copy)
        hrm_new = hpool.tile([128, 512], f32r, tag="hrmn")
        for g in range(4):
            pt = psm.tile([128, 128], f32r, tag="ps_s")
            nc.tensor.transpose(pt[:], hT_new[:, 128*g:128*(g+1)], identr[:])
            nc.scalar.activation(out=hrm_new[:, 128*g:128*(g+1)], in_=pt[:],
                                 func=Act.Copy)
        hT, hrm = hT_new, hrm_new
    if dbg:
        nc.sync.dma_start(out=dbg["dbg_hT"][:], in_=hT[:])

    # ---------- phase 3: sk path (tqT/tcT [32, (8p,32n)])
    def h_cols(par):  # par=0 query, 1 corpus -> [128, (8p, 32n)] AP view
        v = hT[:].rearrange("p (g x n) -> p g x n", x=2, n=32)
        return v[:, :, par, :]

    tqT = persist.tile([32, 256], f32, tag="tqT")
    tcT = persist.tile([32, 256], f32, tag="tcT")
    for par, dst in ((0, tqT), (1, tcT)):
        p1 = psm.tile([32, 256], f32, tag="ps_s")
        mm(p1[:], cs["wsk1"][:], h_cols(par), True, False, dt=f32r)
        mm(p1[:], cs["bsk1r"][:], ones1[:1, :256], False, True, dt=f32r)
        s1 = stg.tile([32, 256], f32r, tag="sk_s1")
        nc.scalar.activation(out=s1[:], in_=p1[:], func=Act.Relu)
        p2 = psm.tile([32, 256], f32, tag="ps_s")
        mm(p2[:], cs["wsk2"][:], s1[:], True, False, dt=f32r)
        mm(p2[:], cs["bsk2r"][:], ones1[:1, :256], False, True, dt=f32r)
        nc.vector.tensor_copy(dst[:], p2[:])
    if dbg:
        nc.sync.dma_start(out=dbg["dbg_tqT"][:], in_=tqT[:])
        nc.sync.dma_start(out=dbg["dbg_tcT"][:], in_=tcT[:])

    # ---------- phase 4: node cost [32, (8p,32j)] via strips
    cost = persist.tile([32, 256], f32, tag="cost")

    def strip_cdist(out_tile, blk_of_ph, cols_of_ph, dpart, blk, nacc, tag, dt_, defer=False):
        """out_tile[i, blk*p+j] = sum_d |blk_of_ph(p,hh)[d,j] - cols_of_ph(p,hh)[d,i]|.

        Wide-batched: per (p, hh, 16-i chunk): one TT subtract + one STT abs over
        [dpart, 16*blk], then 4 strip matmuls (tile_position rotation c=0..3,
        4 i-strips each) reduce over d into PSUM rows {0,32,64,96}; evacuate
        via full-tile copy + strided SBUF->SBUF DMA.
        """
        ones_l = ones128b if dt_ == bf16 else (ones32f if dpart == 32 else ones128f)
        units = []
        for p in range(BL):
            for ib in range(6 if blk == 96 else 2):
                units.append((p, ib))
        closures = []
        def make_unit(p, ib):
            def unit():
                pstr = psm.tile([128, 4 * blk], f32, tag="ps_s")
                st0 = stg2.tile([dpart, 16 * blk], dt_, tag=tag + "_s0")
                st1 = None
                srcs = [st0]
                if nacc == 2:
                    st1 = stg2.tile([dpart, 16 * blk], dt_, tag=tag + "_s1")
                    srcs.append(st1)
                for hh in range(nacc):
                    stt_t = srcs[hh]
                    blk_ap = blk_of_ph(p, hh)
                    cols_ap = cols_of_ph(p, hh, 16 * ib, 16)
                    in0 = bass.AP(tensor=blk_ap.tensor, offset=blk_ap.offset,
                                  ap=[blk_ap.ap[0], [0, 16]] + list(blk_ap.ap[1:]))
                    in1 = bass.AP(tensor=cols_ap.tensor, offset=cols_ap.offset,
                                  ap=list(cols_ap.ap) + [[0, blk]])
                    v3 = stt_t[:].rearrange("p (i j) -> p i j", j=blk)
                    nc.vector.tensor_tensor(out=v3, in0=in0, in1=in1,
                                            op=Alu.subtract)
                    nc.vector.scalar_tensor_tensor(
                        out=v3, in0=v3, scalar=-1.0, in1=v3,
                        op0=Alu.mult, op1=Alu.max)
                for c in range(4):
                    cs_ = slice(4 * blk * c, 4 * blk * (c + 1))
                    mm(pstr[32*c:32*c+1, :], ones_l[:], st0[:, cs_],
                       True, nacc == 1, tile_position=(0, 32*c))
                    if nacc == 2:
                        mm(pstr[32*c:32*c+1, :], ones_l[:], st1[:, cs_],
                           False, True, tile_position=(0, 32*c))
                s2 = stg2.tile([128, 4 * blk], f32, tag=tag + "_s2")
                nc.vector.tensor_copy(s2[:], pstr[:])
                sv = s2[:]
                iv = bass.AP(tensor=sv.tensor, offset=sv.offset,
                             ap=[[32 * sv.ap[0][0], 4], [blk, 4], [1, blk]])
                nc.sync.dma_start(
                    out=out_tile[16*ib:16*(ib+1), blk*p:blk*(p+1)], in_=iv)
            return unit
        for (p, ib) in units:
            closures.append(make_unit(p, ib))
        if defer:
            return closures
        for cl in closures:
            cl()

    strip_cdist(cost,
                blk_of_ph=lambda p, hh: tcT[:, 32*p:32*(p+1)],
                cols_of_ph=lambda p, hh, i0, ni: tqT[:, 32*p+i0:32*p+i0+ni],
                dpart=32, blk=32, nacc=1, tag="nc", dt_=f32)
    if dbg:
        nc.sync.dma_start(out=dbg["dbg_cost"][:], in_=cost[:])

    # ---------- phase 8/9: lrl embeddings + edge cdist D [96, (8p,96j)]
    hidL = message_layer(hrm, cs["wl1a"], cs["wl1b"], cs["wl1c"],
                         cs["wl2a"], cs["wl2b"], cs["bl2r"], lrl=True)
    bid0b = persist.tile([128, EL], bf16, tag="bid0b")
    bid1b = persist.tile([128, EL], bf16, tag="bid1b")
    for mt, dst in ((0, bid0b), (1, bid1b)):
        for n in range(3):
            pb2 = pbig.tile([128, 512], f32, tag="pa")
            ns = slice(512*n, 512*(n+1))
            mm(pb2[:], cs["wl2a"][:, 128*mt:128*(mt+1)],
               hidL[:, EL*0 + 512*n: EL*0 + 512*(n+1)], True, False, dt=f32r)
            mm(pb2[:], cs["wl2b"][:, 128*mt:128*(mt+1)],
               hidL[:, EL*1 + 512*n: EL*1 + 512*(n+1)], False, False, dt=f32r)
            mm(pb2[:], cs["wl2a"][:, 128*mt:128*(mt+1)],
               hidL[:, EL*2 + 512*n: EL*2 + 512*(n+1)], False, False, dt=f32r)
            mm(pb2[:], cs["wl2b"][:, 128*mt:128*(mt+1)],
               hidL[:, EL*3 + 512*n: EL*3 + 512*(n+1)], False, False, dt=f32r)
            mm(pb2[:], cs["bl2r"][:, 128*mt:128*(mt+1)], ones1[:], False, True)
            nc.scalar.activation(out=dst[:, ns], in_=pb2[:], func=Act.Copy)

    # ---------- phase 5: node sinkhorn, log space
    la = persist.tile([32, 256], f32, tag="la")
    nc.vector.tensor_scalar(out=la[:], in0=cost[:], scalar1=float(-1.0/TEMP),
                            scalar2=None, op0=Alu.mult)

    def ns_norm_step(t):
        """log-space normalize along each 32-wide free block of t [32, 256]."""
        t3 = t[:].rearrange("p (b j) -> p b j", j=32)
        rm = stg.tile([32, 8], f32, tag="ns_rm")
        nc.vector.tensor_reduce(out=rm[:], in_=t3, axis=AX.X, op=Alu.max,
                                negate=True)
        tmp = stg.tile([32, 256], f32, tag="ns_tmp")
        nc.vector.scalar_tensor_tensor(
            out=tmp[:].rearrange("p (b j) -> p b j", j=32), in0=t3, scalar=1.0,
            in1=bcast_in(rm[:], 32), op0=Alu.mult, op1=Alu.add)
        ex = stg.tile([32, 256], f32, tag="ns_ex")
        nc.scalar.activation(out=ex[:], in_=tmp[:], func=Act.Exp)
        sm = stg.tile([32, 8], f32, tag="ns_sm")
        nc.vector.tensor_reduce(out=sm[:], in_=ex[:].rearrange(
            "p (b j) -> p b j", j=32), axis=AX.X, op=Alu.add)
        ls = stg.tile([32, 8], f32, tag="ns_ls")
        nc.scalar.activation(out=ls[:], in_=sm[:], func=Act.Ln)
        lse = stg.tile([32, 8], f32, tag="ns_lse")
        nc.vector.tensor_tensor(out=lse[:], in0=ls[:], in1=rm[:], op=Alu.subtract)
        nc.vector.scalar_tensor_tensor(
            out=t3, in0=t3, scalar=1.0,
            in1=bcast_in(lse[:], 32), op0=Alu.mult, op1=Alu.subtract)

    ncd = persist.tile([32, 256], f32, tag="ncd")
    hTb = persist.tile([128, 512], bf16, tag="hTb")
    nc.vector.tensor_copy(hTb[:], hT[:])
    na_units = strip_cdist(ncd,
                blk_of_ph=lambda p, hh: hTb[:, 64*p+32:64*p+64],
                cols_of_ph=lambda p, hh, i0, ni: hTb[:, 64*p+i0:64*p+i0+ni],
                dpart=128, blk=32, nacc=1, tag="na", dt_=bf16, defer=True)

    lat = persist.tile([32, 256], f32, tag="lat")
    for it in range(ITERS):
        ns_norm_step(la)                      # row step
        nc.vector.transpose(lat[:], la[:])    # per-pair 32x32 block transpose
        if na_units:
            na_units.pop(0)()
        ns_norm_step(lat)                     # col step (rows of transposed)
        nc.vector.transpose(la[:], lat[:])
    while na_units:
        na_units.pop(0)()
    nplan = persist.tile([32, 256], f32, tag="nplan")
    nc.scalar.activation(out=nplan[:], in_=la[:], func=Act.Exp)
    if dbg:
        nc.sync.dma_start(out=dbg["dbg_nplan"][:], in_=nplan[:])

    # ---------- phase 6: kron -> M0 edge [96, (8p,96j)]
    Me = persist.tile([96, 768], f32, tag="Me")
    for p in range(BL):
        Pp = nplan[:, 32*p:32*(p+1)]
        put = psm.tile([32, 96], f32, tag="ps_s")
        pvt = psm.tile([32, 96], f32, tag="ps_s")
        mm(put[:], Pp, cs["kfq"][:, 96*p:96*(p+1)], True, True)
        mm(pvt[:], Pp, cs["ktq"][:, 96*p:96*(p+1)], True, True)
        ut = stg.tile([32, 96], f32, tag="kr_ut")
        vt = stg.tile([32, 96], f32, tag="kr_vt")
        nc.vector.tensor_copy(ut[:], put[:])
        nc.vector.tensor_copy(vt[:], pvt[:])
        pA = psm.tile([96, 96], f32, tag="ps_s")
        pB = psm.tile([96, 96], f32, tag="ps_s")
        mm(pA[:], ut[:], cs["kfc"][:, 96*p:96*(p+1)], True, True)
        mm(pB[:], vt[:], cs["ktc"][:, 96*p:96*(p+1)], True, True)
        sA = stg.tile([96, 96], f32, tag="kr_sA")
        nc.scalar.activation(out=sA[:], in_=pA[:], func=Act.Copy)
        straight = stg.tile([96, 96], f32, tag="kr_str")
        nc.vector.tensor_tensor(out=straight[:], in0=sA[:], in1=pB[:], op=Alu.mult)
        pC = psm.tile([96, 96], f32, tag="ps_s")
        pD = psm.tile([96, 96], f32, tag="ps_s")
        mm(pC[:], ut[:], cs["ktc"][:, 96*p:96*(p+1)], True, True)
        mm(pD[:], vt[:], cs["kfc"][:, 96*p:96*(p+1)], True, True)
        sC = stg.tile([96, 96], f32, tag="kr_sC")
        nc.scalar.activation(out=sC[:], in_=pC[:], func=Act.Copy)
        cross = stg.tile([96, 96], f32, tag="kr_crs")
        nc.vector.tensor_tensor(out=cross[:], in0=sC[:], in1=pD[:], op=Alu.mult)
        gmax = stg.tile([96, 96], f32, tag="kr_gmax")
        nc.vector.tensor_tensor(out=gmax[:], in0=straight[:], in1=cross[:],
                                op=Alu.max)
        nc.scalar.activation(out=Me[:, 96*p:96*(p+1)], in_=gmax[:], func=Act.Exp,
                             scale=float(1.0/TEMP))
    if dbg:
        nc.sync.dma_start(out=dbg["dbg_M0"][:], in_=Me[:])

    # ---------- phase 7: edge sinkhorn (multiplicative) + interleaved cdist
    D = persist.tile([96, 768], f32, tag="D")
    ec_units = strip_cdist(D,
                blk_of_ph=lambda p, hh: (bid0b if hh == 0 else bid1b)[:, 192*p+96:192*p+192],
                cols_of_ph=lambda p, hh, i0, ni: (bid0b if hh == 0 else bid1b)
                    [:, 192*p+i0:192*p+i0+ni],
                dpart=128, blk=96, nacc=2, tag="ec", dt_=bf16, defer=True)
    # Lazy row normalization: stored Me is only col-normalized; the current
    # row factors rr (= 1/rowsum(Me)) are folded into the colsum stationary
    # each iteration and into the final plan*D dot, saving one [96,768]
    # DVE pass per iteration.
    Me3 = Me[:].rearrange("p (b j) -> p b j", j=96)
    rr = persist.tile([96, 8], f32, tag="es_rr")
    for it in range(ITERS):
        rs = stg.tile([96, 8], f32, tag="es_rs")
        nc.vector.tensor_reduce(out=rs[:], in_=Me3, axis=AX.X, op=Alu.add)
        nc.vector.reciprocal(out=rr[:], in_=rs[:])
        rc = stg1.tile([96, 768], f32, tag="big768")
        pcs_l = []
        for hh in range(2):
            pcs = psm.tile([96, 384], f32, tag="ps_s")
            for q in range(4):
                pp = 4*hh + q
                mm(pcs[:, 96*q:96*(q+1)], rr[:, pp:pp+1].to_broadcast((96, 96)),
                   Me[:, 96*pp:96*(pp+1)], True, True)
            pcs_l.append(pcs)
        for _ in range(2):
            if ec_units:
                ec_units.pop(0)()
        for hh in range(2):
            nc.vector.reciprocal_approx_fast(out=rc[:, 384*hh:384*(hh+1)],
                                             in_=pcs_l[hh][:])
        nc.vector.tensor_tensor(out=Me[:], in0=Me[:], in1=rc[:], op=Alu.mult)
    while ec_units:
        ec_units.pop(0)()
    if dbg:
        nc.vector.scalar_tensor_tensor(
            out=Me3, in0=Me3, scalar=1.0, in1=bcast_in(rr[:], 96),
            op0=Alu.mult, op1=Alu.mult)
        nc.sync.dma_start(out=dbg["dbg_eplan"][:], in_=Me[:])
        nc.sync.dma_start(out=dbg["dbg_D"][:], in_=D[:])


    # (ncd computed interleaved with node sinkhorn above)
    if dbg:
        nc.sync.dma_start(out=dbg["dbg_ncd"][:], in_=ncd[:])

    # ---------- phase 11: dots + output
    we = stg1.tile([96, 768], f32, tag="big768")
    nc.vector.tensor_tensor(out=we[:], in0=Me[:], in1=D[:], op=Alu.mult)
    ep = stg.tile([96, 8], f32, tag="dot_ep")
    nc.vector.tensor_reduce(out=ep[:], in_=we[:].rearrange(
        "p (b j) -> p b j", j=96), axis=AX.X, op=Alu.add)
    nc.vector.tensor_tensor(out=ep[:], in0=ep[:], in1=rr[:], op=Alu.mult)
    wn = stg.tile([32, 256], f32, tag="dot_wn")
    nc.vector.tensor_tensor(out=wn[:], in0=nplan[:], in1=ncd[:], op=Alu.mult)
    np_ = stg.tile([32, 8], f32, tag="dot_np")
    nc.vector.tensor_reduce(out=np_[:], in_=wn[:].rearrange(
        "p (b j) -> p b j", j=32), axis=AX.X, op=Alu.add)
    pout = psm.tile([1, 8], f32, tag="ps_s")
    mm(pout[:], ones96sq[:, 0:1], ep[:], True, False)
    mm(pout[:], ones32f[:], np_[:], False, True)
    osb = stg.tile([1, 8], f32, tag="osb")
    nc.vector.tensor_copy(osb[:], pout[:])
    nc.sync.dma_start(out=out_ext[:], in_=osb[:])

    ctx.close()


# ----------------------------------------------------------------- entry
def _get_nc(debug=False):
    key = ("nc", debug)
    if key not in _CACHE:
        _CACHE[key] = _build(debug=debug)
    return _CACHE[key]


def run_cores(inputs, debug=False, trace=False):
    from concourse.bass_utils import run_bass_kernel_spmd
    nc = _get_nc(debug=debug)
    in_maps = _host_prep(inputs)
    res = run_bass_kernel_spmd(nc, in_maps, core_ids=list(range(NCORE)),
                               trace=trace)
    return res


def kernel(**inputs):
    res = run_cores(inputs, debug=False, trace=False)
    out = np.concatenate([r["out"].reshape(-1) for r in res.results])
    return out.astype(np.float32)



# revision 27
# speedup vs baseline: 1.6938x; 1.3128x over previous
"""Trainium2 Bass kernel for nn_ABL_SPARSE_87694642250045 (GMN graph matching).

Data-parallel over B=64 graph pairs: 8 pairs (16 graphs) per NeuronCore, 8 cores.
No collectives — output is per-pair scalars, concatenated host-side.

Device decomposition (per core):
  - gathers/segment-sums as one-hot matmuls (one-hots precomputed host-side)
  - message MLP with W-swap trick: both directions in one [*,512] hidden
  - residual update folded into (W_upd_a + I)
  - node sinkhorn in log space (PE transposes for column steps)
  - edge sinkhorn multiplicative (column sums via ones-matmul, no transposes)
  - L1 cdists: tensor_scalar |a-b| with d-on-partitions + ones-matmul reduce
    (strips) + tile_position rotation + SBUF DMA reshape
"""
import numpy as np

NCORE = 8
B, N, E = 64, 32, 96
NPROP, TEMP, ITERS = 5, 0.1, 20
BL = B // NCORE          # 8 pairs / core
GL = 2 * BL              # 16 graphs / core
VL = GL * N              # 512 nodes / core
EL = GL * E              # 1536 edges / core

_CACHE = {}


# ----------------------------------------------------------------- host prep
def _onehot(idx, n):
    out = np.zeros((len(idx), n), np.float32)
    out[np.arange(len(idx)), idx] = 1.0
    return out


def _host_prep(inputs):
    f32 = np.float32
    nf = np.asarray(inputs["node_features"], f32)
    ef = np.asarray(inputs["edge_features"], f32)
    fr_all = np.asarray(inputs["from_idx"]).astype(np.int64)
    to_all = np.asarray(inputs["to_idx"]).astype(np.int64)

    W_enc = np.asarray(inputs["W_enc"], f32); b_enc = np.asarray(inputs["b_enc"], f32)
    W1 = np.asarray(inputs["W_msg1"], f32); b1 = np.asarray(inputs["b_msg1"], f32)
    W2 = np.asarray(inputs["W_msg2"], f32); b2 = np.asarray(inputs["b_msg2"], f32)
    Wu = np.asarray(inputs["W_upd"], f32); bu = np.asarray(inputs["b_upd"], f32)
    Wsk1 = np.asarray(inputs["W_sk1"], f32); bsk1 = np.asarray(inputs["b_sk1"], f32)
    Wsk2 = np.asarray(inputs["W_sk2"], f32); bsk2 = np.asarray(inputs["b_sk2"], f32)
    Wl1 = np.asarray(inputs["W_lrl1"], f32); bl1 = np.asarray(inputs["b_lrl1"], f32)
    Wl2 = np.asarray(inputs["W_lrl2"], f32); bl2 = np.asarray(inputs["b_lrl2"], f32)

    def ext(Wm, bm):
        Wswap = np.concatenate([Wm[128:256], Wm[0:128], Wm[256:257]], axis=0)
        Wcat = np.concatenate([Wm, Wswap], axis=1)               # [257,512]
        bcat = np.concatenate([bm, bm])[None]                    # [1,512]
        return np.ascontiguousarray(np.concatenate([Wcat, bcat], axis=0))  # [258,512]

    W1ext = ext(W1, b1)
    Wl1ext = ext(Wl1, bl1)

    shared = {
        "w1a": W1ext[0:128], "w1b": W1ext[128:256], "w1c": W1ext[256:258],
        "wl1a": Wl1ext[0:128], "wl1b": Wl1ext[128:256], "wl1c": Wl1ext[256:258],
        "w2a": W2[0:128], "w2b": W2[128:256],
        "wl2a": Wl2[0:128], "wl2b": Wl2[128:256],
        "b2r": b2[None], "bl2r": (2.0 * bl2)[None],
        "wuaI": Wu[0:128] + np.eye(128, dtype=f32),
        "wub_a": Wu[128:256], "wub_b": Wu[256:384], "bur": bu[None],
        "wenc": W_enc, "bencr": b_enc[None],
        "wsk1": Wsk1, "bsk1r": bsk1[None], "wsk2": Wsk2, "bsk2r": bsk2[None],
    }
    shared = {k: np.ascontiguousarray(v, f32) for k, v in shared.items()}
    import ml_dtypes
    for k in ("w1c", "wl1c"):
        shared[k] = shared[k].astype(ml_dtypes.bfloat16)

    in_maps = []
    for c in range(NCORE):
        nfc = nf[c*VL:(c+1)*VL]                                  # [512,32]
        efc = ef[c*EL:(c+1)*EL]                                  # [1536,1]
        fr = fr_all[c*EL:(c+1)*EL] - c*VL
        to = to_all[c*EL:(c+1)*EL] - c*VL

        gfT = np.zeros((128, EL), f32)
        gtT = np.zeros((128, EL), f32)
        for g in range(4):
            e0, v0 = 384*g, 128*g
            gfT[:, e0:e0+384] = _onehot(fr[e0:e0+384] - v0, 128).T
            gtT[:, e0:e0+384] = _onehot(to[e0:e0+384] - v0, 128).T

        # scatter one-hots, 256-wide (group-pair local) so the scatter matmul
        # free dim is >=256 and f32r runs at 1 cycle/row
        sT = np.zeros((128, 12*256), f32)
        sF = np.zeros((128, 12*256), f32)
        for kt in range(12):
            e0, g = 128*kt, kt // 3
            gp0 = (g // 2) * 2                # group-pair base group
            sT[:, 256*kt:256*(kt+1)] = _onehot(to[e0:e0+128] - 128*gp0, 256)
            sF[:, 256*kt:256*(kt+1)] = _onehot(fr[e0:e0+128] - 128*gp0, 256)

        frg = fr.reshape(GL, E) - (np.arange(GL) * N)[:, None]
        tog = to.reshape(GL, E) - (np.arange(GL) * N)[:, None]
        kfq = np.zeros((32, BL*E), f32); ktq = np.zeros((32, BL*E), f32)
        kfc = np.zeros((32, BL*E), f32); ktc = np.zeros((32, BL*E), f32)
        for p in range(BL):
            s = slice(E*p, E*(p+1))
            kfq[:, s] = _onehot(frg[2*p], N).T
            ktq[:, s] = _onehot(tog[2*p], N).T
            kfc[:, s] = _onehot(frg[2*p+1], N).T
            ktc[:, s] = _onehot(tog[2*p+1], N).T

        e1 = np.concatenate([efc.T, np.ones((1, EL), f32)], axis=0)  # [2,1536]

        import ml_dtypes
        m = dict(shared)
        m.update({
            "nfT": np.ascontiguousarray(nfc.T),      # [32,512]
            "e1": np.ascontiguousarray(e1).astype(ml_dtypes.bfloat16),
            "gfT": gfT, "gtT": gtT, "sT": sT, "sF": sF,
            "kfq": kfq, "ktq": ktq, "kfc": kfc, "ktc": ktc,
        })
        in_maps.append(m)
    return in_maps


# --------------------------------------------------------------- bass builder
def _build(debug=False):
    import concourse.bass as bass
    import concourse.bacc as bacc
    import concourse.mybir as mybir
    import concourse.tile as tile
    from concourse.masks import make_identity

    f32 = mybir.dt.float32
    bf16 = mybir.dt.bfloat16
    f32r = mybir.dt.float32r
    Alu = mybir.AluOpType
    Act = mybir.ActivationFunctionType
    AX = mybir.AxisListType

    nc = bacc.Bacc("TRN2", target_bir_lowering=False)

    # ---- dram declarations
    dr = {}
    decls = {
        "nfT": (32, VL), "e1": (2, EL), "gfT": (128, EL), "gtT": (128, EL),
        "sT": (128, 12*256), "sF": (128, 12*256),
        "kfq": (32, BL*E), "ktq": (32, BL*E), "kfc": (32, BL*E), "ktc": (32, BL*E),
        "w1a": (128, 512), "w1b": (128, 512), "w1c": (2, 512),
        "wl1a": (128, 512), "wl1b": (128, 512), "wl1c": (2, 512),
        "w2a": (128, 256), "w2b": (128, 256), "wl2a": (128, 256), "wl2b": (128, 256),
        "b2r": (1, 256), "bl2r": (1, 256),
        "wuaI": (128, 128), "wub_a": (128, 128), "wub_b": (128, 128), "bur": (1, 128),
        "wenc": (32, 128), "bencr": (1, 128),
        "wsk1": (128, 32), "bsk1r": (1, 32), "wsk2": (32, 32), "bsk2r": (1, 32),
    }
    f32r_names = set(['wl1a', 'wl1b', 'wl1c', 'wl2a', 'wl2b', 'bl2r',
                      'w1a', 'w1b', 'w1c', 'w2a', 'w2b', 'b2r',
                      'wuaI', 'wub_a', 'wub_b', 'bur', 'gfT', 'gtT',
                      'wsk1', 'bsk1r', 'wsk2', 'bsk2r', 'sT', 'sF'])
    bf16_names = set(['w1c', 'wl1c', 'e1'])
    for k, shp in decls.items():
        dt_ = bf16 if k in bf16_names else (f32r if k in f32r_names else f32)
        dr[k] = nc.declare_dram_parameter(k, list(shp), dt_, isOutput=False)
    out_ext = nc.declare_dram_parameter("out", [1, BL], f32, isOutput=True)
    dbg = {}
    if debug:
        for k, shp in {
            "dbg_hT0": (128, 512), "dbg_hT": (128, 512), "dbg_tqT": (32, 256),
            "dbg_tcT": (32, 256), "dbg_cost": (32, 256), "dbg_nplan": (32, 256),
            "dbg_M0": (96, 768), "dbg_eplan": (96, 768), "dbg_D": (96, 768),
            "dbg_ncd": (32, 256),
        }.items():
            dbg[k] = nc.declare_dram_parameter(k, list(shp), f32, isOutput=True)
        for k, shp in {"dbg_bid0": (128, EL), "dbg_bid1": (128, EL)}.items():
            dbg[k] = nc.declare_dram_parameter(k, list(shp), mybir.dt.bfloat16,
                                               isOutput=True)

    with tile.TileContext(nc) as tc:
        _emit(nc, tc, dr, out_ext, dbg, f32, bf16, f32r, Alu, Act, AX, make_identity)

    # Pin Exp/Ln to the one activation table that holds both so the
    # table-placement fixpoint hoists a single load instead of reloading on
    # every Exp<->Ln alternation in the sinkhorn loop. Table ids are
    # preserved (only the advertised function sets shrink).
    import concourse.bacc as bacc_mod
    orig_tables = bacc_mod.get_activation_tables

    def pinned_tables(arch):
        tabs = orig_tables(arch)
        both = "natural_log_exp_and_others"
        exp_f = Act.Exp
        ln_f = Act.Ln
        if both in tabs and exp_f in tabs[both] and ln_f in tabs[both]:
            tabs = {
                name: (s if name == both
                       else {f for f in s if f not in (exp_f, ln_f)})
                for name, s in tabs.items()
            }
        return tabs

    bacc_mod.get_activation_tables = pinned_tables
    try:
        nc.compile()
    finally:
        bacc_mod.get_activation_tables = orig_tables
    return nc


def _emit(nc, tc, dr, out_ext, dbg, f32, bf16, f32r, Alu, Act, AX, make_identity):
    import concourse.bass as bass
    from contextlib import ExitStack

    ctx = ExitStack()
    const = ctx.enter_context(tc.tile_pool(name="const", bufs=1))
    persist = ctx.enter_context(tc.tile_pool(name="persist", bufs=1))
    wrk = ctx.enter_context(tc.tile_pool(name="wrk", bufs=1))
    hpool = ctx.enter_context(tc.tile_pool(name="hpool", bufs=2))
    hidp = ctx.enter_context(tc.tile_pool(name="hidp", bufs=1))
    stg = ctx.enter_context(tc.tile_pool(name="stg", bufs=3))
    stg2 = ctx.enter_context(tc.tile_pool(name="stg2", bufs=2))
    stg1 = ctx.enter_context(tc.tile_pool(name="stg1", bufs=1))
    pbig = ctx.enter_context(tc.tile_pool(name="pbig", bufs=3, space="PSUM"))
    pmsg = pbig
    pagg = ctx.enter_context(tc.tile_pool(name="pagg", bufs=1, space="PSUM"))
    psm = ctx.enter_context(tc.tile_pool(name="psm", bufs=3, space="PSUM"))

    def mm(out, lhsT, rhs, start, stop, dt=None, tile_position=None):
        if dt is not None:
            lhsT = lhsT.bitcast(dt)
            rhs = rhs.bitcast(dt)
        nc.tensor.matmul(out, lhsT, rhs, start=start, stop=stop,
                         tile_position=tile_position)

    def bcast_in(ap, n):
        # [P, F] -> [P, F, n] with stride-0 inner free dim
        a = ap
        return bass.AP(tensor=a.tensor, offset=a.offset,
                       ap=list(a.ap) + [[0, n]])

    # ---------- constants to SBUF
    cs = {}
    for k, shp in {
        "nfT": (32, VL), "e1": (2, EL), "gfT": (128, EL), "gtT": (128, EL),
        "sT": (128, 12*256), "sF": (128, 12*256),
        "kfq": (32, BL*E), "ktq": (32, BL*E), "kfc": (32, BL*E), "ktc": (32, BL*E),
        "w1a": (128, 512), "w1b": (128, 512), "w1c": (2, 512),
        "wl1a": (128, 512), "wl1b": (128, 512), "wl1c": (2, 512),
        "w2a": (128, 256), "w2b": (128, 256), "wl2a": (128, 256), "wl2b": (128, 256),
        "b2r": (1, 256), "bl2r": (1, 256),
        "wuaI": (128, 128), "wub_a": (128, 128), "wub_b": (128, 128), "bur": (1, 128),
        "wenc": (32, 128), "bencr": (1, 128),
        "wsk1": (128, 32), "bsk1r": (1, 32), "wsk2": (32, 32), "bsk2r": (1, 32),
    }.items():
        if k in ('w1c', 'wl1c', 'e1'):
            dt_ = bf16
        elif k in ['wl1a', 'wl1b', 'wl2a', 'wl2b', 'bl2r',
                   'w1a', 'w1b', 'w2a', 'w2b', 'b2r',
                   'wuaI', 'wub_a', 'wub_b', 'bur', 'gfT', 'gtT',
                   'wsk1', 'bsk1r', 'wsk2', 'bsk2r', 'sT', 'sF']:
            dt_ = f32r
        else:
            dt_ = f32
        t = const.tile(list(shp), dt_, tag=k)
        nc.sync.dma_start(out=t[:], in_=dr[k][:])
        cs[k] = t

    e1r = cs["e1"]
    identf = const.tile([128, 128], f32, tag="identf")
    make_identity(nc, identf[:])
    ones96sq = const.tile([96, 96], f32, tag="ones96sq")
    nc.vector.memset(ones96sq[:], 1.0)
    identr = const.tile([128, 128], f32r, tag="identr")
    nc.vector.tensor_copy(identr[:], identf[:])
    ones1f = const.tile([1, 512], f32, tag="ones1f")
    nc.vector.memset(ones1f[:], 1.0)
    ones1 = const.tile([1, 512], f32r, tag="ones1")
    nc.vector.tensor_copy(ones1[:], ones1f[:])
    ones128f = const.tile([128, 1], f32, tag="ones128f")
    nc.vector.memset(ones128f[:], 1.0)
    ones128r = const.tile([128, 1], f32r, tag="ones128r")
    nc.vector.tensor_copy(ones128r[:], ones128f[:])
    ones128b = const.tile([128, 1], bf16, tag="ones128b")
    nc.vector.memset(ones128b[:], 1.0)
    twos128b = const.tile([128, 1], bf16, tag="twos128b")
    nc.vector.memset(twos128b[:], 2.0)
    negones11 = const.tile([1, 1], f32, tag="negones11")
    nc.vector.memset(negones11[:], -1.0)
    ones32 = const.tile([32, 1], f32r, tag="ones32")
    nc.vector.tensor_copy(ones32[:], ones128f[:32, :])
    ones32f = const.tile([32, 1], f32, tag="ones32f")
    nc.vector.memset(ones32f[:], 1.0)

    # ---------- phase 1: encoder -> hT [128,512], hrm [128,(4g,128f)]
    hT = persist.tile([128, 512], f32r, tag="hT")
    hrm = persist.tile([128, 512], f32r, tag="hrm")

    ps = pbig.tile([128, 512], f32, tag="pa")
    mm(ps[:], cs["wenc"][:], cs["nfT"][:], start=True, stop=False)
    mm(ps[:], cs["bencr"][:], ones1f[:], start=False, stop=True)
    nc.scalar.activation(out=hT[:], in_=ps[:], func=Act.Copy)
    for g in range(4):
        psg = psm.tile([128, 128], f32, tag="ps_s")
        mm(psg[:], cs["nfT"][:, 128*g:128*(g+1)], cs["wenc"][:],
           start=True, stop=False)
        mm(psg[:], ones1f[:1, :128], cs["bencr"][:], start=False, stop=True)
        nc.vector.tensor_copy(hrm[:, 128*g:128*(g+1)], psg[:])
    if dbg:
        nc.sync.dma_start(out=dbg["dbg_hT0"][:], in_=hT[:].bitcast(f32))

    # ---------- phase 2: propagation steps
    def message_layer(hrm_t, wa, wb, wc2, w2_a, w2_b, b2row, lrl):
        """gathers + L1; returns hid tile [128, 4*1536] (mtile m at cols 1536m)"""
        mdt = f32r
        tdt = f32r
        e1t = cs["e1"]
        srcT = wrk.tile([128, EL], tdt, tag="srcT")
        dstT = wrk.tile([128, EL], tdt, tag="dstT")
        for g in range(4):
            psrc = pmsg.tile([128, 384], f32, tag="pa")
            pdst = pmsg.tile([128, 384], f32, tag="pa")
            hg = hrm_t[:, 128*g:128*(g+1)]
            mm(psrc[:], hg, cs["gfT"][:, 384*g:384*(g+1)], start=True, stop=True,
               dt=f32r)
            mm(pdst[:], hg, cs["gtT"][:, 384*g:384*(g+1)], start=True, stop=True,
               dt=f32r)
            nc.scalar.activation(out=srcT[:, 384*g:384*(g+1)], in_=psrc[:],
                                 func=Act.Copy)
            nc.scalar.activation(out=dstT[:, 384*g:384*(g+1)], in_=pdst[:],
                                 func=Act.Copy)
        hid = hidp.tile([128, 4*EL], tdt, tag="hid")
        for m in range(4):
            for n in range(3):
                ph = pbig.tile([128, 512], f32, tag="pa")
                ns = slice(512*n, 512*(n+1))
                mm(ph[:], wa[:, 128*m:128*(m+1)], srcT[:, ns], True, False, dt=mdt)
                mm(ph[:], wb[:, 128*m:128*(m+1)], dstT[:, ns], False, False, dt=mdt)
                mm(ph[:], wc2[:, 128*m:128*(m+1)], e1t[:, ns], False, True)
                dst_ap = hid[:, EL*m + 512*n: EL*m + 512*(n+1)]
                nc.scalar.activation(out=dst_ap, in_=ph[:], func=Act.Relu)
        return hid

    for step in range(NPROP):
        hid = message_layer(hrm, cs["w1a"], cs["w1b"], cs["w1c"],
                            cs["w2a"], cs["w2b"], cs["b2r"], lrl=False)
        # L2 row-major per edge block + wide scatter
        paggT0 = pagg.tile([128, 512], f32, tag="ps_agg0")
        paggT1 = pagg.tile([128, 512], f32, tag="ps_agg1")
        for eb in range(12):
            pmf = pmsg.tile([128, 256], f32, tag="pa")
            pmb = pmsg.tile([128, 256], f32, tag="pa")
            ebs = slice(128*eb, 128*(eb+1))
            mm(pmf[:], hid[:, EL*0 + 128*eb: EL*0 + 128*(eb+1)], cs["w2a"][:],
               True, False, dt=f32r)
            mm(pmf[:], hid[:, EL*1 + 128*eb: EL*1 + 128*(eb+1)], cs["w2b"][:],
               False, False, dt=f32r)
            mm(pmf[:], ones1[:1, :128], cs["b2r"][:], False, True, dt=f32r)
            mm(pmb[:], hid[:, EL*2 + 128*eb: EL*2 + 128*(eb+1)], cs["w2a"][:],
               True, False, dt=f32r)
            mm(pmb[:], hid[:, EL*3 + 128*eb: EL*3 + 128*(eb+1)], cs["w2b"][:],
               False, False, dt=f32r)
            mm(pmb[:], ones1[:1, :128], cs["b2r"][:], False, True, dt=f32r)
            mf = stg.tile([128, 256], f32r, tag="mf")
            mb = stg.tile([128, 256], f32r, tag="mb")
            nc.scalar.activation(out=mf[:], in_=pmf[:], func=Act.Copy)
            nc.scalar.activation(out=mb[:], in_=pmb[:], func=Act.Copy)
            kts = slice(256*eb, 256*(eb+1))
            gp = (eb // 3) // 2               # group pair 0..1
            gs = slice(256*gp, 256*(gp+1))
            first = (eb % 6 == 0)
            last = (eb % 6 == 5)
            mm(paggT0[:, gs], mf[:, 0:128], cs["sT"][:, kts], first, False,
               dt=f32r)
            mm(paggT0[:, gs], mb[:, 0:128], cs["sF"][:, kts], False, last,
               dt=f32r)
            mm(paggT1[:, gs], mf[:, 128:256], cs["sT"][:, kts], first, False,
               dt=f32r)
            mm(paggT1[:, gs], mb[:, 128:256], cs["sF"][:, kts], False, last,
               dt=f32r)
        aggT0 = hpool.tile([128, 512], f32r, tag="aggT0")
        aggT1 = hpool.tile([128, 512], f32r, tag="aggT1")
        nc.scalar.activation(out=aggT0[:], in_=paggT0[:], func=Act.Copy)
        nc.scalar.activation(out=aggT1[:], in_=paggT1[:], func=Act.Copy)
        # update
        pnew = pbig.tile([128, 512], f32, tag="pa")
        mm(pnew[:], cs["wuaI"][:], hT[:], True, False, dt=f32r)
        mm(pnew[:], cs["wub_a"][:], aggT0[:], False, False, dt=f32r)
        mm(pnew[:], cs["wub_b"][:], aggT1[:], False, False, dt=f32r)
        mm(pnew[:], cs["bur"][:], ones1[:], False, True, dt=f32r)
        hT_new = hpool.tile([128, 512], f32r, tag="hTn")
        nc.scalar.activation(out=hT_new[:], in_=pnew[:], func=Act.Copy)
        hrm_new = hpool.tile([128, 512], f32r, tag="hrmn")
        for g in range(4):
            pt = psm.tile([128, 128], f32r, tag="ps_s")
            nc.tensor.transpose(pt[:], hT_new[:, 128*g:128*(g+1)], identr[:])
            nc.scalar.activation(out=hrm_new[:, 128*g:128*(g+1)], in_=pt[:],
                                 func=Act.Copy)
        hT, hrm = hT_new, hrm_new
    if dbg:
        nc.sync.dma_start(out=dbg["dbg_hT"][:], in_=hT[:].bitcast(f32))

    # ---------- phase 3: sk path (tqT/tcT [32, (8p,32n)])
    def h_cols(par):  # par=0 query, 1 corpus -> [128, (8p, 32n)] AP view
        v = hT[:].rearrange("p (g x n) -> p g x n", x=2, n=32)
        return v[:, :, par, :]

    tqT = persist.tile([32, 256], f32, tag="tqT")
    tcT = persist.tile([32, 256], f32, tag="tcT")
    for par, dst in ((0, tqT), (1, tcT)):
        p1 = psm.tile([32, 256], f32, tag="ps_s")
        mm(p1[:], cs["wsk1"][:], h_cols(par), True, False, dt=f32r)
        mm(p1[:], cs["bsk1r"][:], ones1[:1, :256], False, True, dt=f32r)
        s1 = stg.tile([32, 256], f32r, tag="sk_s1")
        nc.scalar.activation(out=s1[:], in_=p1[:], func=Act.Relu)
        p2 = psm.tile([32, 256], f32, tag="ps_s")
        mm(p2[:], cs["wsk2"][:], s1[:], True, False, dt=f32r)
        mm(p2[:], cs["bsk2r"][:], ones1[:1, :256], False, True, dt=f32r)
        nc.vector.tensor_copy(dst[:], p2[:])
    if dbg:
        nc.sync.dma_start(out=dbg["dbg_tqT"][:], in_=tqT[:])
        nc.sync.dma_start(out=dbg["dbg_tcT"][:], in_=tcT[:])

    # ---------- phase 4: node cost [32, (8p,32j)] via strips
    # (written straight into `la`, scaled in place afterwards)
    cost = persist.tile([32, 256], f32, tag="la")

    def strip_cdist(out_tile, blk_of_ph, cols_of_ph, dpart, blk, nacc, tag, dt_, defer=False,
                    ones_override=None, mode="max"):
        """out_tile[i, blk*p+j] = w * sum_d max(blk(p,hh)[d,j], cols(p,hh)[d,i])
        (w = value of the reduce vector, 1 or 2).

        Max-trick: sum_d |a-b| = 2*sum_d max(a,b) - sum_d a - sum_d b; the
        rank-1 terms are either absorbed by sinkhorn (cost) or corrected in
        the final plan dot with exact plan row/col sums. One TT max per
        (p, hh, 16-i chunk) over [dpart, 16*blk], then 4 strip matmuls
        (tile_position rotation) reduce over d into PSUM rows {0,32,64,96};
        evacuate via full-tile copy + strided SBUF->SBUF DMA.
        """
        ones_l = ones_override if ones_override is not None else (
            ones128b if dt_ == bf16 else (ones32f if dpart == 32 else ones128f))
        units = []
        for p in range(BL):
            for ib in range(6 if blk == 96 else 2):
                units.append((p, ib))
        closures = []
        def make_unit(p, ib):
            def unit():
                pstr = psm.tile([128, 4 * blk], f32, tag="ps_s")
                st0 = stg2.tile([dpart, 16 * blk], dt_, tag=tag + "_s0")
                st1 = None
                srcs = [st0]
                if nacc == 2:
                    st1 = stg2.tile([dpart, 16 * blk], dt_, tag=tag + "_s1")
                    srcs.append(st1)
                for hh in range(nacc):
                    stt_t = srcs[hh]
                    blk_ap = blk_of_ph(p, hh)
                    cols_ap = cols_of_ph(p, hh, 16 * ib, 16)
                    in0 = bass.AP(tensor=blk_ap.tensor, offset=blk_ap.offset,
                                  ap=[blk_ap.ap[0], [0, 16]] + list(blk_ap.ap[1:]))
                    in1 = bass.AP(tensor=cols_ap.tensor, offset=cols_ap.offset,
                                  ap=list(cols_ap.ap) + [[0, blk]])
                    v3 = stt_t[:].rearrange("p (i j) -> p i j", j=blk)
                    if mode == "max":
                        nc.vector.tensor_tensor(out=v3, in0=in0, in1=in1,
                                                op=Alu.max)
                    else:
                        nc.vector.tensor_tensor(out=v3, in0=in0, in1=in1,
                                                op=Alu.subtract)
                        nc.vector.scalar_tensor_tensor(
                            out=v3, in0=v3, scalar=-1.0, in1=v3,
                            op0=Alu.mult, op1=Alu.max)
                for c in range(4):
                    cs_ = slice(4 * blk * c, 4 * blk * (c + 1))
                    mm(pstr[32*c:32*c+1, :], ones_l[:], st0[:, cs_],
                       True, nacc == 1, tile_position=(0, 32*c))
                    if nacc == 2:
                        mm(pstr[32*c:32*c+1, :], ones_l[:], st1[:, cs_],
                           False, True, tile_position=(0, 32*c))
                s2 = stg2.tile([128, 4 * blk], f32, tag=tag + "_s2")
                nc.vector.tensor_copy(s2[:], pstr[:])
                sv = s2[:]
                iv = bass.AP(tensor=sv.tensor, offset=sv.offset,
                             ap=[[32 * sv.ap[0][0], 4], [blk, 4], [1, blk]])
                nc.sync.dma_start(
                    out=out_tile[16*ib:16*(ib+1), blk*p:blk*(p+1)], in_=iv)
            return unit
        for (p, ib) in units:
            closures.append(make_unit(p, ib))
        if defer:
            return closures
        for cl in closures:
            cl()

    strip_cdist(cost,
                blk_of_ph=lambda p, hh: tcT[:, 32*p:32*(p+1)],
                cols_of_ph=lambda p, hh, i0, ni: tqT[:, 32*p+i0:32*p+i0+ni],
                dpart=32, blk=32, nacc=1, tag="nc", dt_=f32, mode="abs")
    if dbg:
        nc.sync.dma_start(out=dbg["dbg_cost"][:], in_=cost[:])

    # ---------- phase 8/9: lrl embeddings + edge cdist D [96, (8p,96j)]
    hidL = message_layer(hrm, cs["wl1a"], cs["wl1b"], cs["wl1c"],
                         cs["wl2a"], cs["wl2b"], cs["bl2r"], lrl=True)
    bid0b = persist.tile([128, EL], bf16, tag="bid0b")
    bid1b = persist.tile([128, EL], bf16, tag="bid1b")
    for mt, dst in ((0, bid0b), (1, bid1b)):
        for n in range(3):
            pb2 = pbig.tile([128, 512], f32, tag="pa")
            ns = slice(512*n, 512*(n+1))
            mm(pb2[:], cs["wl2a"][:, 128*mt:128*(mt+1)],
               hidL[:, EL*0 + 512*n: EL*0 + 512*(n+1)], True, False, dt=f32r)
            mm(pb2[:], cs["wl2b"][:, 128*mt:128*(mt+1)],
               hidL[:, EL*1 + 512*n: EL*1 + 512*(n+1)], False, False, dt=f32r)
            mm(pb2[:], cs["wl2a"][:, 128*mt:128*(mt+1)],
               hidL[:, EL*2 + 512*n: EL*2 + 512*(n+1)], False, False, dt=f32r)
            mm(pb2[:], cs["wl2b"][:, 128*mt:128*(mt+1)],
               hidL[:, EL*3 + 512*n: EL*3 + 512*(n+1)], False, False, dt=f32r)
            mm(pb2[:], cs["bl2r"][:, 128*mt:128*(mt+1)], ones1[:], False, True)
            nc.scalar.activation(out=dst[:, ns], in_=pb2[:], func=Act.Copy)

    if dbg:
        nc.sync.dma_start(out=dbg["dbg_bid0"][:], in_=bid0b[:])
        nc.sync.dma_start(out=dbg["dbg_bid1"][:], in_=bid1b[:])

    # ---------- phase 5: node sinkhorn, log space
    la = cost
    nc.vector.tensor_scalar(out=la[:], in0=la[:], scalar1=float(-1.0/TEMP),
                            scalar2=None, op0=Alu.mult)

    def ns_norm_step(t):
        """log-space normalize along each 32-wide free block of t [32, 256]."""
        t3 = t[:].rearrange("p (b j) -> p b j", j=32)
        rm = stg.tile([32, 8], f32, tag="ns_rm")
        nc.vector.tensor_reduce(out=rm[:], in_=t3, axis=AX.X, op=Alu.max,
                                negate=True)
        tmp = stg.tile([32, 256], f32, tag="ns_tmp")
        nc.vector.scalar_tensor_tensor(
            out=tmp[:].rearrange("p (b j) -> p b j", j=32), in0=t3, scalar=1.0,
            in1=bcast_in(rm[:], 32), op0=Alu.mult, op1=Alu.add)
        ex = stg.tile([32, 256], f32, tag="ns_ex")
        nc.scalar.activation(out=ex[:], in_=tmp[:], func=Act.Exp)
        sm = stg.tile([32, 8], f32, tag="ns_sm")
        nc.vector.tensor_reduce(out=sm[:], in_=ex[:].rearrange(
            "p (b j) -> p b j", j=32), axis=AX.X, op=Alu.add)
        ls = stg.tile([32, 8], f32, tag="ns_ls")
        nc.scalar.activation(out=ls[:], in_=sm[:], func=Act.Ln)
        lse = stg.tile([32, 8], f32, tag="ns_lse")
        nc.vector.tensor_tensor(out=lse[:], in0=ls[:], in1=rm[:], op=Alu.subtract)
        nc.vector.scalar_tensor_tensor(
            out=t3, in0=t3, scalar=1.0,
            in1=bcast_in(lse[:], 32), op0=Alu.mult, op1=Alu.subtract)

    ncd = persist.tile([32, 256], f32, tag="ncd")
    hTb = persist.tile([128, 512], bf16, tag="hTb")
    nc.vector.tensor_copy(hTb[:], hT[:])
    na_units = strip_cdist(ncd,
                blk_of_ph=lambda p, hh: hTb[:, 64*p+32:64*p+64],
                cols_of_ph=lambda p, hh, i0, ni: hTb[:, 64*p+i0:64*p+i0+ni],
                dpart=128, blk=32, nacc=1, tag="na", dt_=bf16, defer=True,
                ones_override=twos128b)

    # log-domain sinkhorn (trajectory-exact vs the reference: the node plan
    # is chaotically sensitive — it has NOT converged at 20 iters, so the
    # arithmetic structure must mirror the reference's lse updates).
    lat = persist.tile([32, 256], f32, tag="lat")
    for it in range(ITERS):
        ns_norm_step(la)                      # row step
        nc.vector.transpose(lat[:], la[:])    # per-pair 32x32 block transpose
        if na_units:
            na_units.pop(0)()
        ns_norm_step(lat)                     # col step (rows of transposed)
        nc.vector.transpose(la[:], lat[:])
    while na_units:
        na_units.pop(0)()
    nplan = persist.tile([32, 256], f32, tag="nplan")
    nc.scalar.activation(out=nplan[:], in_=la[:], func=Act.Exp)
    P3 = nplan[:].rearrange("p (b j) -> p b j", j=32)
    if dbg:
        nc.sync.dma_start(out=dbg["dbg_nplan"][:], in_=nplan[:])

    # ---------- phase 6: kron -> M0 edge [96, (8p,96j)]
    Me = persist.tile([96, 768], f32, tag="Me")
    for p in range(BL):
        Pp = nplan[:, 32*p:32*(p+1)]
        put = psm.tile([32, 96], f32, tag="ps_s")
        pvt = psm.tile([32, 96], f32, tag="ps_s")
        mm(put[:], Pp, cs["kfq"][:, 96*p:96*(p+1)], True, True)
        mm(pvt[:], Pp, cs["ktq"][:, 96*p:96*(p+1)], True, True)
        ut = stg.tile([32, 96], f32, tag="kr_ut")
        vt = stg.tile([32, 96], f32, tag="kr_vt")
        nc.vector.tensor_copy(ut[:], put[:])
        nc.vector.tensor_copy(vt[:], pvt[:])
        pA = psm.tile([96, 96], f32, tag="ps_s")
        pB = psm.tile([96, 96], f32, tag="ps_s")
        mm(pA[:], ut[:], cs["kfc"][:, 96*p:96*(p+1)], True, True)
        mm(pB[:], vt[:], cs["ktc"][:, 96*p:96*(p+1)], True, True)
        sA = stg.tile([96, 96], f32, tag="kr_sA")
        nc.scalar.activation(out=sA[:], in_=pA[:], func=Act.Copy)
        straight = stg.tile([96, 96], f32, tag="kr_str")
        nc.vector.tensor_tensor(out=straight[:], in0=sA[:], in1=pB[:], op=Alu.mult)
        pC = psm.tile([96, 96], f32, tag="ps_s")
        pD = psm.tile([96, 96], f32, tag="ps_s")
        mm(pC[:], ut[:], cs["ktc"][:, 96*p:96*(p+1)], True, True)
        mm(pD[:], vt[:], cs["kfc"][:, 96*p:96*(p+1)], True, True)
        sC = stg.tile([96, 96], f32, tag="kr_sC")
        nc.scalar.activation(out=sC[:], in_=pC[:], func=Act.Copy)
        cross = stg.tile([96, 96], f32, tag="kr_crs")
        nc.vector.tensor_tensor(out=cross[:], in0=sC[:], in1=pD[:], op=Alu.mult)
        gmax = stg.tile([96, 96], f32, tag="kr_gmax")
        nc.vector.tensor_tensor(out=gmax[:], in0=straight[:], in1=cross[:],
                                op=Alu.max)
        nc.scalar.activation(out=Me[:, 96*p:96*(p+1)], in_=gmax[:], func=Act.Exp,
                             scale=float(1.0/TEMP))
    if dbg:
        nc.sync.dma_start(out=dbg["dbg_M0"][:], in_=Me[:])

    # ---------- phase 7: edge sinkhorn (multiplicative) + interleaved cdist
    D = persist.tile([96, 768], f32, tag="D")
    ec_units = strip_cdist(D,
                blk_of_ph=lambda p, hh: (bid0b if hh == 0 else bid1b)[:, 192*p+96:192*p+192],
                cols_of_ph=lambda p, hh, i0, ni: (bid0b if hh == 0 else bid1b)
                    [:, 192*p+i0:192*p+i0+ni],
                dpart=128, blk=96, nacc=2, tag="ec", dt_=bf16, defer=True,
                ones_override=twos128b)
    # Lazy row normalization: stored Me is only col-normalized; the current
    # row factors rr (= 1/rowsum(Me)) are folded into the colsum stationary
    # each iteration and into the final plan*D dot, saving one [96,768]
    # DVE pass per iteration.
    Me3 = Me[:].rearrange("p (b j) -> p b j", j=96)
    rr = persist.tile([96, 8], f32, tag="es_rr")
    for it in range(ITERS):
        rs = stg.tile([96, 8], f32, tag="es_rs")
        nc.vector.tensor_reduce(out=rs[:], in_=Me3, axis=AX.X, op=Alu.add)
        nc.vector.reciprocal(out=rr[:], in_=rs[:])
        rc = stg1.tile([96, 768], f32, tag="big768")
        pcs_l = []
        for hh in range(2):
            pcs = psm.tile([96, 384], f32, tag="ps_s")
            for q in range(4):
                pp = 4*hh + q
                mm(pcs[:, 96*q:96*(q+1)], rr[:, pp:pp+1].to_broadcast((96, 96)),
                   Me[:, 96*pp:96*(pp+1)], True, True)
            pcs_l.append(pcs)
        for _ in range(2):
            if ec_units:
                ec_units.pop(0)()
        for hh in range(2):
            nc.vector.reciprocal(out=rc[:, 384*hh:384*(hh+1)],
                                 in_=pcs_l[hh][:])
        nc.vector.tensor_tensor(out=Me[:], in0=Me[:], in1=rc[:], op=Alu.mult)
    while ec_units:
        ec_units.pop(0)()
    if dbg:
        nc.vector.scalar_tensor_tensor(
            out=Me3, in0=Me3, scalar=1.0, in1=bcast_in(rr[:], 96),
            op0=Alu.mult, op1=Alu.mult)
        nc.sync.dma_start(out=dbg["dbg_eplan"][:], in_=Me[:])
        nc.sync.dma_start(out=dbg["dbg_D"][:], in_=D[:])


    # (ncd computed interleaved with node sinkhorn above)
    if dbg:
        nc.sync.dma_start(out=dbg["dbg_ncd"][:], in_=ncd[:])

    # ---------- phase 11: dots + output (with max-trick rank-1 corrections)
    # edge_align_p = sum_ij EP*(2S_e) - sum_j csEP_j*sbe_j - sum_i rsEP_i*sce_i
    # with csEP_j = 1 exactly (final step is an exact column normalize).
    # node_align_p likewise with csP_j = 1.
    #
    # column-layout q-side feature sums: sce_col[i,p] (edges), scn_col[i,p]
    # (nodes) via 1-row matmuls vs a ones column.
    psce = psm.tile([96, 8], f32, tag="ps_s")
    for p in range(BL):
        mm(psce[:, p:p+1], bid0b[:, 192*p:192*p+96], ones128b[:], True, False)
        mm(psce[:, p:p+1], bid1b[:, 192*p:192*p+96], ones128b[:], False, True)
    sce = stg.tile([96, 8], f32, tag="dot_sce")
    nc.vector.tensor_copy(sce[:], psce[:])
    pscn = psm.tile([32, 8], f32, tag="ps_s")
    for p in range(BL):
        mm(pscn[:, p:p+1], hTb[:, 64*p:64*p+32], ones128b[:], True, True)
    scn = stg.tile([32, 8], f32, tag="dot_scn")
    nc.vector.tensor_copy(scn[:], pscn[:])
    # row-layout c-side feature sums -> [1, 8] totals (csEP = csP = 1)
    hTb4 = hTb[:].rearrange("p (g x n) -> p g x n", x=2, n=32)
    bid04 = bid0b[:].rearrange("p (g x n) -> p g x n", x=2, n=96)
    bid14 = bid1b[:].rearrange("p (g x n) -> p g x n", x=2, n=96)
    pn_row = psm.tile([1, 256], f32, tag="ps_s")
    mm(pn_row[:], ones128b[:], hTb4[:, :, 1, :], True, True)
    pe_row0 = psm.tile([1, 384], f32, tag="ps_s")
    mm(pe_row0[:], ones128b[:], bid04[:, 0:4, 1, :], True, False)
    mm(pe_row0[:], ones128b[:], bid14[:, 0:4, 1, :], False, True)
    pe_row1 = psm.tile([1, 384], f32, tag="ps_s")
    mm(pe_row1[:], ones128b[:], bid04[:, 4:8, 1, :], True, False)
    mm(pe_row1[:], ones128b[:], bid14[:, 4:8, 1, :], False, True)
    tA = stg.tile([1, 16], f32, tag="dot_tA")
    nc.vector.tensor_reduce(out=tA[:, :8], in_=pn_row[:].rearrange(
        "p (b j) -> p b j", j=32), axis=AX.X, op=Alu.add)
    nc.vector.tensor_reduce(out=tA[:, 8:12], in_=pe_row0[:].rearrange(
        "p (b j) -> p b j", j=96), axis=AX.X, op=Alu.add)
    nc.vector.tensor_reduce(out=tA[:, 12:16], in_=pe_row1[:].rearrange(
        "p (b j) -> p b j", j=96), axis=AX.X, op=Alu.add)
    tAall = stg.tile([1, 8], f32, tag="dot_tAall")
    nc.vector.tensor_tensor(out=tAall[:], in0=tA[:, :8], in1=tA[:, 8:],
                            op=Alu.add)
    # plan row sums (post final col-normalize)
    rse = stg.tile([96, 8], f32, tag="dot_rse")
    nc.vector.tensor_reduce(out=rse[:], in_=Me3, axis=AX.X, op=Alu.add)
    nc.vector.tensor_tensor(out=rse[:], in0=rse[:], in1=rr[:], op=Alu.mult)
    nc.vector.tensor_tensor(out=rse[:], in0=rse[:], in1=sce[:], op=Alu.mult)
    rsn = stg.tile([32, 8], f32, tag="dot_rsn")
    nc.vector.tensor_reduce(out=rsn[:], in_=P3, axis=AX.X, op=Alu.add)
    nc.vector.tensor_tensor(out=rsn[:], in0=rsn[:], in1=scn[:], op=Alu.mult)
    # main plan (.) 2S dots
    we = stg1.tile([96, 768], f32, tag="big768")
    nc.vector.tensor_tensor(out=we[:], in0=Me[:], in1=D[:], op=Alu.mult)
    ep = stg.tile([96, 8], f32, tag="dot_ep")
    nc.vector.tensor_reduce(out=ep[:], in_=we[:].rearrange(
        "p (b j) -> p b j", j=96), axis=AX.X, op=Alu.add)
    nc.vector.tensor_tensor(out=ep[:], in0=ep[:], in1=rr[:], op=Alu.mult)
    nc.vector.tensor_tensor(out=ep[:], in0=ep[:], in1=rse[:], op=Alu.subtract)
    wn = stg.tile([32, 256], f32, tag="dot_wn")
    nc.vector.tensor_tensor(out=wn[:], in0=nplan[:], in1=ncd[:], op=Alu.mult)
    np_ = stg.tile([32, 8], f32, tag="dot_np")
    nc.vector.tensor_reduce(out=np_[:], in_=wn[:].rearrange(
        "p (b j) -> p b j", j=32), axis=AX.X, op=Alu.add)
    nc.vector.tensor_tensor(out=np_[:], in0=np_[:], in1=rsn[:], op=Alu.subtract)
    pout = psm.tile([1, 8], f32, tag="ps_s")
    mm(pout[:], ones96sq[:, 0:1], ep[:], True, False)
    mm(pout[:], ones32f[:], np_[:], False, False)
    mm(pout[:], negones11[:], tAall[:], False, True)
    osb = stg.tile([1, 8], f32, tag="osb")
    nc.vector.tensor_copy(osb[:], pout[:])
    nc.sync.dma_start(out=out_ext[:], in_=osb[:])

    ctx.close()


# ----------------------------------------------------------------- entry
def _get_nc(debug=False):
    key = ("nc", debug)
    if key not in _CACHE:
        _CACHE[key] = _build(debug=debug)
    return _CACHE[key]


def run_cores(inputs, debug=False, trace=False):
    from concourse.bass_utils import run_bass_kernel_spmd
    nc = _get_nc(debug=debug)
    in_maps = _host_prep(inputs)
    res = run_bass_kernel_spmd(nc, in_maps, core_ids=list(range(NCORE)),
                               trace=trace)
    return res


def kernel(**inputs):
    res = run_cores(inputs, debug=False, trace=False)
    out = np.concatenate([r["out"].reshape(-1) for r in res.results])
    return out.astype(np.float32)



# revision 32
# speedup vs baseline: 1.9147x; 1.1304x over previous
"""Trainium2 Bass kernel for nn_ABL_SPARSE_87694642250045 (GMN graph matching).

Data-parallel over B=64 graph pairs: 8 pairs (16 graphs) per NeuronCore, 8 cores.
No collectives — output is per-pair scalars, concatenated host-side.

Device decomposition (per core):
  - gathers/segment-sums as one-hot matmuls (one-hots precomputed host-side)
  - message MLP with W-swap trick: both directions in one [*,512] hidden
  - residual update folded into (W_upd_a + I)
  - node sinkhorn in log space (PE transposes for column steps)
  - edge sinkhorn multiplicative (column sums via ones-matmul, no transposes)
  - L1 cdists: tensor_scalar |a-b| with d-on-partitions + ones-matmul reduce
    (strips) + tile_position rotation + SBUF DMA reshape
"""
import numpy as np

NCORE = 8
B, N, E = 64, 32, 96
NPROP, TEMP, ITERS = 5, 0.1, 20
BL = B // NCORE          # 8 pairs / core
GL = 2 * BL              # 16 graphs / core
VL = GL * N              # 512 nodes / core
EL = GL * E              # 1536 edges / core

_CACHE = {}


# ----------------------------------------------------------------- host prep
def _onehot(idx, n):
    out = np.zeros((len(idx), n), np.float32)
    out[np.arange(len(idx)), idx] = 1.0
    return out


def _host_prep(inputs):
    f32 = np.float32
    nf = np.asarray(inputs["node_features"], f32)
    ef = np.asarray(inputs["edge_features"], f32)
    fr_all = np.asarray(inputs["from_idx"]).astype(np.int64)
    to_all = np.asarray(inputs["to_idx"]).astype(np.int64)

    W_enc = np.asarray(inputs["W_enc"], f32); b_enc = np.asarray(inputs["b_enc"], f32)
    W1 = np.asarray(inputs["W_msg1"], f32); b1 = np.asarray(inputs["b_msg1"], f32)
    W2 = np.asarray(inputs["W_msg2"], f32); b2 = np.asarray(inputs["b_msg2"], f32)
    Wu = np.asarray(inputs["W_upd"], f32); bu = np.asarray(inputs["b_upd"], f32)
    Wsk1 = np.asarray(inputs["W_sk1"], f32); bsk1 = np.asarray(inputs["b_sk1"], f32)
    Wsk2 = np.asarray(inputs["W_sk2"], f32); bsk2 = np.asarray(inputs["b_sk2"], f32)
    Wl1 = np.asarray(inputs["W_lrl1"], f32); bl1 = np.asarray(inputs["b_lrl1"], f32)
    Wl2 = np.asarray(inputs["W_lrl2"], f32); bl2 = np.asarray(inputs["b_lrl2"], f32)

    def ext(Wm, bm):
        Wswap = np.concatenate([Wm[128:256], Wm[0:128], Wm[256:257]], axis=0)
        Wcat = np.concatenate([Wm, Wswap], axis=1)               # [257,512]
        bcat = np.concatenate([bm, bm])[None]                    # [1,512]
        return np.ascontiguousarray(np.concatenate([Wcat, bcat], axis=0))  # [258,512]

    W1ext = ext(W1, b1)
    Wl1ext = ext(Wl1, bl1)

    shared = {
        "w1a": W1ext[0:128], "w1b": W1ext[128:256], "w1c": W1ext[256:258],
        "wl1a": Wl1ext[0:128], "wl1b": Wl1ext[128:256], "wl1c": Wl1ext[256:258],
        "w2a": W2[0:128], "w2b": W2[128:256],
        "wl2a": Wl2[0:128], "wl2b": Wl2[128:256],
        "b2r": b2[None], "bl2r": (2.0 * bl2)[None],
        "wuaI": Wu[0:128] + np.eye(128, dtype=f32),
        "wub_a": Wu[128:256], "wub_b": Wu[256:384], "bur": bu[None],
        "wenc": W_enc, "bencr": b_enc[None],
        "wsk1": Wsk1, "bsk1r": bsk1[None], "wsk2": Wsk2, "bsk2r": bsk2[None],
    }
    shared = {k: np.ascontiguousarray(v, f32) for k, v in shared.items()}
    import ml_dtypes
    for k in ("w1c", "wl1c"):
        shared[k] = shared[k].astype(ml_dtypes.bfloat16)

    in_maps = []
    for c in range(NCORE):
        nfc = nf[c*VL:(c+1)*VL]                                  # [512,32]
        efc = ef[c*EL:(c+1)*EL]                                  # [1536,1]
        fr = fr_all[c*EL:(c+1)*EL] - c*VL
        to = to_all[c*EL:(c+1)*EL] - c*VL

        gfT = np.zeros((128, EL), f32)
        gtT = np.zeros((128, EL), f32)
        for g in range(4):
            e0, v0 = 384*g, 128*g
            gfT[:, e0:e0+384] = _onehot(fr[e0:e0+384] - v0, 128).T
            gtT[:, e0:e0+384] = _onehot(to[e0:e0+384] - v0, 128).T

        # scatter one-hots, 256-wide (group-pair local) so the scatter matmul
        # free dim is >=256 and f32r runs at 1 cycle/row
        sT = np.zeros((128, 12*256), f32)
        sF = np.zeros((128, 12*256), f32)
        for kt in range(12):
            e0, g = 128*kt, kt // 3
            gp0 = (g // 2) * 2                # group-pair base group
            sT[:, 256*kt:256*(kt+1)] = _onehot(to[e0:e0+128] - 128*gp0, 256)
            sF[:, 256*kt:256*(kt+1)] = _onehot(fr[e0:e0+128] - 128*gp0, 256)

        frg = fr.reshape(GL, E) - (np.arange(GL) * N)[:, None]
        tog = to.reshape(GL, E) - (np.arange(GL) * N)[:, None]
        kfq = np.zeros((32, BL*E), f32); ktq = np.zeros((32, BL*E), f32)
        kfc = np.zeros((32, BL*E), f32); ktc = np.zeros((32, BL*E), f32)
        for p in range(BL):
            s = slice(E*p, E*(p+1))
            kfq[:, s] = _onehot(frg[2*p], N).T
            ktq[:, s] = _onehot(tog[2*p], N).T
            kfc[:, s] = _onehot(frg[2*p+1], N).T
            ktc[:, s] = _onehot(tog[2*p+1], N).T

        e1 = np.concatenate([efc.T, np.ones((1, EL), f32)], axis=0)  # [2,1536]

        import ml_dtypes
        m = dict(shared)
        m.update({
            "nfT": np.ascontiguousarray(nfc.T),      # [32,512]
            "e1": np.ascontiguousarray(e1).astype(ml_dtypes.bfloat16),
            "gfT": gfT, "gtT": gtT, "sT": sT, "sF": sF,
            "kfq": kfq, "ktq": ktq, "kfc": kfc, "ktc": ktc,
        })
        in_maps.append(m)
    return in_maps


# --------------------------------------------------------------- bass builder
def _build(debug=False):
    import concourse.bass as bass
    import concourse.bacc as bacc
    import concourse.mybir as mybir
    import concourse.tile as tile
    from concourse.masks import make_identity

    f32 = mybir.dt.float32
    bf16 = mybir.dt.bfloat16
    f32r = mybir.dt.float32r
    Alu = mybir.AluOpType
    Act = mybir.ActivationFunctionType
    AX = mybir.AxisListType

    nc = bacc.Bacc("TRN2", target_bir_lowering=False)

    # ---- dram declarations
    dr = {}
    decls = {
        "nfT": (32, VL), "e1": (2, EL), "gfT": (128, EL), "gtT": (128, EL),
        "sT": (128, 12*256), "sF": (128, 12*256),
        "kfq": (32, BL*E), "ktq": (32, BL*E), "kfc": (32, BL*E), "ktc": (32, BL*E),
        "w1a": (128, 512), "w1b": (128, 512), "w1c": (2, 512),
        "wl1a": (128, 512), "wl1b": (128, 512), "wl1c": (2, 512),
        "w2a": (128, 256), "w2b": (128, 256), "wl2a": (128, 256), "wl2b": (128, 256),
        "b2r": (1, 256), "bl2r": (1, 256),
        "wuaI": (128, 128), "wub_a": (128, 128), "wub_b": (128, 128), "bur": (1, 128),
        "wenc": (32, 128), "bencr": (1, 128),
        "wsk1": (128, 32), "bsk1r": (1, 32), "wsk2": (32, 32), "bsk2r": (1, 32),
    }
    f32r_names = set(['wl1a', 'wl1b', 'wl1c', 'wl2a', 'wl2b', 'bl2r',
                      'w1a', 'w1b', 'w1c', 'w2a', 'w2b', 'b2r',
                      'wuaI', 'wub_a', 'wub_b', 'bur', 'gfT', 'gtT',
                      'wsk1', 'bsk1r', 'wsk2', 'bsk2r', 'sT', 'sF'])
    bf16_names = set(['w1c', 'wl1c', 'e1'])
    for k, shp in decls.items():
        dt_ = bf16 if k in bf16_names else (f32r if k in f32r_names else f32)
        dr[k] = nc.declare_dram_parameter(k, list(shp), dt_, isOutput=False)
    out_ext = nc.declare_dram_parameter("out", [1, BL], f32, isOutput=True)
    dbg = {}
    if debug:
        for k, shp in {
            "dbg_hT0": (128, 512), "dbg_hT": (128, 512), "dbg_tqT": (32, 256),
            "dbg_tcT": (32, 256), "dbg_cost": (32, 256), "dbg_nplan": (32, 256),
            "dbg_M0": (96, 768), "dbg_eplan": (96, 768), "dbg_D": (96, 768),
            "dbg_ncd": (32, 256),
        }.items():
            dbg[k] = nc.declare_dram_parameter(k, list(shp), f32, isOutput=True)
        for k, shp in {"dbg_bid0": (128, EL), "dbg_bid1": (128, EL)}.items():
            dbg[k] = nc.declare_dram_parameter(k, list(shp), mybir.dt.bfloat16,
                                               isOutput=True)

    with tile.TileContext(nc) as tc:
        _emit(nc, tc, dr, out_ext, dbg, f32, bf16, f32r, Alu, Act, AX, make_identity)

    # Pin Exp/Ln to the one activation table that holds both so the
    # table-placement fixpoint hoists a single load instead of reloading on
    # every Exp<->Ln alternation in the sinkhorn loop. Table ids are
    # preserved (only the advertised function sets shrink).
    import concourse.bacc as bacc_mod
    orig_tables = bacc_mod.get_activation_tables

    def pinned_tables(arch):
        tabs = orig_tables(arch)
        both = "natural_log_exp_and_others"
        exp_f = Act.Exp
        ln_f = Act.Ln
        if both in tabs and exp_f in tabs[both] and ln_f in tabs[both]:
            tabs = {
                name: (s if name == both
                       else {f for f in s if f not in (exp_f, ln_f)})
                for name, s in tabs.items()
            }
        return tabs

    bacc_mod.get_activation_tables = pinned_tables
    try:
        nc.compile()
    finally:
        bacc_mod.get_activation_tables = orig_tables
    return nc


def _emit(nc, tc, dr, out_ext, dbg, f32, bf16, f32r, Alu, Act, AX, make_identity):
    import concourse.bass as bass
    from contextlib import ExitStack

    ctx = ExitStack()
    const = ctx.enter_context(tc.tile_pool(name="const", bufs=1))
    persist = ctx.enter_context(tc.tile_pool(name="persist", bufs=1))
    wrk = ctx.enter_context(tc.tile_pool(name="wrk", bufs=1))
    hpool = ctx.enter_context(tc.tile_pool(name="hpool", bufs=2))
    hidp = ctx.enter_context(tc.tile_pool(name="hidp", bufs=1))
    stg = ctx.enter_context(tc.tile_pool(name="stg", bufs=3))
    stg2 = ctx.enter_context(tc.tile_pool(name="stg2", bufs=2))
    stg1 = ctx.enter_context(tc.tile_pool(name="stg1", bufs=1))
    pbig = ctx.enter_context(tc.tile_pool(name="pbig", bufs=3, space="PSUM"))
    pmsg = pbig
    pagg = ctx.enter_context(tc.tile_pool(name="pagg", bufs=1, space="PSUM"))
    psm = ctx.enter_context(tc.tile_pool(name="psm", bufs=3, space="PSUM"))

    def mm(out, lhsT, rhs, start, stop, dt=None, tile_position=None):
        if dt is not None:
            lhsT = lhsT.bitcast(dt)
            rhs = rhs.bitcast(dt)
        nc.tensor.matmul(out, lhsT, rhs, start=start, stop=stop,
                         tile_position=tile_position)

    def bcast_in(ap, n):
        # [P, F] -> [P, F, n] with stride-0 inner free dim
        a = ap
        return bass.AP(tensor=a.tensor, offset=a.offset,
                       ap=list(a.ap) + [[0, n]])

    # ---------- constants to SBUF
    cs = {}
    for k, shp in {
        "nfT": (32, VL), "e1": (2, EL), "gfT": (128, EL), "gtT": (128, EL),
        "sT": (128, 12*256), "sF": (128, 12*256),
        "kfq": (32, BL*E), "ktq": (32, BL*E), "kfc": (32, BL*E), "ktc": (32, BL*E),
        "w1a": (128, 512), "w1b": (128, 512), "w1c": (2, 512),
        "wl1a": (128, 512), "wl1b": (128, 512), "wl1c": (2, 512),
        "w2a": (128, 256), "w2b": (128, 256), "wl2a": (128, 256), "wl2b": (128, 256),
        "b2r": (1, 256), "bl2r": (1, 256),
        "wuaI": (128, 128), "wub_a": (128, 128), "wub_b": (128, 128), "bur": (1, 128),
        "wenc": (32, 128), "bencr": (1, 128),
        "wsk1": (128, 32), "bsk1r": (1, 32), "wsk2": (32, 32), "bsk2r": (1, 32),
    }.items():
        if k in ('w1c', 'wl1c', 'e1'):
            dt_ = bf16
        elif k in ['wl1a', 'wl1b', 'wl2a', 'wl2b', 'bl2r',
                   'w1a', 'w1b', 'w2a', 'w2b', 'b2r',
                   'wuaI', 'wub_a', 'wub_b', 'bur', 'gfT', 'gtT',
                   'wsk1', 'bsk1r', 'wsk2', 'bsk2r', 'sT', 'sF']:
            dt_ = f32r
        else:
            dt_ = f32
        t = const.tile(list(shp), dt_, tag=k)
        nc.sync.dma_start(out=t[:], in_=dr[k][:])
        cs[k] = t

    e1r = cs["e1"]
    identf = const.tile([128, 128], f32, tag="identf")
    make_identity(nc, identf[:])
    ones96sq = const.tile([96, 96], f32, tag="ones96sq")
    nc.vector.memset(ones96sq[:], 1.0)
    identr = const.tile([128, 128], f32r, tag="identr")
    nc.vector.tensor_copy(identr[:], identf[:])
    ones1f = const.tile([1, 512], f32, tag="ones1f")
    nc.vector.memset(ones1f[:], 1.0)
    ones1 = const.tile([1, 512], f32r, tag="ones1")
    nc.vector.tensor_copy(ones1[:], ones1f[:])
    ones128f = const.tile([128, 1], f32, tag="ones128f")
    nc.vector.memset(ones128f[:], 1.0)
    ones128r = const.tile([128, 1], f32r, tag="ones128r")
    nc.vector.tensor_copy(ones128r[:], ones128f[:])
    ones128b = const.tile([128, 1], bf16, tag="ones128b")
    nc.vector.memset(ones128b[:], 1.0)
    twos128b = const.tile([128, 1], bf16, tag="twos128b")
    nc.vector.memset(twos128b[:], 2.0)
    negones11 = const.tile([1, 1], f32, tag="negones11")
    nc.vector.memset(negones11[:], -1.0)
    ones32 = const.tile([32, 1], f32r, tag="ones32")
    nc.vector.tensor_copy(ones32[:], ones128f[:32, :])
    ones32f = const.tile([32, 1], f32, tag="ones32f")
    nc.vector.memset(ones32f[:], 1.0)

    # ---------- phase 1: encoder -> hT [128,512], hrm [128,(4g,128f)]
    hT = persist.tile([128, 512], f32r, tag="hT")
    hrm = persist.tile([128, 512], f32r, tag="hrm")

    ps = pbig.tile([128, 512], f32, tag="pa")
    mm(ps[:], cs["wenc"][:], cs["nfT"][:], start=True, stop=False)
    mm(ps[:], cs["bencr"][:], ones1f[:], start=False, stop=True)
    nc.scalar.activation(out=hT[:], in_=ps[:], func=Act.Copy)
    for g in range(4):
        psg = psm.tile([128, 128], f32, tag="ps_s")
        mm(psg[:], cs["nfT"][:, 128*g:128*(g+1)], cs["wenc"][:],
           start=True, stop=False)
        mm(psg[:], ones1f[:1, :128], cs["bencr"][:], start=False, stop=True)
        nc.vector.tensor_copy(hrm[:, 128*g:128*(g+1)], psg[:])
    if dbg:
        nc.sync.dma_start(out=dbg["dbg_hT0"][:], in_=hT[:].bitcast(f32))

    # ---------- phase 2: propagation steps
    def message_layer(hrm_t, wa, wb, wc2, w2_a, w2_b, b2row, lrl):
        """gathers + L1; returns hid tile [128, 4*1536] (mtile m at cols 1536m)"""
        mdt = f32r
        tdt = f32r
        e1t = cs["e1"]
        srcT = wrk.tile([128, EL], tdt, tag="srcT")
        dstT = wrk.tile([128, EL], tdt, tag="dstT")
        for g in range(4):
            psrc = pmsg.tile([128, 384], f32, tag="pa")
            pdst = pmsg.tile([128, 384], f32, tag="pa")
            hg = hrm_t[:, 128*g:128*(g+1)]
            mm(psrc[:], hg, cs["gfT"][:, 384*g:384*(g+1)], start=True, stop=True,
               dt=f32r)
            mm(pdst[:], hg, cs["gtT"][:, 384*g:384*(g+1)], start=True, stop=True,
               dt=f32r)
            nc.scalar.activation(out=srcT[:, 384*g:384*(g+1)], in_=psrc[:],
                                 func=Act.Copy)
            nc.scalar.activation(out=dstT[:, 384*g:384*(g+1)], in_=pdst[:],
                                 func=Act.Copy)
        hid = hidp.tile([128, 4*EL], tdt, tag="hid")
        for m in range(4):
            for n in range(3):
                ph = pbig.tile([128, 512], f32, tag="pa")
                ns = slice(512*n, 512*(n+1))
                mm(ph[:], wa[:, 128*m:128*(m+1)], srcT[:, ns], True, False, dt=mdt)
                mm(ph[:], wb[:, 128*m:128*(m+1)], dstT[:, ns], False, False, dt=mdt)
                mm(ph[:], wc2[:, 128*m:128*(m+1)], e1t[:, ns], False, True)
                dst_ap = hid[:, EL*m + 512*n: EL*m + 512*(n+1)]
                nc.scalar.activation(out=dst_ap, in_=ph[:], func=Act.Relu)
        return hid

    for step in range(NPROP):
        hid = message_layer(hrm, cs["w1a"], cs["w1b"], cs["w1c"],
                            cs["w2a"], cs["w2b"], cs["b2r"], lrl=False)
        # L2 row-major per edge block + wide scatter
        paggT0 = pagg.tile([128, 512], f32, tag="ps_agg0")
        paggT1 = pagg.tile([128, 512], f32, tag="ps_agg1")
        for eb in range(12):
            pmf = pmsg.tile([128, 256], f32, tag="pa")
            pmb = pmsg.tile([128, 256], f32, tag="pa")
            ebs = slice(128*eb, 128*(eb+1))
            mm(pmf[:], hid[:, EL*0 + 128*eb: EL*0 + 128*(eb+1)], cs["w2a"][:],
               True, False, dt=f32r)
            mm(pmf[:], hid[:, EL*1 + 128*eb: EL*1 + 128*(eb+1)], cs["w2b"][:],
               False, False, dt=f32r)
            mm(pmf[:], ones1[:1, :128], cs["b2r"][:], False, True, dt=f32r)
            mm(pmb[:], hid[:, EL*2 + 128*eb: EL*2 + 128*(eb+1)], cs["w2a"][:],
               True, False, dt=f32r)
            mm(pmb[:], hid[:, EL*3 + 128*eb: EL*3 + 128*(eb+1)], cs["w2b"][:],
               False, False, dt=f32r)
            mm(pmb[:], ones1[:1, :128], cs["b2r"][:], False, True, dt=f32r)
            mf = stg.tile([128, 256], f32r, tag="mf")
            mb = stg.tile([128, 256], f32r, tag="mb")
            nc.vector.tensor_copy(mf[:], pmf[:])
            nc.scalar.activation(out=mb[:], in_=pmb[:], func=Act.Copy)
            kts = slice(256*eb, 256*(eb+1))
            gp = (eb // 3) // 2               # group pair 0..1
            gs = slice(256*gp, 256*(gp+1))
            first = (eb % 6 == 0)
            last = (eb % 6 == 5)
            mm(paggT0[:, gs], mf[:, 0:128], cs["sT"][:, kts], first, False,
               dt=f32r)
            mm(paggT0[:, gs], mb[:, 0:128], cs["sF"][:, kts], False, last,
               dt=f32r)
            mm(paggT1[:, gs], mf[:, 128:256], cs["sT"][:, kts], first, False,
               dt=f32r)
            mm(paggT1[:, gs], mb[:, 128:256], cs["sF"][:, kts], False, last,
               dt=f32r)
        aggT0 = hpool.tile([128, 512], f32r, tag="aggT0")
        aggT1 = hpool.tile([128, 512], f32r, tag="aggT1")
        nc.scalar.activation(out=aggT0[:], in_=paggT0[:], func=Act.Copy)
        nc.scalar.activation(out=aggT1[:], in_=paggT1[:], func=Act.Copy)
        # update
        pnew = pbig.tile([128, 512], f32, tag="pa")
        mm(pnew[:], cs["wuaI"][:], hT[:], True, False, dt=f32r)
        mm(pnew[:], cs["wub_a"][:], aggT0[:], False, False, dt=f32r)
        mm(pnew[:], cs["wub_b"][:], aggT1[:], False, False, dt=f32r)
        mm(pnew[:], cs["bur"][:], ones1[:], False, True, dt=f32r)
        hT_new = hpool.tile([128, 512], f32r, tag="hTn")
        nc.scalar.activation(out=hT_new[:], in_=pnew[:], func=Act.Copy)
        hrm_new = hpool.tile([128, 512], f32r, tag="hrmn")
        for g in range(4):
            pt = psm.tile([128, 128], f32r, tag="ps_s")
            nc.tensor.transpose(pt[:], hT_new[:, 128*g:128*(g+1)], identr[:])
            nc.vector.tensor_copy(hrm_new[:, 128*g:128*(g+1)], pt[:])
        hT, hrm = hT_new, hrm_new
    if dbg:
        nc.sync.dma_start(out=dbg["dbg_hT"][:], in_=hT[:].bitcast(f32))

    # ---------- phase 3: sk path (tqT/tcT [32, (8p,32n)])
    def h_cols(par):  # par=0 query, 1 corpus -> [128, (8p, 32n)] AP view
        v = hT[:].rearrange("p (g x n) -> p g x n", x=2, n=32)
        return v[:, :, par, :]

    tqT = persist.tile([32, 256], f32, tag="tqT")
    tcT = persist.tile([32, 256], f32, tag="tcT")
    for par, dst in ((0, tqT), (1, tcT)):
        p1 = psm.tile([32, 256], f32, tag="ps_s")
        mm(p1[:], cs["wsk1"][:], h_cols(par), True, False, dt=f32r)
        mm(p1[:], cs["bsk1r"][:], ones1[:1, :256], False, True, dt=f32r)
        s1 = stg.tile([32, 256], f32r, tag="sk_s1")
        nc.scalar.activation(out=s1[:], in_=p1[:], func=Act.Relu)
        p2 = psm.tile([32, 256], f32, tag="ps_s")
        mm(p2[:], cs["wsk2"][:], s1[:], True, False, dt=f32r)
        mm(p2[:], cs["bsk2r"][:], ones1[:1, :256], False, True, dt=f32r)
        nc.vector.tensor_copy(dst[:], p2[:])
    if dbg:
        nc.sync.dma_start(out=dbg["dbg_tqT"][:], in_=tqT[:])
        nc.sync.dma_start(out=dbg["dbg_tcT"][:], in_=tcT[:])

    # ---------- phase 4: node cost [32, (8p,32j)] via strips
    # (written straight into `la`, scaled in place afterwards)
    cost = persist.tile([32, 256], f32, tag="la")

    def strip_cdist(out_tile, blk_of_ph, cols_of_ph, dpart, blk, nacc, tag, dt_, defer=False,
                    ones_override=None, mode="max", pool_rule=None):
        """out_tile[i, blk*p+j] = w * sum_d max(blk(p,hh)[d,j], cols(p,hh)[d,i])
        (w = value of the reduce vector, 1 or 2).

        Max-trick: sum_d |a-b| = 2*sum_d max(a,b) - sum_d a - sum_d b; the
        rank-1 terms are either absorbed by sinkhorn (cost) or corrected in
        the final plan dot with exact plan row/col sums. One TT max per
        (p, hh, 16-i chunk) over [dpart, 16*blk], then 4 strip matmuls
        (tile_position rotation) reduce over d into PSUM rows {0,32,64,96};
        evacuate via full-tile copy + strided SBUF->SBUF DMA.
        """
        ones_l = ones_override if ones_override is not None else (
            ones128b if dt_ == bf16 else (ones32f if dpart == 32 else ones128f))
        units = []
        for p in range(BL):
            for ib in range(6 if blk == 96 else 2):
                units.append((p, ib))
        closures = []
        def make_unit(p, ib):
            def unit():
                pstr = psm.tile([128, 4 * blk], f32, tag="ps_s")
                st0 = stg2.tile([dpart, 16 * blk], dt_, tag=tag + "_s0")
                st1 = None
                srcs = [st0]
                if nacc == 2:
                    st1 = stg2.tile([dpart, 16 * blk], dt_, tag=tag + "_s1")
                    srcs.append(st1)
                for hh in range(nacc):
                    stt_t = srcs[hh]
                    blk_ap = blk_of_ph(p, hh)
                    cols_ap = cols_of_ph(p, hh, 16 * ib, 16)
                    in0 = bass.AP(tensor=blk_ap.tensor, offset=blk_ap.offset,
                                  ap=[blk_ap.ap[0], [0, 16]] + list(blk_ap.ap[1:]))
                    in1 = bass.AP(tensor=cols_ap.tensor, offset=cols_ap.offset,
                                  ap=list(cols_ap.ap) + [[0, blk]])
                    v3 = stt_t[:].rearrange("p (i j) -> p i j", j=blk)
                    use_pool = pool_rule is not None and pool_rule(p, ib, hh)
                    eng = nc.gpsimd if use_pool else nc.vector
                    if mode == "max":
                        eng.tensor_tensor(out=v3, in0=in0, in1=in1,
                                          op=Alu.max)
                    else:
                        eng.tensor_tensor(out=v3, in0=in0, in1=in1,
                                          op=Alu.subtract)
                        eng.scalar_tensor_tensor(
                            out=v3, in0=v3, scalar=-1.0, in1=v3,
                            op0=Alu.mult, op1=Alu.max)
                for c in range(4):
                    cs_ = slice(4 * blk * c, 4 * blk * (c + 1))
                    mm(pstr[32*c:32*c+1, :], ones_l[:], st0[:, cs_],
                       True, nacc == 1, tile_position=(0, 32*c))
                    if nacc == 2:
                        mm(pstr[32*c:32*c+1, :], ones_l[:], st1[:, cs_],
                           False, True, tile_position=(0, 32*c))
                s2 = stg2.tile([128, 4 * blk], f32, tag=tag + "_s2")
                nc.scalar.activation(out=s2[:], in_=pstr[:], func=Act.Copy)
                sv = s2[:]
                iv = bass.AP(tensor=sv.tensor, offset=sv.offset,
                             ap=[[32 * sv.ap[0][0], 4], [blk, 4], [1, blk]])
                nc.sync.dma_start(
                    out=out_tile[16*ib:16*(ib+1), blk*p:blk*(p+1)], in_=iv)
            return unit
        for (p, ib) in units:
            closures.append(make_unit(p, ib))
        if defer:
            return closures
        for cl in closures:
            cl()

    strip_cdist(cost,
                blk_of_ph=lambda p, hh: tcT[:, 32*p:32*(p+1)],
                cols_of_ph=lambda p, hh, i0, ni: tqT[:, 32*p+i0:32*p+i0+ni],
                dpart=32, blk=32, nacc=1, tag="nc", dt_=f32, mode="abs")
    if dbg:
        nc.sync.dma_start(out=dbg["dbg_cost"][:], in_=cost[:])

    # ---------- phase 8/9: lrl embeddings + edge cdist D [96, (8p,96j)]
    hidL = message_layer(hrm, cs["wl1a"], cs["wl1b"], cs["wl1c"],
                         cs["wl2a"], cs["wl2b"], cs["bl2r"], lrl=True)
    bid0b = persist.tile([128, EL], bf16, tag="bid0b")
    bid1b = persist.tile([128, EL], bf16, tag="bid1b")
    for mt, dst in ((0, bid0b), (1, bid1b)):
        for n in range(3):
            pb2 = pbig.tile([128, 512], f32, tag="pa")
            ns = slice(512*n, 512*(n+1))
            mm(pb2[:], cs["wl2a"][:, 128*mt:128*(mt+1)],
               hidL[:, EL*0 + 512*n: EL*0 + 512*(n+1)], True, False, dt=f32r)
            mm(pb2[:], cs["wl2b"][:, 128*mt:128*(mt+1)],
               hidL[:, EL*1 + 512*n: EL*1 + 512*(n+1)], False, False, dt=f32r)
            mm(pb2[:], cs["wl2a"][:, 128*mt:128*(mt+1)],
               hidL[:, EL*2 + 512*n: EL*2 + 512*(n+1)], False, False, dt=f32r)
            mm(pb2[:], cs["wl2b"][:, 128*mt:128*(mt+1)],
               hidL[:, EL*3 + 512*n: EL*3 + 512*(n+1)], False, False, dt=f32r)
            mm(pb2[:], cs["bl2r"][:, 128*mt:128*(mt+1)], ones1[:], False, True)
            nc.scalar.activation(out=dst[:, ns], in_=pb2[:], func=Act.Copy)

    if dbg:
        nc.sync.dma_start(out=dbg["dbg_bid0"][:], in_=bid0b[:])
        nc.sync.dma_start(out=dbg["dbg_bid1"][:], in_=bid1b[:])

    # edge cdist strip units (interleaved into both sinkhorn loops)
    D = persist.tile([96, 768], f32, tag="D")
    ec_units = strip_cdist(D,
                blk_of_ph=lambda p, hh: (bid0b if hh == 0 else bid1b)[:, 192*p+96:192*p+192],
                cols_of_ph=lambda p, hh, i0, ni: (bid0b if hh == 0 else bid1b)
                    [:, 192*p+i0:192*p+i0+ni],
                dpart=128, blk=96, nacc=2, tag="ec", dt_=bf16, defer=True,
                ones_override=twos128b)

    # ---------- phase 5: node sinkhorn, log space
    la = cost
    nc.vector.tensor_scalar(out=la[:], in0=la[:], scalar1=float(-1.0/TEMP),
                            scalar2=None, op0=Alu.mult)

    def ns_norm_step(t):
        """log-space normalize along each 32-wide free block of t [32, 256]."""
        t3 = t[:].rearrange("p (b j) -> p b j", j=32)
        rm = stg.tile([32, 8], f32, tag="ns_rm")
        nc.vector.tensor_reduce(out=rm[:], in_=t3, axis=AX.X, op=Alu.max,
                                negate=True)
        tmp = stg.tile([32, 256], f32, tag="ns_tmp")
        nc.vector.scalar_tensor_tensor(
            out=tmp[:].rearrange("p (b j) -> p b j", j=32), in0=t3, scalar=1.0,
            in1=bcast_in(rm[:], 32), op0=Alu.mult, op1=Alu.add)
        ex = stg.tile([32, 256], f32, tag="ns_ex")
        nc.scalar.activation(out=ex[:], in_=tmp[:], func=Act.Exp)
        sm = stg.tile([32, 8], f32, tag="ns_sm")
        nc.vector.tensor_reduce(out=sm[:], in_=ex[:].rearrange(
            "p (b j) -> p b j", j=32), axis=AX.X, op=Alu.add)
        ls = stg.tile([32, 8], f32, tag="ns_ls")
        nc.scalar.activation(out=ls[:], in_=sm[:], func=Act.Ln)
        lse = stg.tile([32, 8], f32, tag="ns_lse")
        nc.vector.tensor_tensor(out=lse[:], in0=ls[:], in1=rm[:], op=Alu.subtract)
        nc.vector.scalar_tensor_tensor(
            out=t3, in0=t3, scalar=1.0,
            in1=bcast_in(lse[:], 32), op0=Alu.mult, op1=Alu.subtract)

    ncd = persist.tile([32, 256], f32, tag="ncd")
    hTb = persist.tile([128, 512], bf16, tag="hTb")
    nc.vector.tensor_copy(hTb[:], hT[:])
    na_units = strip_cdist(ncd,
                blk_of_ph=lambda p, hh: hTb[:, 64*p+32:64*p+64],
                cols_of_ph=lambda p, hh, i0, ni: hTb[:, 64*p+i0:64*p+i0+ni],
                dpart=128, blk=32, nacc=1, tag="na", dt_=bf16, defer=True,
                ones_override=twos128b)

    # log-domain sinkhorn (trajectory-exact vs the reference: the node plan
    # is chaotically sensitive — it has NOT converged at 20 iters, so the
    # arithmetic structure must mirror the reference's lse updates).
    lat = persist.tile([32, 256], f32, tag="lat")
    fillers = na_units + ec_units
    for it in range(ITERS):
        ns_norm_step(la)                      # row step
        nc.vector.transpose(lat[:], la[:])    # per-pair 32x32 block transpose
        for _ in range(2):
            if fillers:
                fillers.pop(0)()
        ns_norm_step(lat)                     # col step (rows of transposed)
        nc.vector.transpose(la[:], lat[:])
        if fillers:
            fillers.pop(0)()
    nplan = persist.tile([32, 256], f32, tag="nplan")
    nc.scalar.activation(out=nplan[:], in_=la[:], func=Act.Exp)
    P3 = nplan[:].rearrange("p (b j) -> p b j", j=32)
    if dbg:
        nc.sync.dma_start(out=dbg["dbg_nplan"][:], in_=nplan[:])

    # ---------- phase 6: kron -> M0 edge [96, (8p,96j)]
    Me = persist.tile([96, 768], f32, tag="Me")
    for p in range(BL):
        Pp = nplan[:, 32*p:32*(p+1)]
        put = psm.tile([32, 96], f32, tag="ps_s")
        pvt = psm.tile([32, 96], f32, tag="ps_s")
        mm(put[:], Pp, cs["kfq"][:, 96*p:96*(p+1)], True, True)
        mm(pvt[:], Pp, cs["ktq"][:, 96*p:96*(p+1)], True, True)
        ut = stg.tile([32, 96], f32, tag="kr_ut")
        vt = stg.tile([32, 96], f32, tag="kr_vt")
        nc.vector.tensor_copy(ut[:], put[:])
        nc.vector.tensor_copy(vt[:], pvt[:])
        pA = psm.tile([96, 96], f32, tag="ps_s")
        pB = psm.tile([96, 96], f32, tag="ps_s")
        mm(pA[:], ut[:], cs["kfc"][:, 96*p:96*(p+1)], True, True)
        mm(pB[:], vt[:], cs["ktc"][:, 96*p:96*(p+1)], True, True)
        sA = stg.tile([96, 96], f32, tag="kr_sA")
        nc.scalar.activation(out=sA[:], in_=pA[:], func=Act.Copy)
        straight = stg.tile([96, 96], f32, tag="kr_str")
        nc.vector.tensor_tensor(out=straight[:], in0=sA[:], in1=pB[:], op=Alu.mult)
        pC = psm.tile([96, 96], f32, tag="ps_s")
        pD = psm.tile([96, 96], f32, tag="ps_s")
        mm(pC[:], ut[:], cs["ktc"][:, 96*p:96*(p+1)], True, True)
        mm(pD[:], vt[:], cs["kfc"][:, 96*p:96*(p+1)], True, True)
        sC = stg.tile([96, 96], f32, tag="kr_sC")
        nc.scalar.activation(out=sC[:], in_=pC[:], func=Act.Copy)
        cross = stg.tile([96, 96], f32, tag="kr_crs")
        nc.vector.tensor_tensor(out=cross[:], in0=sC[:], in1=pD[:], op=Alu.mult)
        gmax = stg.tile([96, 96], f32, tag="kr_gmax")
        nc.vector.tensor_tensor(out=gmax[:], in0=straight[:], in1=cross[:],
                                op=Alu.max)
        nc.scalar.activation(out=Me[:, 96*p:96*(p+1)], in_=gmax[:], func=Act.Exp,
                             scale=float(1.0/TEMP))
    if dbg:
        nc.sync.dma_start(out=dbg["dbg_M0"][:], in_=Me[:])

    # ---------- phase 7: edge sinkhorn (multiplicative) + interleaved cdist
    # Lazy row normalization: stored Me is only col-normalized; the current
    # row factors rr (= 1/rowsum(Me)) are folded into the colsum stationary
    # each iteration and into the final plan*D dot, saving one [96,768]
    # DVE pass per iteration.
    Me3 = Me[:].rearrange("p (b j) -> p b j", j=96)
    rr = persist.tile([96, 8], f32, tag="es_rr")
    for it in range(ITERS):
        rs = stg.tile([96, 8], f32, tag="es_rs")
        nc.vector.tensor_reduce(out=rs[:], in_=Me3, axis=AX.X, op=Alu.add)
        nc.vector.reciprocal(out=rr[:], in_=rs[:])
        rc = stg1.tile([96, 768], f32, tag="big768")
        pcs_l = []
        for hh in range(2):
            pcs = psm.tile([96, 384], f32, tag="ps_s")
            for q in range(4):
                pp = 4*hh + q
                mm(pcs[:, 96*q:96*(q+1)], rr[:, pp:pp+1].to_broadcast((96, 96)),
                   Me[:, 96*pp:96*(pp+1)], True, True)
            pcs_l.append(pcs)
        for _ in range(2):
            if fillers:
                fillers.pop(0)()
        for hh in range(2):
            nc.vector.reciprocal(out=rc[:, 384*hh:384*(hh+1)],
                                 in_=pcs_l[hh][:])
        nc.vector.tensor_tensor(out=Me[:], in0=Me[:], in1=rc[:], op=Alu.mult)
    while fillers:
        fillers.pop(0)()
    if dbg:
        nc.vector.scalar_tensor_tensor(
            out=Me3, in0=Me3, scalar=1.0, in1=bcast_in(rr[:], 96),
            op0=Alu.mult, op1=Alu.mult)
        nc.sync.dma_start(out=dbg["dbg_eplan"][:], in_=Me[:])
        nc.sync.dma_start(out=dbg["dbg_D"][:], in_=D[:])


    # (ncd computed interleaved with node sinkhorn above)
    if dbg:
        nc.sync.dma_start(out=dbg["dbg_ncd"][:], in_=ncd[:])

    # ---------- phase 11: dots + output (with max-trick rank-1 corrections)
    # edge_align_p = sum_ij EP*(2S_e) - sum_j csEP_j*sbe_j - sum_i rsEP_i*sce_i
    # with csEP_j = 1 exactly (final step is an exact column normalize).
    # node_align_p likewise with csP_j = 1.
    #
    # column-layout q-side feature sums: sce_col[i,p] (edges), scn_col[i,p]
    # (nodes) via 1-row matmuls vs a ones column.
    psce = psm.tile([96, 8], f32, tag="ps_s")
    for p in range(BL):
        mm(psce[:, p:p+1], bid0b[:, 192*p:192*p+96], ones128b[:], True, False)
        mm(psce[:, p:p+1], bid1b[:, 192*p:192*p+96], ones128b[:], False, True)
    sce = stg.tile([96, 8], f32, tag="dot_sce")
    nc.vector.tensor_copy(sce[:], psce[:])
    pscn = psm.tile([32, 8], f32, tag="ps_s")
    for p in range(BL):
        mm(pscn[:, p:p+1], hTb[:, 64*p:64*p+32], ones128b[:], True, True)
    scn = stg.tile([32, 8], f32, tag="dot_scn")
    nc.vector.tensor_copy(scn[:], pscn[:])
    # row-layout c-side feature sums -> [1, 8] totals (csEP = csP = 1)
    hTb4 = hTb[:].rearrange("p (g x n) -> p g x n", x=2, n=32)
    bid04 = bid0b[:].rearrange("p (g x n) -> p g x n", x=2, n=96)
    bid14 = bid1b[:].rearrange("p (g x n) -> p g x n", x=2, n=96)
    pn_row = psm.tile([1, 256], f32, tag="ps_s")
    mm(pn_row[:], ones128b[:], hTb4[:, :, 1, :], True, True)
    pe_row0 = psm.tile([1, 384], f32, tag="ps_s")
    mm(pe_row0[:], ones128b[:], bid04[:, 0:4, 1, :], True, False)
    mm(pe_row0[:], ones128b[:], bid14[:, 0:4, 1, :], False, True)
    pe_row1 = psm.tile([1, 384], f32, tag="ps_s")
    mm(pe_row1[:], ones128b[:], bid04[:, 4:8, 1, :], True, False)
    mm(pe_row1[:], ones128b[:], bid14[:, 4:8, 1, :], False, True)
    tA = stg.tile([1, 16], f32, tag="dot_tA")
    nc.vector.tensor_reduce(out=tA[:, :8], in_=pn_row[:].rearrange(
        "p (b j) -> p b j", j=32), axis=AX.X, op=Alu.add)
    nc.vector.tensor_reduce(out=tA[:, 8:12], in_=pe_row0[:].rearrange(
        "p (b j) -> p b j", j=96), axis=AX.X, op=Alu.add)
    nc.vector.tensor_reduce(out=tA[:, 12:16], in_=pe_row1[:].rearrange(
        "p (b j) -> p b j", j=96), axis=AX.X, op=Alu.add)
    tAall = stg.tile([1, 8], f32, tag="dot_tAall")
    nc.vector.tensor_tensor(out=tAall[:], in0=tA[:, :8], in1=tA[:, 8:],
                            op=Alu.add)
    # plan row sums (post final col-normalize)
    rse = stg.tile([96, 8], f32, tag="dot_rse")
    nc.vector.tensor_reduce(out=rse[:], in_=Me3, axis=AX.X, op=Alu.add)
    nc.vector.tensor_tensor(out=rse[:], in0=rse[:], in1=rr[:], op=Alu.mult)
    nc.vector.tensor_tensor(out=rse[:], in0=rse[:], in1=sce[:], op=Alu.mult)
    rsn = stg.tile([32, 8], f32, tag="dot_rsn")
    nc.vector.tensor_reduce(out=rsn[:], in_=P3, axis=AX.X, op=Alu.add)
    nc.vector.tensor_tensor(out=rsn[:], in0=rsn[:], in1=scn[:], op=Alu.mult)
    # main plan (.) 2S dots
    we = stg1.tile([96, 768], f32, tag="big768")
    nc.vector.tensor_tensor(out=we[:], in0=Me[:], in1=D[:], op=Alu.mult)
    ep = stg.tile([96, 8], f32, tag="dot_ep")
    nc.vector.tensor_reduce(out=ep[:], in_=we[:].rearrange(
        "p (b j) -> p b j", j=96), axis=AX.X, op=Alu.add)
    nc.vector.tensor_tensor(out=ep[:], in0=ep[:], in1=rr[:], op=Alu.mult)
    nc.vector.tensor_tensor(out=ep[:], in0=ep[:], in1=rse[:], op=Alu.subtract)
    wn = stg.tile([32, 256], f32, tag="dot_wn")
    nc.vector.tensor_tensor(out=wn[:], in0=nplan[:], in1=ncd[:], op=Alu.mult)
    np_ = stg.tile([32, 8], f32, tag="dot_np")
    nc.vector.tensor_reduce(out=np_[:], in_=wn[:].rearrange(
        "p (b j) -> p b j", j=32), axis=AX.X, op=Alu.add)
    nc.vector.tensor_tensor(out=np_[:], in0=np_[:], in1=rsn[:], op=Alu.subtract)
    pout = psm.tile([1, 8], f32, tag="ps_s")
    mm(pout[:], ones96sq[:, 0:1], ep[:], True, False)
    mm(pout[:], ones32f[:], np_[:], False, False)
    mm(pout[:], negones11[:], tAall[:], False, True)
    osb = stg.tile([1, 8], f32, tag="osb")
    nc.vector.tensor_copy(osb[:], pout[:])
    nc.sync.dma_start(out=out_ext[:], in_=osb[:])

    ctx.close()


# ----------------------------------------------------------------- entry
def _get_nc(debug=False):
    key = ("nc", debug)
    if key not in _CACHE:
        _CACHE[key] = _build(debug=debug)
    return _CACHE[key]


def run_cores(inputs, debug=False, trace=False):
    from concourse.bass_utils import run_bass_kernel_spmd
    nc = _get_nc(debug=debug)
    in_maps = _host_prep(inputs)
    res = run_bass_kernel_spmd(nc, in_maps, core_ids=list(range(NCORE)),
                               trace=trace)
    return res


def kernel(**inputs):
    res = run_cores(inputs, debug=False, trace=False)
    out = np.concatenate([r["out"].reshape(-1) for r in res.results])
    return out.astype(np.float32)

